# revision 81
# baseline (speedup 1.0000x reference)
"""Causal cross-attention kernel for 8 trn2 NeuronCores.

Sharding: 4-way data-parallel over batch x 2-way tensor-parallel over heads
(8 heads per core).  Per core:
  - Q/K/V/O projections run as fp8e4m3 DoubleRow matmuls (2 k-subtiles per
    instruction at 0.5 cyc/row) with an error-compensated hi/lo split:
    x = xh + xl and w = wh + wl quantized at scale 32, and the three
    products xh*wh + xl*wh + xh*wl accumulate at a common PSUM scale of
    1024, so projection error is below bf16 rounding at 0.75x bf16 cost.
  - Scores are a single one-sided DoubleRow matmul per (head, k-block):
    kT is stored as an exact fp8 hi/lo pair, qT as single fp8, with q
    broadcast across both subtiles (2x bf16 throughput, ~1.5e-2 rel err).
  - Attention in transposed layout: scores^T[k,q] -> exp on ACT (a slice of
    late-q-tile exps uses a bf16 Schraudolph fast-exp on DVE to unload
    ACT) -> stationary-P^T PV step with the 65-wide bf16 V-aug (ones
    column accumulates the softmax denominator).  Normalized O blocks are
    DMA-transposed and split into fp8 hi/lo for the DoubleRow outproj.
  - Orchestration is a fine-grained pull model: scores (hp, qt, kb) pull
    exactly Qproj(qt, hp)/Kproj(kb//4, hp) just in time; PV units emit
    inline one k-block behind the score stream (trailing blocks spill into
    the next group); remaining projection and outproj work is debt-paced
    PE filler between score emissions.

All host-side work (transposes, hi/lo packing) is data marshaling; the
device kernel is a single NEFF launch per core.
"""

import sys

sys.path.insert(0, "/opt/trn_rl_repo")

import numpy as np

import concourse.bass as bass
import concourse.tile as tile
from concourse import bacc, mybir
from concourse.bass import ts
from concourse.masks import make_upper_triangular

F32 = mybir.dt.float32
F32R = mybir.dt.float32r
BF16 = mybir.dt.bfloat16
FP8 = mybir.dt.float8e4
I16 = mybir.dt.int16
P = 128

# full-problem constants
B_FULL = 4
S_FULL = 2048
D_FULL = 1024
HG_FULL = 8  # heads per core (16 heads / 2-way TP)
N_CORES = 8


def build_bass(S=S_FULL, D=D_FULL, HG=HG_FULL):
    """One-core program; SPMD across 8 cores with different data."""
    GO = HG * 64  # output-feature width of this core's head group
    ND = D // P  # d-blocks (contraction)
    NM = GO // P  # o-tiles of Q/K projections
    NQT = S // 512  # q-tiles (512 wide)
    NTB = S // P  # token blocks of 128
    TCH = 512  # projection t-chunk (one q-tile per chunk)
    NCH = S // TCH

    ND2 = ND // 2  # d-block pairs for fp8 DoubleRow
    # fp8 hi/lo packed inputs: sub-index s = 4*j + 2*hl + i selects
    # (d-pair j, hi/lo, d-block within pair); value = e4m3 of 32*x (hi)
    # or 32*x - hi (lo).  PSUM accumulates at scale 1024.
    nc = bacc.Bacc("TRN2", target_bir_lowering=False, debug=False)
    xq8 = nc.dram_tensor("xq8", [P, 4 * ND2, S], FP8, kind="ExternalInput")
    xkv8 = nc.dram_tensor("xkv8", [P, 4 * ND2, S], FP8, kind="ExternalInput")
    wq8 = nc.dram_tensor("wq8", [P, 4 * ND2, GO], FP8, kind="ExternalInput")
    wk8 = nc.dram_tensor("wk8", [P, 4 * ND2, GO], FP8, kind="ExternalInput")
    wv8 = nc.dram_tensor("wv8", [P, 4 * ND2, GO], FP8, kind="ExternalInput")
    wo8 = nc.dram_tensor("wo8", [P, 4 * (GO // P // 2), D], FP8, kind="ExternalInput")
    y = nc.dram_tensor("y", [S, D], F32, kind="ExternalOutput")

    Exp = mybir.ActivationFunctionType.Exp
    Mult = mybir.AluOpType.mult
    Add = mybir.AluOpType.add
    DR = mybir.MatmulPerfMode.DoubleRow
    EXPSCALE = 0.125 / 1024.0  # scores psum = (32q).(32k) = 1024*s
    # bf16 Schraudolph fast-exp for the Pool engine: int16(x*A + B) bitcast
    # to bf16 ~= exp(x) within ~3.6%; softmax renormalization absorbs most
    # of the sawtooth.  A fraction of exp tiles go to Pool to unload ACT.
    SCH_A = 128.0 * 1.4426950408889634
    SCH_B = 16251.0
    SCH_MOD = 6  # 1/6 of late-q-tile exps take the fast-exp path

    with tile.TileContext(nc) as tc:
        from contextlib import ExitStack

        with ExitStack() as ctx:
            ctx.enter_context(
                nc.allow_low_precision(reason="bf16/fp32r matmul input rounding")
            )
            # ---- persistent SBUF buffers ----
            pers = ctx.enter_context(tc.tile_pool(name="pers", bufs=1))
            # qT: e4m3 at scale 32; kT: [hi, lo] e4m3 pair at scale 32
            qT = [pers.tile([P, S], FP8, tag=f"qT{i}", name=f"qT{i}") for i in range(NM)]
            kT = [pers.tile([P, 2, S], FP8, tag=f"kT{i}", name=f"kT{i}") for i in range(NM)]
            vaug = [pers.tile([P, HG * 65], BF16, tag=f"va{i}", name=f"va{i}") for i in range(NTB)]
            # attention output in fp8 hi/lo pairs per ob-pair jj for the
            # DoubleRow output projection; oT blocks are transient
            oh8 = [pers.tile([P, 2, S], FP8, tag=f"oh{j}", name=f"oh{j}") for j in range(NM // 2)]
            ol8 = [pers.tile([P, 2, S], FP8, tag=f"ol{j}", name=f"ol{j}") for j in range(NM // 2)]
            consts = ctx.enter_context(tc.tile_pool(name="consts", bufs=1))
            tri_f = consts.tile([P, P], F32)  # tri[k,q] = 1 if q >= k else 0
            make_upper_triangular(nc, tri_f[:], val=1.0, diag=True)
            # two side-by-side bf16 copies so one strided op masks 2 heads
            tri2 = consts.tile([P, 2 * P], BF16)
            nc.vector.tensor_copy(tri2[:, 0:P], tri_f[:])
            nc.vector.tensor_copy(tri2[:, P : 2 * P], tri_f[:])
            for i in range(NTB):
                # ones columns survive the V evictions (cols h*65+64)
                nc.gpsimd.memset(vaug[i][:], 1.0)

            w_pool = ctx.enter_context(tc.tile_pool(name="wp", bufs=1))
            x_pool = ctx.enter_context(tc.tile_pool(name="xp", bufs=2))
            big_pool = ctx.enter_context(tc.tile_pool(name="big", bufs=2, space="PSUM"))
            spool = ctx.enter_context(tc.tile_pool(name="ps_s", bufs=2, space="PSUM"))
            opool = ctx.enter_context(tc.tile_pool(name="ps_o", bufs=2, space="PSUM"))
            apool = ctx.enter_context(tc.tile_pool(name="att", bufs=2))
            apool2 = ctx.enter_context(tc.tile_pool(name="attn2", bufs=2))
            opool_sb = ctx.enter_context(tc.tile_pool(name="osb", bufs=2))
            y_pool = ctx.enter_context(tc.tile_pool(name="yev", bufs=3))

            wq_t = w_pool.tile([P, 4 * ND2, GO], FP8, tag="wq", name="wq")
            wk_t = w_pool.tile([P, 4 * ND2, GO], FP8, tag="wk", name="wk")
            wv_t = w_pool.tile([P, 4 * ND2, GO], FP8, tag="wv", name="wv")
            wo_t = w_pool.tile([P, 4 * (NM // 2), D], FP8, tag="wo", name="wo")
            ot_pool = ctx.enter_context(tc.tile_pool(name="otp", bufs=2))
            def emit_w_dmas():
                # critical path first: wq then wk feed the first score tile
                for j in range(1, ND2):
                    nc.sync.dma_start(wq_t[:, 4 * j : 4 * j + 4, :], wq8[:, 4 * j : 4 * j + 4, :])
                for j in range(ND2):
                    nc.sync.dma_start(wk_t[:, 4 * j : 4 * j + 4, :], wk8[:, 4 * j : 4 * j + 4, :])

            def emit_w_dmas_late():
                for j in range(ND2):
                    nc.sync.dma_start(wv_t[:, 4 * j : 4 * j + 4, :], wv8[:, 4 * j : 4 * j + 4, :])
                for j in range(NM // 2):
                    nc.sync.dma_start(wo_t[:, 4 * j : 4 * j + 4, :], wo8[:, 4 * j : 4 * j + 4, :])

            # ---------------- emitter units ----------------
            xq_tiles = {}
            xkv_tiles = {}
            ysb_tiles = {}
            x_dmas_done = set()

            def emit_x_dma(tc_i):
                x_dmas_done.add(tc_i)
                xq = x_pool.tile([P, 4 * ND2, TCH], FP8, tag="xq", name=f"xq_{tc_i}")
                xkv = x_pool.tile([P, 4 * ND2, TCH], FP8, tag="xk", name=f"xkv_{tc_i}")
                for h in range(2):
                    sl = slice(8 * h, 8 * h + 8)
                    nc.sync.dma_start(xq[:, sl, :], xq8[:, sl, ts(tc_i, TCH)])
                    nc.sync.dma_start(xkv[:, sl, :], xkv8[:, sl, ts(tc_i, TCH)])
                xq_tiles[tc_i] = xq
                xkv_tiles[tc_i] = xkv

            def dr3(ps, wt, xt, mcols, j, first, last):
                # 3-term error-compensated fp8 DoubleRow: hi*hi + hi*lo + lo*hi
                wh = wt[:, 4 * j : 4 * j + 2, mcols]
                wl = wt[:, 4 * j + 2 : 4 * j + 4, mcols]
                xh = xt[:, 4 * j : 4 * j + 2, :]
                xl = xt[:, 4 * j + 2 : 4 * j + 4, :]
                nc.tensor.matmul(ps, wh, xh, start=first, stop=False, perf_mode=DR)
                nc.tensor.matmul(ps, wh, xl, start=False, stop=False, perf_mode=DR)
                nc.tensor.matmul(ps, wl, xh, start=False, stop=last, perf_mode=DR)

            def emit_proj_q_m(tc_i, m):
                xq = xq_tiles[tc_i]
                ps = big_pool.tile([P, 512], F32, tag="big", name=f"pq{tc_i}_{m}")
                for j in range(ND2):
                    dr3(ps[:, 0:TCH], wq_t, xq, ts(m, P), j, j == 0, j == ND2 - 1)
                # evict 1024q -> e4m3(32q)
                nc.vector.tensor_scalar_mul(qT[m][:, ts(tc_i, TCH)], ps[:, 0:TCH], 1.0 / 32.0)

            def emit_proj_k_m(tc_i, m):
                xkv = xkv_tiles[tc_i]
                ps = big_pool.tile([P, 512], F32, tag="big", name=f"pk{tc_i}_{m}")
                for j in range(ND2):
                    dr3(ps[:, 0:TCH], wk_t, xkv, ts(m, P), j, j == 0, j == ND2 - 1)
                # evict 1024k -> hi = e4m3(32k), lo = e4m3(32k - hi)
                nc.vector.tensor_scalar_mul(kT[m][:, 0, ts(tc_i, TCH)], ps[:, 0:TCH], 1.0 / 32.0)
                nc.vector.scalar_tensor_tensor(
                    kT[m][:, 1, ts(tc_i, TCH)],
                    ps[:, 0:TCH],
                    1.0 / 32.0,
                    kT[m][:, 0, ts(tc_i, TCH)],
                    Mult,
                    mybir.AluOpType.subtract,
                )

            def emit_proj_v_mt(tc_i, mt):
                xkv = xkv_tiles[tc_i]
                ps = big_pool.tile([P, 512], F32, tag="big", name=f"pv{tc_i}_{mt}")
                for j in range(ND2):
                    xh = xkv[:, 4 * j : 4 * j + 2, ts(mt, P)]
                    xl = xkv[:, 4 * j + 2 : 4 * j + 4, ts(mt, P)]
                    wh = wv_t[:, 4 * j : 4 * j + 2, :]
                    wl = wv_t[:, 4 * j + 2 : 4 * j + 4, :]
                    nc.tensor.matmul(ps[:], xh, wh, start=(j == 0), stop=False, perf_mode=DR)
                    nc.tensor.matmul(ps[:], xl, wh, start=False, stop=False, perf_mode=DR)
                    nc.tensor.matmul(ps[:], xh, wl, start=False, stop=(j == ND2 - 1), perf_mode=DR)
                vt = vaug[tc_i * (TCH // P) + mt]
                nc.vector.tensor_copy(
                    vt[:].rearrange("p (h c) -> p h c", c=65)[:, :, 0:64],
                    ps[:].rearrange("p (h c) -> p h c", c=64),
                )

            def emit_proj_q(tc_i):
                for m in range(NM):
                    emit_proj_q_m(tc_i, m)

            def emit_proj_k(tc_i):
                for m in range(NM):
                    emit_proj_k_m(tc_i, m)

            def emit_proj_v(tc_i):
                for mt in range(TCH // P):
                    emit_proj_v_mt(tc_i, mt)
                del xq_tiles[tc_i], xkv_tiles[tc_i]

            def emit_outproj_nt(mt, nt):
                # fp8 DoubleRow 3-term: psum = 1024*y over ob-pairs jj
                ps = big_pool.tile([P, 512], F32, tag="big", name=f"y{mt}_{nt}")
                NJ = NM // 2
                for jj in range(NJ):
                    oh = oh8[jj][:, :, ts(mt, P)]
                    ol = ol8[jj][:, :, ts(mt, P)]
                    wh = wo_t[:, 4 * jj : 4 * jj + 2, ts(nt, 512)]
                    wl = wo_t[:, 4 * jj + 2 : 4 * jj + 4, ts(nt, 512)]
                    nc.tensor.matmul(ps[:], oh, wh, start=(jj == 0), stop=False, perf_mode=DR)
                    nc.tensor.matmul(ps[:], ol, wh, start=False, stop=False, perf_mode=DR)
                    nc.tensor.matmul(ps[:], oh, wl, start=False, stop=(jj == NJ - 1), perf_mode=DR)
                ysb = y_pool.tile([P, 512], F32, tag="ysb", name=f"ysb{mt}_{nt}")
                nc.vector.tensor_scalar_mul(ysb[:], ps[:], 1.0 / 1024.0)
                nc.sync.dma_start(y[ts(mt, P), ts(nt, 512)], ysb[:])

            def emit_score_kb(hp, qt, kb):
                j = kb - 4 * qt
                ce = max(j, 0) * P
                # both heads' scores in one 2-bank PSUM tile so a single
                # strided activation does both exps
                pss = spool.tile([P, 1024], F32, tag="s", name=f"s{hp}_{qt}_{kb}")
                w1 = 512 - ce
                for g, po in ((0, 0), (1, 64)):
                    # one fp8 DoubleRow matmul: (kh + kl) . q, q broadcast
                    qv = (
                        qT[hp][po : po + 64, qt * 512 + ce : (qt + 1) * 512]
                        .unsqueeze(1)
                        .broadcast_to([64, 2, w1])
                    )
                    nc.tensor.matmul(
                        pss[:, g * 512 + ce : (g + 1) * 512],
                        kT[hp][po : po + 64, :, ts(kb, P)],
                        qv,
                        start=True,
                        stop=True,
                        perf_mode=DR,
                    )
                pexp = apool.tile([P, 1024], BF16, tag=f"p{kb}", name=f"p{hp}_{qt}_{kb}")
                out_ap = pexp[:].rearrange("p (g c) -> p g c", g=2)[:, :, ce:]
                in_ap = pss[:].rearrange("p (g c) -> p g c", g=2)[:, :, ce:]
                if qt >= 2 and (kb + hp) % SCH_MOD == 0:
                    # DVE fast-exp, only in the ACT-bound late q-tiles
                    nc.vector.tensor_scalar(
                        out_ap.bitcast(I16), in_ap, SCH_A * EXPSCALE, SCH_B, Mult, Add
                    )
                else:
                    nc.scalar.activation(out_ap, in_ap, Exp, scale=EXPSCALE)
                if j >= 0:
                    # mask the boundary block for both heads in one op
                    nc.gpsimd.tensor_tensor(
                        pexp[:].rearrange("p (g c) -> p g c", g=2)[:, :, ts(j, P)],
                        pexp[:].rearrange("p (g c) -> p g c", g=2)[:, :, ts(j, P)],
                        tri2[:].rearrange("p (g c) -> p g c", g=2),
                        Mult,
                    )
                return pexp



            def make_pv_unit(hp, qt, qb, pexps, osb, otq):
                def fn():
                    # both heads' PV accumulation groups, sequentially, into
                    # one [128, 130] PSUM tile (cols h*65+64 = denominators);
                    # each group runs start-to-stop before the next opens
                    # (2KB PSUM zero-region rule).
                    pv = opool.tile([P, 130], F32, tag="pv", name=f"pv{hp}_{qt}_{qb}")
                    for g, hh in ((0, 2 * hp), (1, 2 * hp + 1)):
                        for kb in range(4 * qt + qb + 1):
                            nc.tensor.matmul(
                                pv[:, g * 65 : g * 65 + 65],
                                pexps[kb][:, g * 512 + qb * P : g * 512 + (qb + 1) * P],
                                vaug[kb][:, hh * 65 : hh * 65 + 65],
                                start=(kb == 0),
                                stop=(kb == 4 * qt + qb),
                            )
                    rec = apool2.tile([P, 2], F32, tag="rec", name=f"rec{hp}_{qt}_{qb}")
                    nc.vector.reciprocal(
                        rec[:].rearrange("p (g c) -> p g c", c=1),
                        pv[:].rearrange("p (g c) -> p g c", c=65)[:, :, 64:65],
                    )
                    for g in (0, 1):
                        # pv holds sum(p * 1024*v); rescale by 1/1024 here
                        nc.vector.tensor_scalar(
                            osb[:, g * 64 : (g + 1) * 64],
                            pv[:, g * 65 : g * 65 + 64],
                            rec[:, g : g + 1],
                            1.0 / 1024.0,
                            Mult,
                            Mult,
                        )
                    # one DMA-transpose moves both heads' normalized O[q, dh]
                    # block into O^T[dh, q] inside the per-(hp,qt) staging
                    # tile; after the last block, DVE splits the 512-wide
                    # strip into the fp8 hi/lo pair the outproj consumes
                    nc.sync.dma_start_transpose(otq[:, ts(qb, P)], osb[:])
                    if qt == NQT - 1:
                        # last q-tile: split per block so outproj token
                        # blocks unlock as early as possible (short tail)
                        blk = slice((4 * qt + qb) * P, (4 * qt + qb + 1) * P)
                        ohs = oh8[hp // 2][:, hp % 2, blk]
                        nc.vector.tensor_scalar_mul(ohs, otq[:, ts(qb, P)], 32.0)
                        nc.vector.scalar_tensor_tensor(
                            ol8[hp // 2][:, hp % 2, blk],
                            otq[:, ts(qb, P)],
                            32.0,
                            ohs,
                            Mult,
                            mybir.AluOpType.subtract,
                        )
                        attn_cnt[4 * qt + qb] += 1
                    elif qb == 3:
                        blk = slice(qt * 512, (qt + 1) * 512)
                        ohs = oh8[hp // 2][:, hp % 2, blk]
                        nc.vector.tensor_scalar_mul(ohs, otq[:], 32.0)
                        nc.vector.scalar_tensor_tensor(
                            ol8[hp // 2][:, hp % 2, blk],
                            otq[:],
                            32.0,
                            ohs,
                            Mult,
                            mybir.AluOpType.subtract,
                        )
                        for i in range(4):
                            attn_cnt[4 * qt + i] += 1

                return (2 * (4 * qt + qb + 1) * 65 * 0.42 + 120, fn)

            # ---------------- orchestration ----------------
            # Fine-grained pull model: scores (hp, qt, kb) pull exactly
            # Qproj(qt, hp) and Kproj(kb//4, hp) just in time, so exp work
            # flows to ACT as early as the data allows.  Remaining proj
            # units, PV units and outproj blocks are debt-paced PE filler.
            NVT = TCH // P  # V-proj token-blocks per chunk
            q_done = [[False] * NM for _ in range(NCH)]
            k_done = [[False] * NM for _ in range(NCH)]
            v_done = [[False] * NVT for _ in range(NCH)]

            def chunk_all_done(c):
                return all(q_done[c]) and all(k_done[c]) and all(v_done[c])

            def ensure_x(c):
                if c not in x_dmas_done:
                    # tile-slot hazard: chunk c's x DMA reuses chunk c-2's
                    # buffers; all chunk c-2 readers must be emitted first
                    if c >= 2 and not chunk_all_done(c - 2):
                        pull_chunk(c - 2)
                    emit_x_dma(c)

            def pull_q(c, m):
                if not q_done[c][m]:
                    q_done[c][m] = True
                    ensure_x(c)
                    emit_proj_q_m(c, m)
                    return 1280
                return 0

            def pull_k(c, m):
                if not k_done[c][m]:
                    k_done[c][m] = True
                    ensure_x(c)
                    emit_proj_k_m(c, m)
                    return 1280
                return 0

            def pull_v(c, mt):
                if not v_done[c][mt]:
                    v_done[c][mt] = True
                    ensure_x(c)
                    emit_proj_v_mt(c, mt)
                    return 1280
                return 0

            def pull_chunk(c):
                for m in range(NM):
                    pull_q(c, m)
                for m in range(NM):
                    pull_k(c, m)
                for mt in range(NVT):
                    pull_v(c, mt)

            # workq: debt-paced PE filler (cost_ns, fn); fn returns actual
            # cost (0 if the unit was already pulled directly)
            workq = []
            ogate = {}
            attn_cnt = [0] * NTB  # per token block: heads with split done

            def queue_fillers():
                for c in range(NCH):
                    for m in range(NM):
                        workq.append((f"c{c}", 1280, lambda c=c, m=m: pull_q(c, m)))
                    for m in range(NM):
                        workq.append((f"c{c}", 1280, lambda c=c, m=m: pull_k(c, m)))
                    for mt in range(NVT):
                        workq.append((f"c{c}", 1280, lambda c=c, mt=mt: pull_v(c, mt)))
                for mt in range(NTB):
                    for nt in range(D // 512):
                        def fo(mt=mt, nt=nt):
                            emit_outproj_nt(mt, nt)
                            return 853

                        ogate[id(fo)] = mt
                        workq.append(("o", 640, fo))

            def pop_work(budget_ns):
                spent = 0.0
                i = 0
                while i < len(workq) and spent < budget_ns:
                    kind, cost, fn = workq[i]
                    if kind == "o" and attn_cnt[ogate[id(fn)]] < HG // 2:
                        i += 1
                        continue
                    r = fn()
                    spent += cost if r is None else r
                    workq.pop(i)
                return spent

            # critical-path DMAs first: wq, x chunk 0, wk feed the first
            # scores; wv/wo and chunk 1 follow
            nc.sync.dma_start(wq_t[:, 0:4, :], wq8[:, 0:4, :])
            emit_x_dma(0)
            emit_w_dmas()
            emit_w_dmas_late()
            emit_x_dma(1)
            queue_fillers()
            # PE prewarm: dummy matmuls on the tri constant ramp the tensor
            # engine to full clock while the first weight/x DMAs land
            pwt = big_pool.tile([P, 512], F32, tag="big", name="prewarm")
            for _ in range(36):
                nc.tensor.matmul(
                    pwt[:, 0:P], tri2[:, 0:P], tri2[:, 0:P], start=True, stop=True
                )

            # debt-paced weave: pop a PE filler unit only once the consumer
            # engine's exp backlog exceeds its cost, so the PE stays just
            # behind ACT/DVE.  PV units are emitted inline, one k-block
            # behind the score stream, so nothing drains at the end.
            debt = 0.0
            deferred_pv = []
            groups = [(qt, hp) for qt in range(NQT) for hp in range(HG // 2)]
            for gi, (qt, hp) in enumerate(groups):
                if hp == 0 and qt + 1 < NCH:
                    ensure_x(qt + 1)
                pull_q(qt, hp)
                nkb = 4 * qt + 4
                pexps = []
                osbs = [
                    opool_sb.tile([P, P], BF16, tag=f"osb{qb}", name=f"osb{hp}_{qt}_{qb}")
                    for qb in range(4)
                ]
                otq = ot_pool.tile([P, 512], BF16, tag=f"ot{hp}", name=f"ot{hp}_{qt}")

                def emit_pv(qb, qt=qt, hp=hp, pexps=pexps, osbs=osbs, otq=otq):
                    # vaug writes must be emitted before the PV reads them
                    for j in range(4 * qt + qb + 1):
                        pull_v(j // 4, j % 4)
                    cost, fn = make_pv_unit(hp, qt, qb, pexps, osbs[qb], otq)
                    fn()
                    return cost

                last = gi == len(groups) - 1
                for kb in range(nkb):
                    pull_k(kb // 4, hp)
                    pexps.append(emit_score_kb(hp, qt, kb))
                    if kb == 1 and not last:
                        # prefetch the next group's Q/K so its first score
                        # fires the moment this group's exps are drained
                        nqt, nhp = groups[gi + 1]
                        pull_q(nqt, nhp)
                        pull_k(0, nhp)
                    if kb <= 1 and deferred_pv:
                        # previous group's trailing PV blocks: their exps are
                        # long done, so no PE stall and no ACT gap
                        debt -= deferred_pv.pop(0)()
                    w = 2 * (512 - max(kb - 4 * qt, 0) * P)
                    if qt >= 2 and (kb + hp) % SCH_MOD == 0:
                        debt += (w * 1.04 + 170) - (w * 0.21 + 10)
                    else:
                        debt += (w * 0.833 + 242) - (w * 0.21 + 10)
                    if last:
                        continue  # emit the final scores back-to-back
                    qb = kb - 4 * qt - 2
                    if 0 <= qb <= 1:
                        debt -= emit_pv(qb)
                    npops = 0
                    cap = 2
                    while workq and npops < cap:
                        # prefer proj units; spend outproj units only when
                        # nothing else is ready (saves them for the
                        # filler-starved late q-tiles)
                        pick = None
                        for i, (kind, cost, fn) in enumerate(workq):
                            if kind == "o":
                                continue
                            pick = i
                            break
                        if pick is None:
                            for i, (kind, cost, fn) in enumerate(workq):
                                if kind == "o" and attn_cnt[ogate[id(fn)]] >= HG // 2:
                                    pick = i
                                    break
                        if pick is None or (
                            workq[pick][1] > debt
                            and not (last and workq[pick][0] == "o")
                        ):
                            break
                        kind, cost, fn = workq.pop(pick)
                        r = fn()
                        debt -= r if r is not None else cost
                        npops += 1
                if not last:
                    deferred_pv.append(lambda e=emit_pv: e(2))
                    deferred_pv.append(lambda e=emit_pv: e(3))
                else:
                    # tail: PV per block, then its outproj immediately
                    for qb in range(4):
                        emit_pv(qb)
                        pop_work(1 << 30)
            while workq:
                pop_work(1 << 30)
    nc.finalize()
    return nc


_NC_CACHE = {}


def _get_nc():
    if "full" not in _NC_CACHE:
        _NC_CACHE["full"] = build_bass()
    return _NC_CACHE["full"]


def _pack_hilo(mT):
    """[D, C] fp32 -> [128, 4*ND2, C] fp8 hi/lo pack at scale 32.

    sub-index s = 4*j + 2*hl + i: (d-pair j, hi/lo, block i); value
    hi = e4m3(32*x), lo = e4m3(32*x - hi).
    """
    import ml_dtypes

    e4 = ml_dtypes.float8_e4m3
    D, C = mT.shape
    nd = D // P
    nd2 = nd // 2
    blocks = mT.reshape(nd2, 2, P, C)  # [j, i, p, c]
    hi = (32.0 * blocks).astype(e4)
    lo = (32.0 * blocks - hi.astype(np.float32)).astype(e4)
    out = np.empty((P, 4 * nd2, C), dtype=e4)
    for j in range(nd2):
        for i in range(2):
            out[:, 4 * j + i, :] = hi[j, i]
            out[:, 4 * j + 2 + i, :] = lo[j, i]
    return out


def make_in_maps(query, key_value, Wq, Wk, Wv, Wo):
    import ml_dtypes

    query = np.asarray(query, dtype=np.float32)
    key_value = np.asarray(key_value, dtype=np.float32)
    Wq, Wk, Wv, Wo = (np.asarray(w, dtype=np.float32) for w in (Wq, Wk, Wv, Wo))
    GO = Wq.shape[0] // 2
    bf = ml_dtypes.bfloat16
    xq8_b = [_pack_hilo(np.ascontiguousarray(query[b].T)) for b in range(B_FULL)]
    xkv8_b = [_pack_hilo(np.ascontiguousarray(key_value[b].T)) for b in range(B_FULL)]
    w8 = {}
    for g in range(2):
        sl = slice(g * GO, (g + 1) * GO)
        w8[g] = (
            _pack_hilo(np.ascontiguousarray(Wq[sl, :].T)),
            _pack_hilo(np.ascontiguousarray(Wk[sl, :].T)),
            _pack_hilo(np.ascontiguousarray(Wv[sl, :].T)),
            _pack_hilo(np.ascontiguousarray(Wo[:, sl].T)),
        )
    in_maps = []
    for c in range(N_CORES):
        b, g = c // 2, c % 2
        sl = slice(g * GO, (g + 1) * GO)
        in_maps.append(
            {
                "xq8": xq8_b[b],
                "xkv8": xkv8_b[b],
                "wq8": w8[g][0],
                "wk8": w8[g][1],
                "wv8": w8[g][2],
                "wo8": w8[g][3],
            }
        )
    return in_maps


def kernel(query, key_value, Wq, Wk, Wv, Wo):
    from concourse import bass_utils

    nc = _get_nc()
    in_maps = make_in_maps(query, key_value, Wq, Wk, Wv, Wo)
    res = bass_utils.run_bass_kernel_spmd(nc, in_maps, core_ids=list(range(N_CORES)))
    ys = [r["y"] for r in res.results]
    out = np.stack([ys[2 * b] + ys[2 * b + 1] for b in range(B_FULL)])
    return out.astype(np.float32)



# revision 82
# speedup vs baseline: 1.0001x; 1.0001x over previous
"""Causal cross-attention kernel for 8 trn2 NeuronCores.

Sharding: 4-way data-parallel over batch x 2-way tensor-parallel over heads
(8 heads per core).  Per core:
  - Q/K/V/O projections run as fp8e4m3 DoubleRow matmuls (2 k-subtiles per
    instruction at 0.5 cyc/row) with an error-compensated hi/lo split:
    x = xh + xl and w = wh + wl quantized at scale 32, and the three
    products xh*wh + xl*wh + xh*wl accumulate at a common PSUM scale of
    1024, so projection error is below bf16 rounding at 0.75x bf16 cost.
  - Scores are a single one-sided DoubleRow matmul per (head, k-block):
    kT is stored as an exact fp8 hi/lo pair, qT as single fp8, with q
    broadcast across both subtiles (2x bf16 throughput, ~1.5e-2 rel err).
  - Attention in transposed layout: scores^T[k,q] -> exp on ACT (a slice of
    late-q-tile exps uses a bf16 Schraudolph fast-exp on DVE to unload
    ACT) -> stationary-P^T PV step with the 65-wide bf16 V-aug (ones
    column accumulates the softmax denominator).  Normalized O blocks are
    DMA-transposed and split into fp8 hi/lo for the DoubleRow outproj.
  - Orchestration is a fine-grained pull model: scores (hp, qt, kb) pull
    exactly Qproj(qt, hp)/Kproj(kb//4, hp) just in time; PV units emit
    inline one k-block behind the score stream (trailing blocks spill into
    the next group); remaining projection and outproj work is debt-paced
    PE filler between score emissions.

All host-side work (transposes, hi/lo packing) is data marshaling; the
device kernel is a single NEFF launch per core.
"""

import sys

sys.path.insert(0, "/opt/trn_rl_repo")

import numpy as np

import concourse.bass as bass
import concourse.tile as tile
from concourse import bacc, mybir
from concourse.bass import ts
from concourse.masks import make_upper_triangular

F32 = mybir.dt.float32
F32R = mybir.dt.float32r
BF16 = mybir.dt.bfloat16
FP8 = mybir.dt.float8e4
I16 = mybir.dt.int16
P = 128

# full-problem constants
B_FULL = 4
S_FULL = 2048
D_FULL = 1024
HG_FULL = 8  # heads per core (16 heads / 2-way TP)
N_CORES = 8


def build_bass(S=S_FULL, D=D_FULL, HG=HG_FULL):
    """One-core program; SPMD across 8 cores with different data."""
    GO = HG * 64  # output-feature width of this core's head group
    ND = D // P  # d-blocks (contraction)
    NM = GO // P  # o-tiles of Q/K projections
    NQT = S // 512  # q-tiles (512 wide)
    NTB = S // P  # token blocks of 128
    TCH = 512  # projection t-chunk (one q-tile per chunk)
    NCH = S // TCH

    ND2 = ND // 2  # d-block pairs for fp8 DoubleRow
    # fp8 hi/lo packed inputs: sub-index s = 4*j + 2*hl + i selects
    # (d-pair j, hi/lo, d-block within pair); value = e4m3 of 32*x (hi)
    # or 32*x - hi (lo).  PSUM accumulates at scale 1024.
    nc = bacc.Bacc("TRN2", target_bir_lowering=False, debug=False)
    xq8 = nc.dram_tensor("xq8", [P, 4 * ND2, S], FP8, kind="ExternalInput")
    xkv8 = nc.dram_tensor("xkv8", [P, 4 * ND2, S], FP8, kind="ExternalInput")
    wq8 = nc.dram_tensor("wq8", [P, 4 * ND2, GO], FP8, kind="ExternalInput")
    wk8 = nc.dram_tensor("wk8", [P, 4 * ND2, GO], FP8, kind="ExternalInput")
    wv8 = nc.dram_tensor("wv8", [P, 4 * ND2, GO], FP8, kind="ExternalInput")
    wo8 = nc.dram_tensor("wo8", [P, 4 * (GO // P // 2), D], FP8, kind="ExternalInput")
    y = nc.dram_tensor("y", [S, D], F32, kind="ExternalOutput")

    Exp = mybir.ActivationFunctionType.Exp
    Mult = mybir.AluOpType.mult
    Add = mybir.AluOpType.add
    DR = mybir.MatmulPerfMode.DoubleRow
    EXPSCALE = 0.125 / 1024.0  # scores psum = (32q).(32k) = 1024*s
    # bf16 Schraudolph fast-exp for the Pool engine: int16(x*A + B) bitcast
    # to bf16 ~= exp(x) within ~3.6%; softmax renormalization absorbs most
    # of the sawtooth.  A fraction of exp tiles go to Pool to unload ACT.
    SCH_A = 128.0 * 1.4426950408889634
    SCH_B = 16251.0
    SCH_MOD = 6  # 1/6 of late-q-tile exps take the fast-exp path

    with tile.TileContext(nc) as tc:
        from contextlib import ExitStack

        with ExitStack() as ctx:
            ctx.enter_context(
                nc.allow_low_precision(reason="bf16/fp32r matmul input rounding")
            )
            # ---- persistent SBUF buffers ----
            pers = ctx.enter_context(tc.tile_pool(name="pers", bufs=1))
            # qT: e4m3 at scale 32; kT: [hi, lo] e4m3 pair at scale 32
            qT = [pers.tile([P, S], FP8, tag=f"qT{i}", name=f"qT{i}") for i in range(NM)]
            kT = [pers.tile([P, 2, S], FP8, tag=f"kT{i}", name=f"kT{i}") for i in range(NM)]
            vaug = [pers.tile([P, HG * 65], BF16, tag=f"va{i}", name=f"va{i}") for i in range(NTB)]
            # attention output in fp8 hi/lo pairs per ob-pair jj for the
            # DoubleRow output projection; oT blocks are transient
            oh8 = [pers.tile([P, 2, S], FP8, tag=f"oh{j}", name=f"oh{j}") for j in range(NM // 2)]
            ol8 = [pers.tile([P, 2, S], FP8, tag=f"ol{j}", name=f"ol{j}") for j in range(NM // 2)]
            consts = ctx.enter_context(tc.tile_pool(name="consts", bufs=1))
            tri_f = consts.tile([P, P], F32)  # tri[k,q] = 1 if q >= k else 0
            make_upper_triangular(nc, tri_f[:], val=1.0, diag=True)
            # two side-by-side bf16 copies so one strided op masks 2 heads
            tri2 = consts.tile([P, 2 * P], BF16)
            nc.vector.tensor_copy(tri2[:, 0:P], tri_f[:])
            nc.vector.tensor_copy(tri2[:, P : 2 * P], tri_f[:])
            for i in range(NTB):
                # ones columns survive the V evictions (cols h*65+64)
                nc.gpsimd.memset(vaug[i][:], 1.0)

            w_pool = ctx.enter_context(tc.tile_pool(name="wp", bufs=1))
            x_pool = ctx.enter_context(tc.tile_pool(name="xp", bufs=2))
            big_pool = ctx.enter_context(tc.tile_pool(name="big", bufs=2, space="PSUM"))
            spool = ctx.enter_context(tc.tile_pool(name="ps_s", bufs=2, space="PSUM"))
            opool = ctx.enter_context(tc.tile_pool(name="ps_o", bufs=2, space="PSUM"))
            apool = ctx.enter_context(tc.tile_pool(name="att", bufs=2))
            apool2 = ctx.enter_context(tc.tile_pool(name="attn2", bufs=2))
            opool_sb = ctx.enter_context(tc.tile_pool(name="osb", bufs=2))
            y_pool = ctx.enter_context(tc.tile_pool(name="yev", bufs=3))

            wq_t = w_pool.tile([P, 4 * ND2, GO], FP8, tag="wq", name="wq")
            wk_t = w_pool.tile([P, 4 * ND2, GO], FP8, tag="wk", name="wk")
            wv_t = w_pool.tile([P, 4 * ND2, GO], FP8, tag="wv", name="wv")
            wo_t = w_pool.tile([P, 4 * (NM // 2), D], FP8, tag="wo", name="wo")
            ot_pool = ctx.enter_context(tc.tile_pool(name="otp", bufs=2))
            def emit_w_dmas():
                # critical path first: wq then wk feed the first score tile
                for j in range(1, ND2):
                    nc.sync.dma_start(wq_t[:, 4 * j : 4 * j + 4, :], wq8[:, 4 * j : 4 * j + 4, :])
                for j in range(ND2):
                    nc.sync.dma_start(wk_t[:, 4 * j : 4 * j + 4, :], wk8[:, 4 * j : 4 * j + 4, :])

            def emit_w_dmas_late():
                for j in range(ND2):
                    nc.sync.dma_start(wv_t[:, 4 * j : 4 * j + 4, :], wv8[:, 4 * j : 4 * j + 4, :])
                for j in range(NM // 2):
                    nc.sync.dma_start(wo_t[:, 4 * j : 4 * j + 4, :], wo8[:, 4 * j : 4 * j + 4, :])

            # ---------------- emitter units ----------------
            xq_tiles = {}
            xkv_tiles = {}
            ysb_tiles = {}
            x_dmas_done = set()

            def emit_x_dma(tc_i):
                x_dmas_done.add(tc_i)
                xq = x_pool.tile([P, 4 * ND2, TCH], FP8, tag="xq", name=f"xq_{tc_i}")
                xkv = x_pool.tile([P, 4 * ND2, TCH], FP8, tag="xk", name=f"xkv_{tc_i}")
                for h in range(2):
                    sl = slice(8 * h, 8 * h + 8)
                    nc.sync.dma_start(xq[:, sl, :], xq8[:, sl, ts(tc_i, TCH)])
                    nc.sync.dma_start(xkv[:, sl, :], xkv8[:, sl, ts(tc_i, TCH)])
                xq_tiles[tc_i] = xq
                xkv_tiles[tc_i] = xkv

            def dr3(ps, wt, xt, mcols, j, first, last):
                # 3-term error-compensated fp8 DoubleRow: hi*hi + hi*lo + lo*hi
                wh = wt[:, 4 * j : 4 * j + 2, mcols]
                wl = wt[:, 4 * j + 2 : 4 * j + 4, mcols]
                xh = xt[:, 4 * j : 4 * j + 2, :]
                xl = xt[:, 4 * j + 2 : 4 * j + 4, :]
                nc.tensor.matmul(ps, wh, xh, start=first, stop=False, perf_mode=DR)
                nc.tensor.matmul(ps, wh, xl, start=False, stop=False, perf_mode=DR)
                nc.tensor.matmul(ps, wl, xh, start=False, stop=last, perf_mode=DR)

            def emit_proj_q_m(tc_i, m):
                xq = xq_tiles[tc_i]
                ps = big_pool.tile([P, 512], F32, tag="big", name=f"pq{tc_i}_{m}")
                for j in range(ND2):
                    dr3(ps[:, 0:TCH], wq_t, xq, ts(m, P), j, j == 0, j == ND2 - 1)
                # evict 1024q -> e4m3(32q)
                nc.vector.tensor_scalar_mul(qT[m][:, ts(tc_i, TCH)], ps[:, 0:TCH], 1.0 / 32.0)

            def emit_proj_k_m(tc_i, m):
                xkv = xkv_tiles[tc_i]
                ps = big_pool.tile([P, 512], F32, tag="big", name=f"pk{tc_i}_{m}")
                for j in range(ND2):
                    dr3(ps[:, 0:TCH], wk_t, xkv, ts(m, P), j, j == 0, j == ND2 - 1)
                # evict 1024k -> hi = e4m3(32k), lo = e4m3(32k - hi)
                nc.vector.tensor_scalar_mul(kT[m][:, 0, ts(tc_i, TCH)], ps[:, 0:TCH], 1.0 / 32.0)
                nc.vector.scalar_tensor_tensor(
                    kT[m][:, 1, ts(tc_i, TCH)],
                    ps[:, 0:TCH],
                    1.0 / 32.0,
                    kT[m][:, 0, ts(tc_i, TCH)],
                    Mult,
                    mybir.AluOpType.subtract,
                )

            def emit_proj_v_mt(tc_i, mt):
                xkv = xkv_tiles[tc_i]
                ps = big_pool.tile([P, 512], F32, tag="big", name=f"pv{tc_i}_{mt}")
                for j in range(ND2):
                    xh = xkv[:, 4 * j : 4 * j + 2, ts(mt, P)]
                    xl = xkv[:, 4 * j + 2 : 4 * j + 4, ts(mt, P)]
                    wh = wv_t[:, 4 * j : 4 * j + 2, :]
                    wl = wv_t[:, 4 * j + 2 : 4 * j + 4, :]
                    nc.tensor.matmul(ps[:], xh, wh, start=(j == 0), stop=False, perf_mode=DR)
                    nc.tensor.matmul(ps[:], xl, wh, start=False, stop=False, perf_mode=DR)
                    nc.tensor.matmul(ps[:], xh, wl, start=False, stop=(j == ND2 - 1), perf_mode=DR)
                vt = vaug[tc_i * (TCH // P) + mt]
                nc.vector.tensor_copy(
                    vt[:].rearrange("p (h c) -> p h c", c=65)[:, :, 0:64],
                    ps[:].rearrange("p (h c) -> p h c", c=64),
                )

            def emit_proj_q(tc_i):
                for m in range(NM):
                    emit_proj_q_m(tc_i, m)

            def emit_proj_k(tc_i):
                for m in range(NM):
                    emit_proj_k_m(tc_i, m)

            def emit_proj_v(tc_i):
                for mt in range(TCH // P):
                    emit_proj_v_mt(tc_i, mt)
                del xq_tiles[tc_i], xkv_tiles[tc_i]

            def emit_outproj_nt(mt, nt):
                # fp8 DoubleRow 3-term: psum = 1024*y over ob-pairs jj
                ps = big_pool.tile([P, 512], F32, tag="big", name=f"y{mt}_{nt}")
                NJ = NM // 2
                for jj in range(NJ):
                    oh = oh8[jj][:, :, ts(mt, P)]
                    ol = ol8[jj][:, :, ts(mt, P)]
                    wh = wo_t[:, 4 * jj : 4 * jj + 2, ts(nt, 512)]
                    wl = wo_t[:, 4 * jj + 2 : 4 * jj + 4, ts(nt, 512)]
                    nc.tensor.matmul(ps[:], oh, wh, start=(jj == 0), stop=False, perf_mode=DR)
                    nc.tensor.matmul(ps[:], ol, wh, start=False, stop=False, perf_mode=DR)
                    nc.tensor.matmul(ps[:], oh, wl, start=False, stop=(jj == NJ - 1), perf_mode=DR)
                ysb = y_pool.tile([P, 512], F32, tag="ysb", name=f"ysb{mt}_{nt}")
                nc.vector.tensor_scalar_mul(ysb[:], ps[:], 1.0 / 1024.0)
                nc.sync.dma_start(y[ts(mt, P), ts(nt, 512)], ysb[:])

            def emit_score_kb(hp, qt, kb):
                j = kb - 4 * qt
                ce = max(j, 0) * P
                # both heads' scores in one 2-bank PSUM tile so a single
                # strided activation does both exps
                pss = spool.tile([P, 1024], F32, tag="s", name=f"s{hp}_{qt}_{kb}")
                w1 = 512 - ce
                for g, po in ((0, 0), (1, 64)):
                    # one fp8 DoubleRow matmul: (kh + kl) . q, q broadcast
                    qv = (
                        qT[hp][po : po + 64, qt * 512 + ce : (qt + 1) * 512]
                        .unsqueeze(1)
                        .broadcast_to([64, 2, w1])
                    )
                    nc.tensor.matmul(
                        pss[:, g * 512 + ce : (g + 1) * 512],
                        kT[hp][po : po + 64, :, ts(kb, P)],
                        qv,
                        start=True,
                        stop=True,
                        perf_mode=DR,
                    )
                pexp = apool.tile([P, 1024], BF16, tag=f"p{kb}", name=f"p{hp}_{qt}_{kb}")
                out_ap = pexp[:].rearrange("p (g c) -> p g c", g=2)[:, :, ce:]
                in_ap = pss[:].rearrange("p (g c) -> p g c", g=2)[:, :, ce:]
                if qt >= 2 and (kb + hp) % SCH_MOD == 0:
                    # DVE fast-exp, only in the ACT-bound late q-tiles
                    nc.vector.tensor_scalar(
                        out_ap.bitcast(I16), in_ap, SCH_A * EXPSCALE, SCH_B, Mult, Add
                    )
                else:
                    nc.scalar.activation(out_ap, in_ap, Exp, scale=EXPSCALE)
                if j >= 0:
                    # mask the boundary block for both heads in one op
                    nc.gpsimd.tensor_tensor(
                        pexp[:].rearrange("p (g c) -> p g c", g=2)[:, :, ts(j, P)],
                        pexp[:].rearrange("p (g c) -> p g c", g=2)[:, :, ts(j, P)],
                        tri2[:].rearrange("p (g c) -> p g c", g=2),
                        Mult,
                    )
                return pexp



            def make_pv_unit(hp, qt, qb, pexps, osb, otq):
                def fn():
                    # both heads' PV accumulation groups, sequentially, into
                    # one [128, 130] PSUM tile (cols h*65+64 = denominators);
                    # each group runs start-to-stop before the next opens
                    # (2KB PSUM zero-region rule).
                    pv = opool.tile([P, 130], F32, tag="pv", name=f"pv{hp}_{qt}_{qb}")
                    for g, hh in ((0, 2 * hp), (1, 2 * hp + 1)):
                        for kb in range(4 * qt + qb + 1):
                            nc.tensor.matmul(
                                pv[:, g * 65 : g * 65 + 65],
                                pexps[kb][:, g * 512 + qb * P : g * 512 + (qb + 1) * P],
                                vaug[kb][:, hh * 65 : hh * 65 + 65],
                                start=(kb == 0),
                                stop=(kb == 4 * qt + qb),
                            )
                    rec = apool2.tile([P, 2], F32, tag="rec", name=f"rec{hp}_{qt}_{qb}")
                    nc.vector.reciprocal(
                        rec[:].rearrange("p (g c) -> p g c", c=1),
                        pv[:].rearrange("p (g c) -> p g c", c=65)[:, :, 64:65],
                    )
                    for g in (0, 1):
                        # pv holds sum(p * 1024*v); rescale by 1/1024 here
                        nc.vector.tensor_scalar(
                            osb[:, g * 64 : (g + 1) * 64],
                            pv[:, g * 65 : g * 65 + 64],
                            rec[:, g : g + 1],
                            1.0 / 1024.0,
                            Mult,
                            Mult,
                        )
                    # one DMA-transpose moves both heads' normalized O[q, dh]
                    # block into O^T[dh, q] inside the per-(hp,qt) staging
                    # tile; after the last block, DVE splits the 512-wide
                    # strip into the fp8 hi/lo pair the outproj consumes
                    nc.sync.dma_start_transpose(otq[:, ts(qb, P)], osb[:])
                    if qt == NQT - 1:
                        # last q-tile: split per block so outproj token
                        # blocks unlock as early as possible (short tail)
                        blk = slice((4 * qt + qb) * P, (4 * qt + qb + 1) * P)
                        ohs = oh8[hp // 2][:, hp % 2, blk]
                        nc.vector.tensor_scalar_mul(ohs, otq[:, ts(qb, P)], 32.0)
                        nc.vector.scalar_tensor_tensor(
                            ol8[hp // 2][:, hp % 2, blk],
                            otq[:, ts(qb, P)],
                            32.0,
                            ohs,
                            Mult,
                            mybir.AluOpType.subtract,
                        )
                        attn_cnt[4 * qt + qb] += 1
                    elif qb == 3:
                        blk = slice(qt * 512, (qt + 1) * 512)
                        ohs = oh8[hp // 2][:, hp % 2, blk]
                        nc.vector.tensor_scalar_mul(ohs, otq[:], 32.0)
                        nc.vector.scalar_tensor_tensor(
                            ol8[hp // 2][:, hp % 2, blk],
                            otq[:],
                            32.0,
                            ohs,
                            Mult,
                            mybir.AluOpType.subtract,
                        )
                        for i in range(4):
                            attn_cnt[4 * qt + i] += 1

                return (2 * (4 * qt + qb + 1) * 65 * 0.42 + 120, fn)

            # ---------------- orchestration ----------------
            # Fine-grained pull model: scores (hp, qt, kb) pull exactly
            # Qproj(qt, hp) and Kproj(kb//4, hp) just in time, so exp work
            # flows to ACT as early as the data allows.  Remaining proj
            # units, PV units and outproj blocks are debt-paced PE filler.
            NVT = TCH // P  # V-proj token-blocks per chunk
            q_done = [[False] * NM for _ in range(NCH)]
            k_done = [[False] * NM for _ in range(NCH)]
            v_done = [[False] * NVT for _ in range(NCH)]

            def chunk_all_done(c):
                return all(q_done[c]) and all(k_done[c]) and all(v_done[c])

            def ensure_x(c):
                if c not in x_dmas_done:
                    # tile-slot hazard: chunk c's x DMA reuses chunk c-2's
                    # buffers; all chunk c-2 readers must be emitted first
                    if c >= 2 and not chunk_all_done(c - 2):
                        pull_chunk(c - 2)
                    emit_x_dma(c)

            def pull_q(c, m):
                if not q_done[c][m]:
                    q_done[c][m] = True
                    ensure_x(c)
                    emit_proj_q_m(c, m)
                    return 1280
                return 0

            def pull_k(c, m):
                if not k_done[c][m]:
                    k_done[c][m] = True
                    ensure_x(c)
                    emit_proj_k_m(c, m)
                    return 1280
                return 0

            def pull_v(c, mt):
                if not v_done[c][mt]:
                    v_done[c][mt] = True
                    ensure_x(c)
                    emit_proj_v_mt(c, mt)
                    return 1280
                return 0

            def pull_chunk(c):
                for m in range(NM):
                    pull_q(c, m)
                for m in range(NM):
                    pull_k(c, m)
                for mt in range(NVT):
                    pull_v(c, mt)

            # workq: debt-paced PE filler (cost_ns, fn); fn returns actual
            # cost (0 if the unit was already pulled directly)
            workq = []
            ogate = {}
            attn_cnt = [0] * NTB  # per token block: heads with split done

            def queue_fillers():
                for c in range(NCH):
                    for m in range(NM):
                        workq.append((f"c{c}", 1280, lambda c=c, m=m: pull_q(c, m)))
                    for m in range(NM):
                        workq.append((f"c{c}", 1280, lambda c=c, m=m: pull_k(c, m)))
                    for mt in range(NVT):
                        workq.append((f"c{c}", 1280, lambda c=c, mt=mt: pull_v(c, mt)))
                for mt in range(NTB):
                    for nt in range(D // 512):
                        def fo(mt=mt, nt=nt):
                            emit_outproj_nt(mt, nt)
                            return 853

                        ogate[id(fo)] = mt
                        workq.append(("o", 640, fo))

            def pop_work(budget_ns):
                spent = 0.0
                i = 0
                while i < len(workq) and spent < budget_ns:
                    kind, cost, fn = workq[i]
                    if kind == "o" and attn_cnt[ogate[id(fn)]] < HG // 2:
                        i += 1
                        continue
                    r = fn()
                    spent += cost if r is None else r
                    workq.pop(i)
                return spent

            # critical-path DMAs first: wq, x chunk 0, wk feed the first
            # scores; wv/wo and chunk 1 follow
            nc.sync.dma_start(wq_t[:, 0:4, :], wq8[:, 0:4, :])
            emit_x_dma(0)
            emit_w_dmas()
            emit_w_dmas_late()
            emit_x_dma(1)
            queue_fillers()
            # PE prewarm: dummy matmuls on the tri constant ramp the tensor
            # engine to full clock while the first weight/x DMAs land
            pwt = big_pool.tile([P, 512], F32, tag="big", name="prewarm")
            for _ in range(36):
                nc.tensor.matmul(
                    pwt[:, 0:P], tri2[:, 0:P], tri2[:, 0:P], start=True, stop=True
                )

            # debt-paced weave: pop a PE filler unit only once the consumer
            # engine's exp backlog exceeds its cost, so the PE stays just
            # behind ACT/DVE.  PV units are emitted inline, one k-block
            # behind the score stream, so nothing drains at the end.
            debt = 0.0
            deferred_pv = []
            groups = [(qt, hp) for qt in range(NQT) for hp in range(HG // 2)]
            for gi, (qt, hp) in enumerate(groups):
                if hp == 0 and qt + 1 < NCH:
                    ensure_x(qt + 1)
                pull_q(qt, hp)
                nkb = 4 * qt + 4
                pexps = []
                osbs = [
                    opool_sb.tile([P, P], BF16, tag=f"osb{qb}", name=f"osb{hp}_{qt}_{qb}")
                    for qb in range(4)
                ]
                otq = ot_pool.tile([P, 512], BF16, tag=f"ot{hp}", name=f"ot{hp}_{qt}")

                def emit_pv(qb, qt=qt, hp=hp, pexps=pexps, osbs=osbs, otq=otq):
                    # vaug writes must be emitted before the PV reads them
                    for j in range(4 * qt + qb + 1):
                        pull_v(j // 4, j % 4)
                    cost, fn = make_pv_unit(hp, qt, qb, pexps, osbs[qb], otq)
                    fn()
                    return cost

                last = gi == len(groups) - 1
                for kb in range(nkb):
                    pull_k(kb // 4, hp)
                    pexps.append(emit_score_kb(hp, qt, kb))
                    if kb == 1 and not last:
                        # prefetch the next group's Q/K so its first score
                        # fires the moment this group's exps are drained
                        nqt, nhp = groups[gi + 1]
                        pull_q(nqt, nhp)
                        pull_k(0, nhp)
                    if kb % 4 == 2 and kb + 2 < nkb:
                        # prefetch the next k-chunk's K projection so its
                        # DVE eviction lands before the scores need it
                        pull_k((kb + 2) // 4, hp)
                    if kb <= 1 and deferred_pv:
                        # previous group's trailing PV blocks: their exps are
                        # long done, so no PE stall and no ACT gap
                        debt -= deferred_pv.pop(0)()
                    w = 2 * (512 - max(kb - 4 * qt, 0) * P)
                    if qt >= 2 and (kb + hp) % SCH_MOD == 0:
                        debt += (w * 1.04 + 170) - (w * 0.21 + 10)
                    else:
                        debt += (w * 0.833 + 242) - (w * 0.21 + 10)
                    if last:
                        continue  # emit the final scores back-to-back
                    qb = kb - 4 * qt - 2
                    if 0 <= qb <= 1:
                        debt -= emit_pv(qb)
                    npops = 0
                    cap = 2
                    while workq and npops < cap:
                        # prefer proj units; spend outproj units only when
                        # nothing else is ready (saves them for the
                        # filler-starved late q-tiles)
                        pick = None
                        for i, (kind, cost, fn) in enumerate(workq):
                            if kind == "o":
                                continue
                            pick = i
                            break
                        if pick is None:
                            for i, (kind, cost, fn) in enumerate(workq):
                                if kind == "o" and attn_cnt[ogate[id(fn)]] >= HG // 2:
                                    pick = i
                                    break
                        if pick is None or (
                            workq[pick][1] > debt
                            and not (last and workq[pick][0] == "o")
                        ):
                            break
                        kind, cost, fn = workq.pop(pick)
                        r = fn()
                        debt -= r if r is not None else cost
                        npops += 1
                if not last:
                    deferred_pv.append(lambda e=emit_pv: e(2))
                    deferred_pv.append(lambda e=emit_pv: e(3))
                else:
                    # tail: PV per block, then its outproj immediately
                    for qb in range(4):
                        emit_pv(qb)
                        pop_work(1 << 30)
            while workq:
                pop_work(1 << 30)
    nc.finalize()
    return nc


_NC_CACHE = {}


def _get_nc():
    if "full" not in _NC_CACHE:
        _NC_CACHE["full"] = build_bass()
    return _NC_CACHE["full"]


def _pack_hilo(mT):
    """[D, C] fp32 -> [128, 4*ND2, C] fp8 hi/lo pack at scale 32.

    sub-index s = 4*j + 2*hl + i: (d-pair j, hi/lo, block i); value
    hi = e4m3(32*x), lo = e4m3(32*x - hi).
    """
    import ml_dtypes

    e4 = ml_dtypes.float8_e4m3
    D, C = mT.shape
    nd = D // P
    nd2 = nd // 2
    blocks = mT.reshape(nd2, 2, P, C)  # [j, i, p, c]
    hi = (32.0 * blocks).astype(e4)
    lo = (32.0 * blocks - hi.astype(np.float32)).astype(e4)
    out = np.empty((P, 4 * nd2, C), dtype=e4)
    for j in range(nd2):
        for i in range(2):
            out[:, 4 * j + i, :] = hi[j, i]
            out[:, 4 * j + 2 + i, :] = lo[j, i]
    return out


def make_in_maps(query, key_value, Wq, Wk, Wv, Wo):
    import ml_dtypes

    query = np.asarray(query, dtype=np.float32)
    key_value = np.asarray(key_value, dtype=np.float32)
    Wq, Wk, Wv, Wo = (np.asarray(w, dtype=np.float32) for w in (Wq, Wk, Wv, Wo))
    GO = Wq.shape[0] // 2
    bf = ml_dtypes.bfloat16
    xq8_b = [_pack_hilo(np.ascontiguousarray(query[b].T)) for b in range(B_FULL)]
    xkv8_b = [_pack_hilo(np.ascontiguousarray(key_value[b].T)) for b in range(B_FULL)]
    w8 = {}
    for g in range(2):
        sl = slice(g * GO, (g + 1) * GO)
        w8[g] = (
            _pack_hilo(np.ascontiguousarray(Wq[sl, :].T)),
            _pack_hilo(np.ascontiguousarray(Wk[sl, :].T)),
            _pack_hilo(np.ascontiguousarray(Wv[sl, :].T)),
            _pack_hilo(np.ascontiguousarray(Wo[:, sl].T)),
        )
    in_maps = []
    for c in range(N_CORES):
        b, g = c // 2, c % 2
        sl = slice(g * GO, (g + 1) * GO)
        in_maps.append(
            {
                "xq8": xq8_b[b],
                "xkv8": xkv8_b[b],
                "wq8": w8[g][0],
                "wk8": w8[g][1],
                "wv8": w8[g][2],
                "wo8": w8[g][3],
            }
        )
    return in_maps


def kernel(query, key_value, Wq, Wk, Wv, Wo):
    from concourse import bass_utils

    nc = _get_nc()
    in_maps = make_in_maps(query, key_value, Wq, Wk, Wv, Wo)
    res = bass_utils.run_bass_kernel_spmd(nc, in_maps, core_ids=list(range(N_CORES)))
    ys = [r["y"] for r in res.results]
    out = np.stack([ys[2 * b] + ys[2 * b + 1] for b in range(B_FULL)])
    return out.astype(np.float32)



# revision 83
# speedup vs baseline: 1.0076x; 1.0076x over previous
"""Causal cross-attention kernel for 8 trn2 NeuronCores.

Sharding: 4-way data-parallel over batch x 2-way tensor-parallel over heads
(8 heads per core).  Per core:
  - Q/K/V/O projections run as fp8e4m3 DoubleRow matmuls (2 k-subtiles per
    instruction at 0.5 cyc/row) with an error-compensated hi/lo split:
    x = xh + xl and w = wh + wl quantized at scale 32, and the three
    products xh*wh + xl*wh + xh*wl accumulate at a common PSUM scale of
    1024, so projection error is below bf16 rounding at 0.75x bf16 cost.
  - Scores are a single one-sided DoubleRow matmul per (head, k-block):
    kT is stored as an exact fp8 hi/lo pair, qT as single fp8, with q
    broadcast across both subtiles (2x bf16 throughput, ~1.5e-2 rel err).
  - Attention in transposed layout: scores^T[k,q] -> exp on ACT (a slice of
    late-q-tile exps uses a bf16 Schraudolph fast-exp on DVE to unload
    ACT) -> stationary-P^T PV step with the 65-wide bf16 V-aug (ones
    column accumulates the softmax denominator).  Normalized O blocks are
    DMA-transposed and split into fp8 hi/lo for the DoubleRow outproj.
  - Orchestration is a fine-grained pull model: scores (hp, qt, kb) pull
    exactly Qproj(qt, hp)/Kproj(kb//4, hp) just in time; PV units emit
    inline one k-block behind the score stream (trailing blocks spill into
    the next group); remaining projection and outproj work is debt-paced
    PE filler between score emissions.

All host-side work (transposes, hi/lo packing) is data marshaling; the
device kernel is a single NEFF launch per core.
"""

import sys

sys.path.insert(0, "/opt/trn_rl_repo")

import numpy as np

import concourse.bass as bass
import concourse.tile as tile
from concourse import bacc, mybir
from concourse.bass import ts
from concourse.masks import make_upper_triangular

F32 = mybir.dt.float32
F32R = mybir.dt.float32r
BF16 = mybir.dt.bfloat16
FP8 = mybir.dt.float8e4
I16 = mybir.dt.int16
P = 128

# full-problem constants
B_FULL = 4
S_FULL = 2048
D_FULL = 1024
HG_FULL = 8  # heads per core (16 heads / 2-way TP)
N_CORES = 8


def build_bass(S=S_FULL, D=D_FULL, HG=HG_FULL):
    """One-core program; SPMD across 8 cores with different data."""
    GO = HG * 64  # output-feature width of this core's head group
    ND = D // P  # d-blocks (contraction)
    NM = GO // P  # o-tiles of Q/K projections
    NQT = S // 512  # q-tiles (512 wide)
    NTB = S // P  # token blocks of 128
    TCH = 512  # projection t-chunk (one q-tile per chunk)
    NCH = S // TCH

    ND2 = ND // 2  # d-block pairs for fp8 DoubleRow
    # fp8 hi/lo packed inputs: sub-index s = 4*j + 2*hl + i selects
    # (d-pair j, hi/lo, d-block within pair); value = e4m3 of 32*x (hi)
    # or 32*x - hi (lo).  PSUM accumulates at scale 1024.
    nc = bacc.Bacc("TRN2", target_bir_lowering=False, debug=False)
    xq8 = nc.dram_tensor("xq8", [P, 4 * ND2, S], FP8, kind="ExternalInput")
    xkv8 = nc.dram_tensor("xkv8", [P, 4 * ND2, S], FP8, kind="ExternalInput")
    wq8 = nc.dram_tensor("wq8", [P, 4 * ND2, GO], FP8, kind="ExternalInput")
    wk8 = nc.dram_tensor("wk8", [P, 4 * ND2, GO], FP8, kind="ExternalInput")
    wv8 = nc.dram_tensor("wv8", [P, 4 * ND2, GO], FP8, kind="ExternalInput")
    wo8 = nc.dram_tensor("wo8", [P, 4 * (GO // P // 2), D], FP8, kind="ExternalInput")
    y = nc.dram_tensor("y", [S, D], F32, kind="ExternalOutput")

    Exp = mybir.ActivationFunctionType.Exp
    Mult = mybir.AluOpType.mult
    Add = mybir.AluOpType.add
    DR = mybir.MatmulPerfMode.DoubleRow
    EXPSCALE = 0.125 / 1024.0  # scores psum = (32q).(32k) = 1024*s
    # bf16 Schraudolph fast-exp for the Pool engine: int16(x*A + B) bitcast
    # to bf16 ~= exp(x) within ~3.6%; softmax renormalization absorbs most
    # of the sawtooth.  A fraction of exp tiles go to Pool to unload ACT.
    SCH_A = 128.0 * 1.4426950408889634
    SCH_B = 16251.0
    SCH_MOD = 6  # 1/6 of late-q-tile exps take the fast-exp path

    with tile.TileContext(nc) as tc:
        from contextlib import ExitStack

        with ExitStack() as ctx:
            ctx.enter_context(
                nc.allow_low_precision(reason="bf16/fp32r matmul input rounding")
            )
            # ---- persistent SBUF buffers ----
            pers = ctx.enter_context(tc.tile_pool(name="pers", bufs=1))
            # qT: e4m3 at scale 32; kT: [hi, lo] e4m3 pair at scale 32
            qT = [pers.tile([P, S], FP8, tag=f"qT{i}", name=f"qT{i}") for i in range(NM)]
            kT = [pers.tile([P, 2, S], FP8, tag=f"kT{i}", name=f"kT{i}") for i in range(NM)]
            vaug = [pers.tile([P, HG * 65], BF16, tag=f"va{i}", name=f"va{i}") for i in range(NTB)]
            # attention output in fp8 hi/lo pairs per ob-pair jj for the
            # DoubleRow output projection; oT blocks are transient
            oh8 = [pers.tile([P, 2, S], FP8, tag=f"oh{j}", name=f"oh{j}") for j in range(NM // 2)]
            ol8 = [pers.tile([P, 2, S], FP8, tag=f"ol{j}", name=f"ol{j}") for j in range(NM // 2)]
            consts = ctx.enter_context(tc.tile_pool(name="consts", bufs=1))
            tri_f = consts.tile([P, P], F32)  # tri[k,q] = 1 if q >= k else 0
            make_upper_triangular(nc, tri_f[:], val=1.0, diag=True)
            # two side-by-side bf16 copies so one strided op masks 2 heads
            tri2 = consts.tile([P, 2 * P], BF16)
            nc.vector.tensor_copy(tri2[:, 0:P], tri_f[:])
            nc.vector.tensor_copy(tri2[:, P : 2 * P], tri_f[:])
            for i in range(NTB):
                # ones columns survive the V evictions (cols h*65+64)
                nc.gpsimd.memset(vaug[i][:], 1.0)

            w_pool = ctx.enter_context(tc.tile_pool(name="wp", bufs=1))
            x_pool = ctx.enter_context(tc.tile_pool(name="xp", bufs=2))
            big_pool = ctx.enter_context(tc.tile_pool(name="big", bufs=2, space="PSUM"))
            spool = ctx.enter_context(tc.tile_pool(name="ps_s", bufs=2, space="PSUM"))
            opool = ctx.enter_context(tc.tile_pool(name="ps_o", bufs=2, space="PSUM"))
            apool = ctx.enter_context(tc.tile_pool(name="att", bufs=2))
            apool2 = ctx.enter_context(tc.tile_pool(name="attn2", bufs=2))
            opool_sb = ctx.enter_context(tc.tile_pool(name="osb", bufs=2))
            y_pool = ctx.enter_context(tc.tile_pool(name="yev", bufs=3))

            wq_t = w_pool.tile([P, 4 * ND2, GO], FP8, tag="wq", name="wq")
            wk_t = w_pool.tile([P, 4 * ND2, GO], FP8, tag="wk", name="wk")
            wv_t = w_pool.tile([P, 4 * ND2, GO], FP8, tag="wv", name="wv")
            wo_t = w_pool.tile([P, 4 * (NM // 2), D], FP8, tag="wo", name="wo")
            ot_pool = ctx.enter_context(tc.tile_pool(name="otp", bufs=2))
            def emit_w_dmas():
                # critical path first: wq then wk feed the first score tile
                for j in range(1, ND2):
                    nc.sync.dma_start(wq_t[:, 4 * j : 4 * j + 4, :], wq8[:, 4 * j : 4 * j + 4, :])
                for j in range(ND2):
                    nc.sync.dma_start(wk_t[:, 4 * j : 4 * j + 4, :], wk8[:, 4 * j : 4 * j + 4, :])

            def emit_w_dmas_late():
                for j in range(ND2):
                    nc.sync.dma_start(wv_t[:, 4 * j : 4 * j + 4, :], wv8[:, 4 * j : 4 * j + 4, :])
                for j in range(NM // 2):
                    nc.sync.dma_start(wo_t[:, 4 * j : 4 * j + 4, :], wo8[:, 4 * j : 4 * j + 4, :])

            # ---------------- emitter units ----------------
            xq_tiles = {}
            xkv_tiles = {}
            ysb_tiles = {}
            x_dmas_done = set()

            def emit_x_dma(tc_i):
                x_dmas_done.add(tc_i)
                xq = x_pool.tile([P, 4 * ND2, TCH], FP8, tag="xq", name=f"xq_{tc_i}")
                xkv = x_pool.tile([P, 4 * ND2, TCH], FP8, tag="xk", name=f"xkv_{tc_i}")
                for h in range(2):
                    sl = slice(8 * h, 8 * h + 8)
                    nc.sync.dma_start(xq[:, sl, :], xq8[:, sl, ts(tc_i, TCH)])
                    nc.sync.dma_start(xkv[:, sl, :], xkv8[:, sl, ts(tc_i, TCH)])
                xq_tiles[tc_i] = xq
                xkv_tiles[tc_i] = xkv

            def dr3(ps, wt, xt, mcols, j, first, last):
                # 3-term error-compensated fp8 DoubleRow: hi*hi + hi*lo + lo*hi
                wh = wt[:, 4 * j : 4 * j + 2, mcols]
                wl = wt[:, 4 * j + 2 : 4 * j + 4, mcols]
                xh = xt[:, 4 * j : 4 * j + 2, :]
                xl = xt[:, 4 * j + 2 : 4 * j + 4, :]
                nc.tensor.matmul(ps, wh, xh, start=first, stop=False, perf_mode=DR)
                nc.tensor.matmul(ps, wh, xl, start=False, stop=False, perf_mode=DR)
                nc.tensor.matmul(ps, wl, xh, start=False, stop=last, perf_mode=DR)

            def emit_proj_q_m(tc_i, m):
                xq = xq_tiles[tc_i]
                ps = big_pool.tile([P, 512], F32, tag="big", name=f"pq{tc_i}_{m}")
                for j in range(ND2):
                    dr3(ps[:, 0:TCH], wq_t, xq, ts(m, P), j, j == 0, j == ND2 - 1)
                # evict 1024q -> e4m3(32q)
                nc.vector.tensor_scalar_mul(qT[m][:, ts(tc_i, TCH)], ps[:, 0:TCH], 1.0 / 32.0)

            def emit_proj_k_m(tc_i, m):
                xkv = xkv_tiles[tc_i]
                ps = big_pool.tile([P, 512], F32, tag="big", name=f"pk{tc_i}_{m}")
                for j in range(ND2):
                    dr3(ps[:, 0:TCH], wk_t, xkv, ts(m, P), j, j == 0, j == ND2 - 1)
                # evict 1024k -> hi = e4m3(32k), lo = e4m3(32k - hi)
                nc.vector.tensor_scalar_mul(kT[m][:, 0, ts(tc_i, TCH)], ps[:, 0:TCH], 1.0 / 32.0)
                nc.vector.scalar_tensor_tensor(
                    kT[m][:, 1, ts(tc_i, TCH)],
                    ps[:, 0:TCH],
                    1.0 / 32.0,
                    kT[m][:, 0, ts(tc_i, TCH)],
                    Mult,
                    mybir.AluOpType.subtract,
                )

            def emit_proj_v_mt(tc_i, mt):
                xkv = xkv_tiles[tc_i]
                ps = big_pool.tile([P, 512], F32, tag="big", name=f"pv{tc_i}_{mt}")
                for j in range(ND2):
                    xh = xkv[:, 4 * j : 4 * j + 2, ts(mt, P)]
                    xl = xkv[:, 4 * j + 2 : 4 * j + 4, ts(mt, P)]
                    wh = wv_t[:, 4 * j : 4 * j + 2, :]
                    wl = wv_t[:, 4 * j + 2 : 4 * j + 4, :]
                    nc.tensor.matmul(ps[:], xh, wh, start=(j == 0), stop=False, perf_mode=DR)
                    nc.tensor.matmul(ps[:], xl, wh, start=False, stop=False, perf_mode=DR)
                    nc.tensor.matmul(ps[:], xh, wl, start=False, stop=(j == ND2 - 1), perf_mode=DR)
                vt = vaug[tc_i * (TCH // P) + mt]
                nc.vector.tensor_copy(
                    vt[:].rearrange("p (h c) -> p h c", c=65)[:, :, 0:64],
                    ps[:].rearrange("p (h c) -> p h c", c=64),
                )

            def emit_proj_q(tc_i):
                for m in range(NM):
                    emit_proj_q_m(tc_i, m)

            def emit_proj_k(tc_i):
                for m in range(NM):
                    emit_proj_k_m(tc_i, m)

            def emit_proj_v(tc_i):
                for mt in range(TCH // P):
                    emit_proj_v_mt(tc_i, mt)
                del xq_tiles[tc_i], xkv_tiles[tc_i]

            def emit_outproj_nt(mt, nt):
                # fp8 DoubleRow 3-term: psum = 1024*y over ob-pairs jj
                ps = big_pool.tile([P, 512], F32, tag="big", name=f"y{mt}_{nt}")
                NJ = NM // 2
                for jj in range(NJ):
                    oh = oh8[jj][:, :, ts(mt, P)]
                    ol = ol8[jj][:, :, ts(mt, P)]
                    wh = wo_t[:, 4 * jj : 4 * jj + 2, ts(nt, 512)]
                    wl = wo_t[:, 4 * jj + 2 : 4 * jj + 4, ts(nt, 512)]
                    nc.tensor.matmul(ps[:], oh, wh, start=(jj == 0), stop=False, perf_mode=DR)
                    nc.tensor.matmul(ps[:], ol, wh, start=False, stop=False, perf_mode=DR)
                    nc.tensor.matmul(ps[:], oh, wl, start=False, stop=(jj == NJ - 1), perf_mode=DR)
                ysb = y_pool.tile([P, 512], F32, tag="ysb", name=f"ysb{mt}_{nt}")
                nc.vector.tensor_scalar_mul(ysb[:], ps[:], 1.0 / 1024.0)
                nc.sync.dma_start(y[ts(mt, P), ts(nt, 512)], ysb[:])

            def emit_score_kb(hp, qt, kb):
                j = kb - 4 * qt
                ce = max(j, 0) * P
                # both heads' scores in one 2-bank PSUM tile so a single
                # strided activation does both exps
                pss = spool.tile([P, 1024], F32, tag="s", name=f"s{hp}_{qt}_{kb}")
                w1 = 512 - ce
                for g, po in ((0, 0), (1, 64)):
                    # one fp8 DoubleRow matmul: (kh + kl) . q, q broadcast
                    qv = (
                        qT[hp][po : po + 64, qt * 512 + ce : (qt + 1) * 512]
                        .unsqueeze(1)
                        .broadcast_to([64, 2, w1])
                    )
                    nc.tensor.matmul(
                        pss[:, g * 512 + ce : (g + 1) * 512],
                        kT[hp][po : po + 64, :, ts(kb, P)],
                        qv,
                        start=True,
                        stop=True,
                        perf_mode=DR,
                    )
                pexp = apool.tile([P, 1024], BF16, tag=f"p{kb}", name=f"p{hp}_{qt}_{kb}")
                out_ap = pexp[:].rearrange("p (g c) -> p g c", g=2)[:, :, ce:]
                in_ap = pss[:].rearrange("p (g c) -> p g c", g=2)[:, :, ce:]
                if qt >= 2 and (kb + hp) % SCH_MOD == 0:
                    # DVE fast-exp, only in the ACT-bound late q-tiles
                    nc.vector.tensor_scalar(
                        out_ap.bitcast(I16), in_ap, SCH_A * EXPSCALE, SCH_B, Mult, Add
                    )
                else:
                    nc.scalar.activation(out_ap, in_ap, Exp, scale=EXPSCALE)
                if j >= 0:
                    # mask the boundary block for both heads in one op
                    nc.gpsimd.tensor_tensor(
                        pexp[:].rearrange("p (g c) -> p g c", g=2)[:, :, ts(j, P)],
                        pexp[:].rearrange("p (g c) -> p g c", g=2)[:, :, ts(j, P)],
                        tri2[:].rearrange("p (g c) -> p g c", g=2),
                        Mult,
                    )
                return pexp



            def make_pv_unit(hp, qt, qb, pexps, osb, otq):
                def fn():
                    # both heads' PV accumulation groups, sequentially, into
                    # one [128, 130] PSUM tile (cols h*65+64 = denominators);
                    # each group runs start-to-stop before the next opens
                    # (2KB PSUM zero-region rule).
                    pv = opool.tile([P, 130], F32, tag="pv", name=f"pv{hp}_{qt}_{qb}")
                    for g, hh in ((0, 2 * hp), (1, 2 * hp + 1)):
                        for kb in range(4 * qt + qb + 1):
                            nc.tensor.matmul(
                                pv[:, g * 65 : g * 65 + 65],
                                pexps[kb][:, g * 512 + qb * P : g * 512 + (qb + 1) * P],
                                vaug[kb][:, hh * 65 : hh * 65 + 65],
                                start=(kb == 0),
                                stop=(kb == 4 * qt + qb),
                            )
                    rec = apool2.tile([P, 2], F32, tag="rec", name=f"rec{hp}_{qt}_{qb}")
                    nc.vector.reciprocal(
                        rec[:].rearrange("p (g c) -> p g c", c=1),
                        pv[:].rearrange("p (g c) -> p g c", c=65)[:, :, 64:65],
                    )
                    for g in (0, 1):
                        # pv holds sum(p * 1024*v); rescale by 1/1024 here
                        nc.vector.tensor_scalar(
                            osb[:, g * 64 : (g + 1) * 64],
                            pv[:, g * 65 : g * 65 + 64],
                            rec[:, g : g + 1],
                            1.0 / 1024.0,
                            Mult,
                            Mult,
                        )
                    # one DMA-transpose moves both heads' normalized O[q, dh]
                    # block into O^T[dh, q] inside the per-(hp,qt) staging
                    # tile; after the last block, DVE splits the 512-wide
                    # strip into the fp8 hi/lo pair the outproj consumes
                    nc.sync.dma_start_transpose(otq[:, ts(qb, P)], osb[:])
                    if qt == NQT - 1:
                        # last q-tile: split per block so outproj token
                        # blocks unlock as early as possible (short tail)
                        blk = slice((4 * qt + qb) * P, (4 * qt + qb + 1) * P)
                        ohs = oh8[hp // 2][:, hp % 2, blk]
                        nc.vector.tensor_scalar_mul(ohs, otq[:, ts(qb, P)], 32.0)
                        nc.vector.scalar_tensor_tensor(
                            ol8[hp // 2][:, hp % 2, blk],
                            otq[:, ts(qb, P)],
                            32.0,
                            ohs,
                            Mult,
                            mybir.AluOpType.subtract,
                        )
                        attn_cnt[4 * qt + qb] += 1
                    elif qb == 3:
                        blk = slice(qt * 512, (qt + 1) * 512)
                        ohs = oh8[hp // 2][:, hp % 2, blk]
                        nc.vector.tensor_scalar_mul(ohs, otq[:], 32.0)
                        nc.vector.scalar_tensor_tensor(
                            ol8[hp // 2][:, hp % 2, blk],
                            otq[:],
                            32.0,
                            ohs,
                            Mult,
                            mybir.AluOpType.subtract,
                        )
                        for i in range(4):
                            attn_cnt[4 * qt + i] += 1

                return (2 * (4 * qt + qb + 1) * 65 * 0.42 + 120, fn)

            # ---------------- orchestration ----------------
            # Fine-grained pull model: scores (hp, qt, kb) pull exactly
            # Qproj(qt, hp) and Kproj(kb//4, hp) just in time, so exp work
            # flows to ACT as early as the data allows.  Remaining proj
            # units, PV units and outproj blocks are debt-paced PE filler.
            NVT = TCH // P  # V-proj token-blocks per chunk
            q_done = [[False] * NM for _ in range(NCH)]
            k_done = [[False] * NM for _ in range(NCH)]
            v_done = [[False] * NVT for _ in range(NCH)]

            def chunk_all_done(c):
                return all(q_done[c]) and all(k_done[c]) and all(v_done[c])

            def ensure_x(c):
                if c not in x_dmas_done:
                    # tile-slot hazard: chunk c's x DMA reuses chunk c-2's
                    # buffers; all chunk c-2 readers must be emitted first
                    if c >= 2 and not chunk_all_done(c - 2):
                        pull_chunk(c - 2)
                    emit_x_dma(c)

            def pull_q(c, m):
                if not q_done[c][m]:
                    q_done[c][m] = True
                    ensure_x(c)
                    emit_proj_q_m(c, m)
                    return 1280
                return 0

            def pull_k(c, m):
                if not k_done[c][m]:
                    k_done[c][m] = True
                    ensure_x(c)
                    emit_proj_k_m(c, m)
                    return 1280
                return 0

            def pull_v(c, mt):
                if not v_done[c][mt]:
                    v_done[c][mt] = True
                    ensure_x(c)
                    emit_proj_v_mt(c, mt)
                    return 1280
                return 0

            def pull_chunk(c):
                for m in range(NM):
                    pull_q(c, m)
                for m in range(NM):
                    pull_k(c, m)
                for mt in range(NVT):
                    pull_v(c, mt)

            # workq: debt-paced PE filler (cost_ns, fn); fn returns actual
            # cost (0 if the unit was already pulled directly)
            workq = []
            ogate = {}
            attn_cnt = [0] * NTB  # per token block: heads with split done

            def queue_fillers():
                for c in range(NCH):
                    for m in range(NM):
                        workq.append((f"c{c}", 1280, lambda c=c, m=m: pull_q(c, m)))
                    for m in range(NM):
                        workq.append((f"c{c}", 1280, lambda c=c, m=m: pull_k(c, m)))
                    for mt in range(NVT):
                        workq.append((f"c{c}", 1280, lambda c=c, mt=mt: pull_v(c, mt)))
                for mt in range(NTB):
                    for nt in range(D // 512):
                        def fo(mt=mt, nt=nt):
                            emit_outproj_nt(mt, nt)
                            return 853

                        ogate[id(fo)] = mt
                        workq.append(("o", 640, fo))

            def pop_work(budget_ns):
                spent = 0.0
                i = 0
                while i < len(workq) and spent < budget_ns:
                    kind, cost, fn = workq[i]
                    if kind == "o" and attn_cnt[ogate[id(fn)]] < HG // 2:
                        i += 1
                        continue
                    r = fn()
                    spent += cost if r is None else r
                    workq.pop(i)
                return spent

            # critical-path DMAs first: wq, x chunk 0, wk feed the first
            # scores; wv/wo and chunk 1 follow
            nc.sync.dma_start(wq_t[:, 0:4, :], wq8[:, 0:4, :])
            emit_x_dma(0)
            emit_w_dmas()
            emit_w_dmas_late()
            emit_x_dma(1)
            queue_fillers()
            # PE prewarm: dummy matmuls on the tri constant ramp the tensor
            # engine to full clock while the first weight/x DMAs land
            pwt = big_pool.tile([P, 512], F32, tag="big", name="prewarm")
            for _ in range(36):
                nc.tensor.matmul(
                    pwt[:, 0:P], tri2[:, 0:P], tri2[:, 0:P], start=True, stop=True
                )

            # debt-paced weave: pop a PE filler unit only once the consumer
            # engine's exp backlog exceeds its cost, so the PE stays just
            # behind ACT/DVE.  PV units are emitted inline, one k-block
            # behind the score stream, so nothing drains at the end.
            debt = 0.0
            deferred_pv = []
            groups = [(qt, hp) for qt in range(NQT) for hp in range(HG // 2)]
            for gi, (qt, hp) in enumerate(groups):
                if hp == 0 and qt + 1 < NCH:
                    ensure_x(qt + 1)
                pull_q(qt, hp)
                nkb = 4 * qt + 4
                pexps = []
                osbs = [
                    opool_sb.tile([P, P], BF16, tag=f"osb{qb}", name=f"osb{hp}_{qt}_{qb}")
                    for qb in range(4)
                ]
                otq = ot_pool.tile([P, 512], BF16, tag=f"ot{hp}", name=f"ot{hp}_{qt}")

                def emit_pv(qb, qt=qt, hp=hp, pexps=pexps, osbs=osbs, otq=otq):
                    # vaug writes must be emitted before the PV reads them
                    for j in range(4 * qt + qb + 1):
                        pull_v(j // 4, j % 4)
                    cost, fn = make_pv_unit(hp, qt, qb, pexps, osbs[qb], otq)
                    fn()
                    return cost

                last = gi == len(groups) - 1
                for kb in range(nkb):
                    pull_k(kb // 4, hp)
                    pexps.append(emit_score_kb(hp, qt, kb))
                    if kb == 1 and not last:
                        # prefetch the next group's Q/K so its first score
                        # fires the moment this group's exps are drained
                        nqt, nhp = groups[gi + 1]
                        pull_q(nqt, nhp)
                        pull_k(0, nhp)
                    if kb % 4 == 2 and kb + 2 < nkb:
                        # prefetch the next k-chunk's K projection so its
                        # DVE eviction lands before the scores need it
                        pull_k((kb + 2) // 4, hp)
                    dpv_slot = (2, 3) if nkb > 4 else (0, 1)
                    if kb in dpv_slot and deferred_pv:
                        # previous group's trailing PV blocks: their exps are
                        # long done, so no PE stall and no ACT gap
                        debt -= deferred_pv.pop(0)()
                    w = 2 * (512 - max(kb - 4 * qt, 0) * P)
                    if qt >= 2 and (kb + hp) % SCH_MOD == 0:
                        debt += (w * 1.04 + 170) - (w * 0.21 + 10)
                    else:
                        debt += (w * 0.833 + 242) - (w * 0.21 + 10)
                    if last:
                        continue  # emit the final scores back-to-back
                    qb = kb - 4 * qt - 2
                    if 0 <= qb <= 1:
                        debt -= emit_pv(qb)
                    npops = 0
                    cap = 2
                    while workq and npops < cap:
                        # prefer proj units; spend outproj units only when
                        # nothing else is ready (saves them for the
                        # filler-starved late q-tiles)
                        pick = None
                        for i, (kind, cost, fn) in enumerate(workq):
                            if kind == "o":
                                continue
                            pick = i
                            break
                        if pick is None:
                            for i, (kind, cost, fn) in enumerate(workq):
                                if kind == "o" and attn_cnt[ogate[id(fn)]] >= HG // 2:
                                    pick = i
                                    break
                        if pick is None or (
                            workq[pick][1] > debt
                            and not (last and workq[pick][0] == "o")
                        ):
                            break
                        kind, cost, fn = workq.pop(pick)
                        r = fn()
                        debt -= r if r is not None else cost
                        npops += 1
                if not last:
                    deferred_pv.append(lambda e=emit_pv: e(2))
                    deferred_pv.append(lambda e=emit_pv: e(3))
                else:
                    # tail: PV per block, then its outproj immediately
                    for qb in range(4):
                        emit_pv(qb)
                        pop_work(1 << 30)
            while workq:
                pop_work(1 << 30)
    nc.finalize()
    return nc


_NC_CACHE = {}


def _get_nc():
    if "full" not in _NC_CACHE:
        _NC_CACHE["full"] = build_bass()
    return _NC_CACHE["full"]


def _pack_hilo(mT):
    """[D, C] fp32 -> [128, 4*ND2, C] fp8 hi/lo pack at scale 32.

    sub-index s = 4*j + 2*hl + i: (d-pair j, hi/lo, block i); value
    hi = e4m3(32*x), lo = e4m3(32*x - hi).
    """
    import ml_dtypes

    e4 = ml_dtypes.float8_e4m3
    D, C = mT.shape
    nd = D // P
    nd2 = nd // 2
    blocks = mT.reshape(nd2, 2, P, C)  # [j, i, p, c]
    hi = (32.0 * blocks).astype(e4)
    lo = (32.0 * blocks - hi.astype(np.float32)).astype(e4)
    out = np.empty((P, 4 * nd2, C), dtype=e4)
    for j in range(nd2):
        for i in range(2):
            out[:, 4 * j + i, :] = hi[j, i]
            out[:, 4 * j + 2 + i, :] = lo[j, i]
    return out


def make_in_maps(query, key_value, Wq, Wk, Wv, Wo):
    import ml_dtypes

    query = np.asarray(query, dtype=np.float32)
    key_value = np.asarray(key_value, dtype=np.float32)
    Wq, Wk, Wv, Wo = (np.asarray(w, dtype=np.float32) for w in (Wq, Wk, Wv, Wo))
    GO = Wq.shape[0] // 2
    bf = ml_dtypes.bfloat16
    xq8_b = [_pack_hilo(np.ascontiguousarray(query[b].T)) for b in range(B_FULL)]
    xkv8_b = [_pack_hilo(np.ascontiguousarray(key_value[b].T)) for b in range(B_FULL)]
    w8 = {}
    for g in range(2):
        sl = slice(g * GO, (g + 1) * GO)
        w8[g] = (
            _pack_hilo(np.ascontiguousarray(Wq[sl, :].T)),
            _pack_hilo(np.ascontiguousarray(Wk[sl, :].T)),
            _pack_hilo(np.ascontiguousarray(Wv[sl, :].T)),
            _pack_hilo(np.ascontiguousarray(Wo[:, sl].T)),
        )
    in_maps = []
    for c in range(N_CORES):
        b, g = c // 2, c % 2
        sl = slice(g * GO, (g + 1) * GO)
        in_maps.append(
            {
                "xq8": xq8_b[b],
                "xkv8": xkv8_b[b],
                "wq8": w8[g][0],
                "wk8": w8[g][1],
                "wv8": w8[g][2],
                "wo8": w8[g][3],
            }
        )
    return in_maps


def kernel(query, key_value, Wq, Wk, Wv, Wo):
    from concourse import bass_utils

    nc = _get_nc()
    in_maps = make_in_maps(query, key_value, Wq, Wk, Wv, Wo)
    res = bass_utils.run_bass_kernel_spmd(nc, in_maps, core_ids=list(range(N_CORES)))
    ys = [r["y"] for r in res.results]
    out = np.stack([ys[2 * b] + ys[2 * b + 1] for b in range(B_FULL)])
    return out.astype(np.float32)



# revision 84
# speedup vs baseline: 1.0143x; 1.0066x over previous
"""Causal cross-attention kernel for 8 trn2 NeuronCores.

Sharding: 4-way data-parallel over batch x 2-way tensor-parallel over heads
(8 heads per core).  Per core:
  - Q/K/V/O projections run as fp8e4m3 DoubleRow matmuls (2 k-subtiles per
    instruction at 0.5 cyc/row) with an error-compensated hi/lo split:
    x = xh + xl and w = wh + wl quantized at scale 32, and the three
    products xh*wh + xl*wh + xh*wl accumulate at a common PSUM scale of
    1024, so projection error is below bf16 rounding at 0.75x bf16 cost.
  - Scores are a single one-sided DoubleRow matmul per (head, k-block):
    kT is stored as an exact fp8 hi/lo pair, qT as single fp8, with q
    broadcast across both subtiles (2x bf16 throughput, ~1.5e-2 rel err).
  - Attention in transposed layout: scores^T[k,q] -> exp on ACT (a slice of
    late-q-tile exps uses a bf16 Schraudolph fast-exp on DVE to unload
    ACT) -> stationary-P^T PV step with the 65-wide bf16 V-aug (ones
    column accumulates the softmax denominator).  Normalized O blocks are
    DMA-transposed and split into fp8 hi/lo for the DoubleRow outproj.
  - Orchestration is a fine-grained pull model: scores (hp, qt, kb) pull
    exactly Qproj(qt, hp)/Kproj(kb//4, hp) just in time; PV units emit
    inline one k-block behind the score stream (trailing blocks spill into
    the next group); remaining projection and outproj work is debt-paced
    PE filler between score emissions.

All host-side work (transposes, hi/lo packing) is data marshaling; the
device kernel is a single NEFF launch per core.
"""

import sys

sys.path.insert(0, "/opt/trn_rl_repo")

import numpy as np

import concourse.bass as bass
import concourse.tile as tile
from concourse import bacc, mybir
from concourse.bass import ts
from concourse.masks import make_upper_triangular

F32 = mybir.dt.float32
F32R = mybir.dt.float32r
BF16 = mybir.dt.bfloat16
FP8 = mybir.dt.float8e4
I16 = mybir.dt.int16
P = 128

# full-problem constants
B_FULL = 4
S_FULL = 2048
D_FULL = 1024
HG_FULL = 8  # heads per core (16 heads / 2-way TP)
N_CORES = 8


def build_bass(S=S_FULL, D=D_FULL, HG=HG_FULL):
    """One-core program; SPMD across 8 cores with different data."""
    GO = HG * 64  # output-feature width of this core's head group
    ND = D // P  # d-blocks (contraction)
    NM = GO // P  # o-tiles of Q/K projections
    NQT = S // 512  # q-tiles (512 wide)
    NTB = S // P  # token blocks of 128
    TCH = 512  # projection t-chunk (one q-tile per chunk)
    NCH = S // TCH

    ND2 = ND // 2  # d-block pairs for fp8 DoubleRow
    # fp8 hi/lo packed inputs: sub-index s = 4*j + 2*hl + i selects
    # (d-pair j, hi/lo, d-block within pair); value = e4m3 of 32*x (hi)
    # or 32*x - hi (lo).  PSUM accumulates at scale 1024.
    nc = bacc.Bacc("TRN2", target_bir_lowering=False, debug=False)
    xq8 = nc.dram_tensor("xq8", [P, 4 * ND2, S], FP8, kind="ExternalInput")
    xkv8 = nc.dram_tensor("xkv8", [P, 4 * ND2, S], FP8, kind="ExternalInput")
    wq8 = nc.dram_tensor("wq8", [P, 4 * ND2, GO], FP8, kind="ExternalInput")
    wk8 = nc.dram_tensor("wk8", [P, 4 * ND2, GO], FP8, kind="ExternalInput")
    wv8 = nc.dram_tensor("wv8", [P, 4 * ND2, GO], FP8, kind="ExternalInput")
    wo8 = nc.dram_tensor("wo8", [P, 4 * (GO // P // 2), D], FP8, kind="ExternalInput")
    y = nc.dram_tensor("y", [S, D], F32, kind="ExternalOutput")

    Exp = mybir.ActivationFunctionType.Exp
    Mult = mybir.AluOpType.mult
    Add = mybir.AluOpType.add
    DR = mybir.MatmulPerfMode.DoubleRow
    EXPSCALE = 0.125 / 1024.0  # scores psum = (32q).(32k) = 1024*s
    # bf16 Schraudolph fast-exp for the Pool engine: int16(x*A + B) bitcast
    # to bf16 ~= exp(x) within ~3.6%; softmax renormalization absorbs most
    # of the sawtooth.  A fraction of exp tiles go to Pool to unload ACT.
    SCH_A = 128.0 * 1.4426950408889634
    SCH_B = 16251.0
    SCH_MOD = 6  # 1/6 of late-q-tile exps take the fast-exp path

    with tile.TileContext(nc) as tc:
        from contextlib import ExitStack

        with ExitStack() as ctx:
            ctx.enter_context(
                nc.allow_low_precision(reason="bf16/fp32r matmul input rounding")
            )
            # ---- persistent SBUF buffers ----
            pers = ctx.enter_context(tc.tile_pool(name="pers", bufs=1))
            # qT: e4m3 at scale 32; kT: [hi, lo] e4m3 pair at scale 32
            qT = [pers.tile([P, S], FP8, tag=f"qT{i}", name=f"qT{i}") for i in range(NM)]
            kT = [pers.tile([P, 2, S], FP8, tag=f"kT{i}", name=f"kT{i}") for i in range(NM)]
            vaug = [pers.tile([P, HG * 65], BF16, tag=f"va{i}", name=f"va{i}") for i in range(NTB)]
            # attention output in fp8 hi/lo pairs per ob-pair jj for the
            # DoubleRow output projection; oT blocks are transient
            oh8 = [pers.tile([P, 2, S], FP8, tag=f"oh{j}", name=f"oh{j}") for j in range(NM // 2)]
            ol8 = [pers.tile([P, 2, S], FP8, tag=f"ol{j}", name=f"ol{j}") for j in range(NM // 2)]
            consts = ctx.enter_context(tc.tile_pool(name="consts", bufs=1))
            tri_f = consts.tile([P, P], F32)  # tri[k,q] = 1 if q >= k else 0
            make_upper_triangular(nc, tri_f[:], val=1.0, diag=True)
            # two side-by-side bf16 copies so one strided op masks 2 heads
            tri2 = consts.tile([P, 2 * P], BF16)
            nc.vector.tensor_copy(tri2[:, 0:P], tri_f[:])
            nc.vector.tensor_copy(tri2[:, P : 2 * P], tri_f[:])
            for i in range(NTB):
                # ones columns survive the V evictions (cols h*65+64)
                nc.gpsimd.memset(vaug[i][:], 1.0)

            w_pool = ctx.enter_context(tc.tile_pool(name="wp", bufs=1))
            x_pool = ctx.enter_context(tc.tile_pool(name="xp", bufs=2))
            big_pool = ctx.enter_context(tc.tile_pool(name="big", bufs=2, space="PSUM"))
            spool = ctx.enter_context(tc.tile_pool(name="ps_s", bufs=2, space="PSUM"))
            opool = ctx.enter_context(tc.tile_pool(name="ps_o", bufs=2, space="PSUM"))
            apool = ctx.enter_context(tc.tile_pool(name="att", bufs=2))
            apool2 = ctx.enter_context(tc.tile_pool(name="attn2", bufs=2))
            opool_sb = ctx.enter_context(tc.tile_pool(name="osb", bufs=2))
            y_pool = ctx.enter_context(tc.tile_pool(name="yev", bufs=3))

            wq_t = w_pool.tile([P, 4 * ND2, GO], FP8, tag="wq", name="wq")
            wk_t = w_pool.tile([P, 4 * ND2, GO], FP8, tag="wk", name="wk")
            wv_t = w_pool.tile([P, 4 * ND2, GO], FP8, tag="wv", name="wv")
            wo_t = w_pool.tile([P, 4 * (NM // 2), D], FP8, tag="wo", name="wo")
            ot_pool = ctx.enter_context(tc.tile_pool(name="otp", bufs=2))
            def emit_w_dmas():
                # interleave wq/wk so the Q- and K-proj data paths become
                # ready together (first score waits on the later of the two)
                nc.sync.dma_start(wk_t[:, 0:4, :], wk8[:, 0:4, :])
                for j in range(1, ND2):
                    nc.sync.dma_start(wq_t[:, 4 * j : 4 * j + 4, :], wq8[:, 4 * j : 4 * j + 4, :])
                    nc.sync.dma_start(wk_t[:, 4 * j : 4 * j + 4, :], wk8[:, 4 * j : 4 * j + 4, :])

            def emit_w_dmas_late():
                for j in range(ND2):
                    nc.sync.dma_start(wv_t[:, 4 * j : 4 * j + 4, :], wv8[:, 4 * j : 4 * j + 4, :])
                for j in range(NM // 2):
                    nc.sync.dma_start(wo_t[:, 4 * j : 4 * j + 4, :], wo8[:, 4 * j : 4 * j + 4, :])

            # ---------------- emitter units ----------------
            xq_tiles = {}
            xkv_tiles = {}
            ysb_tiles = {}
            x_dmas_done = set()

            def emit_x_dma(tc_i):
                x_dmas_done.add(tc_i)
                xq = x_pool.tile([P, 4 * ND2, TCH], FP8, tag="xq", name=f"xq_{tc_i}")
                xkv = x_pool.tile([P, 4 * ND2, TCH], FP8, tag="xk", name=f"xkv_{tc_i}")
                for h in range(2):
                    sl = slice(8 * h, 8 * h + 8)
                    nc.sync.dma_start(xq[:, sl, :], xq8[:, sl, ts(tc_i, TCH)])
                    nc.sync.dma_start(xkv[:, sl, :], xkv8[:, sl, ts(tc_i, TCH)])
                xq_tiles[tc_i] = xq
                xkv_tiles[tc_i] = xkv

            def dr3(ps, wt, xt, mcols, j, first, last):
                # 3-term error-compensated fp8 DoubleRow: hi*hi + hi*lo + lo*hi
                wh = wt[:, 4 * j : 4 * j + 2, mcols]
                wl = wt[:, 4 * j + 2 : 4 * j + 4, mcols]
                xh = xt[:, 4 * j : 4 * j + 2, :]
                xl = xt[:, 4 * j + 2 : 4 * j + 4, :]
                nc.tensor.matmul(ps, wh, xh, start=first, stop=False, perf_mode=DR)
                nc.tensor.matmul(ps, wh, xl, start=False, stop=False, perf_mode=DR)
                nc.tensor.matmul(ps, wl, xh, start=False, stop=last, perf_mode=DR)

            def emit_proj_q_m(tc_i, m):
                xq = xq_tiles[tc_i]
                ps = big_pool.tile([P, 512], F32, tag="big", name=f"pq{tc_i}_{m}")
                for j in range(ND2):
                    dr3(ps[:, 0:TCH], wq_t, xq, ts(m, P), j, j == 0, j == ND2 - 1)
                # evict 1024q -> e4m3(32q)
                nc.vector.tensor_scalar_mul(qT[m][:, ts(tc_i, TCH)], ps[:, 0:TCH], 1.0 / 32.0)

            def emit_proj_k_m(tc_i, m):
                xkv = xkv_tiles[tc_i]
                ps = big_pool.tile([P, 512], F32, tag="big", name=f"pk{tc_i}_{m}")
                for j in range(ND2):
                    dr3(ps[:, 0:TCH], wk_t, xkv, ts(m, P), j, j == 0, j == ND2 - 1)
                # evict 1024k -> hi = e4m3(32k), lo = e4m3(32k - hi)
                nc.vector.tensor_scalar_mul(kT[m][:, 0, ts(tc_i, TCH)], ps[:, 0:TCH], 1.0 / 32.0)
                nc.vector.scalar_tensor_tensor(
                    kT[m][:, 1, ts(tc_i, TCH)],
                    ps[:, 0:TCH],
                    1.0 / 32.0,
                    kT[m][:, 0, ts(tc_i, TCH)],
                    Mult,
                    mybir.AluOpType.subtract,
                )

            def emit_proj_v_mt(tc_i, mt):
                xkv = xkv_tiles[tc_i]
                ps = big_pool.tile([P, 512], F32, tag="big", name=f"pv{tc_i}_{mt}")
                for j in range(ND2):
                    xh = xkv[:, 4 * j : 4 * j + 2, ts(mt, P)]
                    xl = xkv[:, 4 * j + 2 : 4 * j + 4, ts(mt, P)]
                    wh = wv_t[:, 4 * j : 4 * j + 2, :]
                    wl = wv_t[:, 4 * j + 2 : 4 * j + 4, :]
                    nc.tensor.matmul(ps[:], xh, wh, start=(j == 0), stop=False, perf_mode=DR)
                    nc.tensor.matmul(ps[:], xl, wh, start=False, stop=False, perf_mode=DR)
                    nc.tensor.matmul(ps[:], xh, wl, start=False, stop=(j == ND2 - 1), perf_mode=DR)
                vt = vaug[tc_i * (TCH // P) + mt]
                nc.vector.tensor_copy(
                    vt[:].rearrange("p (h c) -> p h c", c=65)[:, :, 0:64],
                    ps[:].rearrange("p (h c) -> p h c", c=64),
                )

            def emit_proj_q(tc_i):
                for m in range(NM):
                    emit_proj_q_m(tc_i, m)

            def emit_proj_k(tc_i):
                for m in range(NM):
                    emit_proj_k_m(tc_i, m)

            def emit_proj_v(tc_i):
                for mt in range(TCH // P):
                    emit_proj_v_mt(tc_i, mt)
                del xq_tiles[tc_i], xkv_tiles[tc_i]

            def emit_outproj_nt(mt, nt):
                # fp8 DoubleRow 3-term: psum = 1024*y over ob-pairs jj
                ps = big_pool.tile([P, 512], F32, tag="big", name=f"y{mt}_{nt}")
                NJ = NM // 2
                for jj in range(NJ):
                    oh = oh8[jj][:, :, ts(mt, P)]
                    ol = ol8[jj][:, :, ts(mt, P)]
                    wh = wo_t[:, 4 * jj : 4 * jj + 2, ts(nt, 512)]
                    wl = wo_t[:, 4 * jj + 2 : 4 * jj + 4, ts(nt, 512)]
                    nc.tensor.matmul(ps[:], oh, wh, start=(jj == 0), stop=False, perf_mode=DR)
                    nc.tensor.matmul(ps[:], ol, wh, start=False, stop=False, perf_mode=DR)
                    nc.tensor.matmul(ps[:], oh, wl, start=False, stop=(jj == NJ - 1), perf_mode=DR)
                ysb = y_pool.tile([P, 512], F32, tag="ysb", name=f"ysb{mt}_{nt}")
                nc.vector.tensor_scalar_mul(ysb[:], ps[:], 1.0 / 1024.0)
                nc.sync.dma_start(y[ts(mt, P), ts(nt, 512)], ysb[:])

            def emit_score_kb(hp, qt, kb):
                j = kb - 4 * qt
                ce = max(j, 0) * P
                # both heads' scores in one 2-bank PSUM tile so a single
                # strided activation does both exps
                pss = spool.tile([P, 1024], F32, tag="s", name=f"s{hp}_{qt}_{kb}")
                w1 = 512 - ce
                for g, po in ((0, 0), (1, 64)):
                    # one fp8 DoubleRow matmul: (kh + kl) . q, q broadcast
                    qv = (
                        qT[hp][po : po + 64, qt * 512 + ce : (qt + 1) * 512]
                        .unsqueeze(1)
                        .broadcast_to([64, 2, w1])
                    )
                    nc.tensor.matmul(
                        pss[:, g * 512 + ce : (g + 1) * 512],
                        kT[hp][po : po + 64, :, ts(kb, P)],
                        qv,
                        start=True,
                        stop=True,
                        perf_mode=DR,
                    )
                pexp = apool.tile([P, 1024], BF16, tag=f"p{kb}", name=f"p{hp}_{qt}_{kb}")
                out_ap = pexp[:].rearrange("p (g c) -> p g c", g=2)[:, :, ce:]
                in_ap = pss[:].rearrange("p (g c) -> p g c", g=2)[:, :, ce:]
                if qt >= 2 and (kb + hp) % SCH_MOD == 0:
                    # DVE fast-exp, only in the ACT-bound late q-tiles
                    nc.vector.tensor_scalar(
                        out_ap.bitcast(I16), in_ap, SCH_A * EXPSCALE, SCH_B, Mult, Add
                    )
                else:
                    nc.scalar.activation(out_ap, in_ap, Exp, scale=EXPSCALE)
                if j >= 0:
                    # mask the boundary block for both heads in one op
                    nc.gpsimd.tensor_tensor(
                        pexp[:].rearrange("p (g c) -> p g c", g=2)[:, :, ts(j, P)],
                        pexp[:].rearrange("p (g c) -> p g c", g=2)[:, :, ts(j, P)],
                        tri2[:].rearrange("p (g c) -> p g c", g=2),
                        Mult,
                    )
                return pexp



            def make_pv_unit(hp, qt, qb, pexps, osb, otq):
                def fn():
                    # both heads' PV accumulation groups, sequentially, into
                    # one [128, 130] PSUM tile (cols h*65+64 = denominators);
                    # each group runs start-to-stop before the next opens
                    # (2KB PSUM zero-region rule).
                    pv = opool.tile([P, 130], F32, tag="pv", name=f"pv{hp}_{qt}_{qb}")
                    for g, hh in ((0, 2 * hp), (1, 2 * hp + 1)):
                        for kb in range(4 * qt + qb + 1):
                            nc.tensor.matmul(
                                pv[:, g * 65 : g * 65 + 65],
                                pexps[kb][:, g * 512 + qb * P : g * 512 + (qb + 1) * P],
                                vaug[kb][:, hh * 65 : hh * 65 + 65],
                                start=(kb == 0),
                                stop=(kb == 4 * qt + qb),
                            )
                    rec = apool2.tile([P, 2], F32, tag="rec", name=f"rec{hp}_{qt}_{qb}")
                    nc.vector.reciprocal(
                        rec[:].rearrange("p (g c) -> p g c", c=1),
                        pv[:].rearrange("p (g c) -> p g c", c=65)[:, :, 64:65],
                    )
                    for g in (0, 1):
                        # pv holds sum(p * 1024*v); rescale by 1/1024 here
                        nc.vector.tensor_scalar(
                            osb[:, g * 64 : (g + 1) * 64],
                            pv[:, g * 65 : g * 65 + 64],
                            rec[:, g : g + 1],
                            1.0 / 1024.0,
                            Mult,
                            Mult,
                        )
                    # one DMA-transpose moves both heads' normalized O[q, dh]
                    # block into O^T[dh, q] inside the per-(hp,qt) staging
                    # tile; after the last block, DVE splits the 512-wide
                    # strip into the fp8 hi/lo pair the outproj consumes
                    nc.sync.dma_start_transpose(otq[:, ts(qb, P)], osb[:])
                    if qt == NQT - 1:
                        # last q-tile: split per block so outproj token
                        # blocks unlock as early as possible (short tail)
                        blk = slice((4 * qt + qb) * P, (4 * qt + qb + 1) * P)
                        ohs = oh8[hp // 2][:, hp % 2, blk]
                        nc.vector.tensor_scalar_mul(ohs, otq[:, ts(qb, P)], 32.0)
                        nc.vector.scalar_tensor_tensor(
                            ol8[hp // 2][:, hp % 2, blk],
                            otq[:, ts(qb, P)],
                            32.0,
                            ohs,
                            Mult,
                            mybir.AluOpType.subtract,
                        )
                        attn_cnt[4 * qt + qb] += 1
                    elif qb == 3:
                        blk = slice(qt * 512, (qt + 1) * 512)
                        ohs = oh8[hp // 2][:, hp % 2, blk]
                        nc.vector.tensor_scalar_mul(ohs, otq[:], 32.0)
                        nc.vector.scalar_tensor_tensor(
                            ol8[hp // 2][:, hp % 2, blk],
                            otq[:],
                            32.0,
                            ohs,
                            Mult,
                            mybir.AluOpType.subtract,
                        )
                        for i in range(4):
                            attn_cnt[4 * qt + i] += 1

                return (2 * (4 * qt + qb + 1) * 65 * 0.42 + 120, fn)

            # ---------------- orchestration ----------------
            # Fine-grained pull model: scores (hp, qt, kb) pull exactly
            # Qproj(qt, hp) and Kproj(kb//4, hp) just in time, so exp work
            # flows to ACT as early as the data allows.  Remaining proj
            # units, PV units and outproj blocks are debt-paced PE filler.
            NVT = TCH // P  # V-proj token-blocks per chunk
            q_done = [[False] * NM for _ in range(NCH)]
            k_done = [[False] * NM for _ in range(NCH)]
            v_done = [[False] * NVT for _ in range(NCH)]

            def chunk_all_done(c):
                return all(q_done[c]) and all(k_done[c]) and all(v_done[c])

            def ensure_x(c):
                if c not in x_dmas_done:
                    # tile-slot hazard: chunk c's x DMA reuses chunk c-2's
                    # buffers; all chunk c-2 readers must be emitted first
                    if c >= 2 and not chunk_all_done(c - 2):
                        pull_chunk(c - 2)
                    emit_x_dma(c)

            def pull_q(c, m):
                if not q_done[c][m]:
                    q_done[c][m] = True
                    ensure_x(c)
                    emit_proj_q_m(c, m)
                    return 1280
                return 0

            def pull_k(c, m):
                if not k_done[c][m]:
                    k_done[c][m] = True
                    ensure_x(c)
                    emit_proj_k_m(c, m)
                    return 1280
                return 0

            def pull_v(c, mt):
                if not v_done[c][mt]:
                    v_done[c][mt] = True
                    ensure_x(c)
                    emit_proj_v_mt(c, mt)
                    return 1280
                return 0

            def pull_chunk(c):
                for m in range(NM):
                    pull_q(c, m)
                for m in range(NM):
                    pull_k(c, m)
                for mt in range(NVT):
                    pull_v(c, mt)

            # workq: debt-paced PE filler (cost_ns, fn); fn returns actual
            # cost (0 if the unit was already pulled directly)
            workq = []
            ogate = {}
            attn_cnt = [0] * NTB  # per token block: heads with split done

            def queue_fillers():
                for c in range(NCH):
                    for m in range(NM):
                        workq.append((f"c{c}", 1280, lambda c=c, m=m: pull_q(c, m)))
                    for m in range(NM):
                        workq.append((f"c{c}", 1280, lambda c=c, m=m: pull_k(c, m)))
                    for mt in range(NVT):
                        workq.append((f"c{c}", 1280, lambda c=c, mt=mt: pull_v(c, mt)))
                for mt in range(NTB):
                    for nt in range(D // 512):
                        def fo(mt=mt, nt=nt):
                            emit_outproj_nt(mt, nt)
                            return 853

                        ogate[id(fo)] = mt
                        workq.append(("o", 640, fo))

            def pop_work(budget_ns):
                spent = 0.0
                i = 0
                while i < len(workq) and spent < budget_ns:
                    kind, cost, fn = workq[i]
                    if kind == "o" and attn_cnt[ogate[id(fn)]] < HG // 2:
                        i += 1
                        continue
                    r = fn()
                    spent += cost if r is None else r
                    workq.pop(i)
                return spent

            # critical-path DMAs first: wq, x chunk 0, wk feed the first
            # scores; wv/wo and chunk 1 follow
            nc.sync.dma_start(wq_t[:, 0:4, :], wq8[:, 0:4, :])
            emit_x_dma(0)
            emit_w_dmas()
            emit_w_dmas_late()
            emit_x_dma(1)
            queue_fillers()
            # PE prewarm: dummy matmuls on the tri constant ramp the tensor
            # engine to full clock while the first weight/x DMAs land
            pwt = big_pool.tile([P, 512], F32, tag="big", name="prewarm")
            for _ in range(36):
                nc.tensor.matmul(
                    pwt[:, 0:P], tri2[:, 0:P], tri2[:, 0:P], start=True, stop=True
                )

            # debt-paced weave: pop a PE filler unit only once the consumer
            # engine's exp backlog exceeds its cost, so the PE stays just
            # behind ACT/DVE.  PV units are emitted inline, one k-block
            # behind the score stream, so nothing drains at the end.
            debt = 0.0
            deferred_pv = []
            groups = [(qt, hp) for qt in range(NQT) for hp in range(HG // 2)]
            for gi, (qt, hp) in enumerate(groups):
                if hp == 0 and qt + 1 < NCH:
                    ensure_x(qt + 1)
                pull_q(qt, hp)
                nkb = 4 * qt + 4
                pexps = []
                osbs = [
                    opool_sb.tile([P, P], BF16, tag=f"osb{qb}", name=f"osb{hp}_{qt}_{qb}")
                    for qb in range(4)
                ]
                otq = ot_pool.tile([P, 512], BF16, tag=f"ot{hp}", name=f"ot{hp}_{qt}")

                def emit_pv(qb, qt=qt, hp=hp, pexps=pexps, osbs=osbs, otq=otq):
                    # vaug writes must be emitted before the PV reads them
                    for j in range(4 * qt + qb + 1):
                        pull_v(j // 4, j % 4)
                    cost, fn = make_pv_unit(hp, qt, qb, pexps, osbs[qb], otq)
                    fn()
                    return cost

                last = gi == len(groups) - 1
                for kb in range(nkb):
                    pull_k(kb // 4, hp)
                    pexps.append(emit_score_kb(hp, qt, kb))
                    if kb == 1 and not last:
                        # prefetch the next group's Q/K so its first score
                        # fires the moment this group's exps are drained
                        nqt, nhp = groups[gi + 1]
                        pull_q(nqt, nhp)
                        pull_k(0, nhp)
                    if kb % 4 == 2 and kb + 2 < nkb:
                        # prefetch the next k-chunk's K projection so its
                        # DVE eviction lands before the scores need it
                        pull_k((kb + 2) // 4, hp)
                    dpv_slot = (4, 5) if nkb > 8 else ((2, 3) if nkb > 4 else (0, 1))
                    if kb in dpv_slot and deferred_pv:
                        # previous group's trailing PV blocks: their exps are
                        # long done, so no PE stall and no ACT gap
                        debt -= deferred_pv.pop(0)()
                    w = 2 * (512 - max(kb - 4 * qt, 0) * P)
                    if qt >= 2 and (kb + hp) % SCH_MOD == 0:
                        debt += (w * 1.04 + 170) - (w * 0.21 + 10)
                    else:
                        debt += (w * 0.833 + 242) - (w * 0.21 + 10)
                    if last:
                        continue  # emit the final scores back-to-back
                    qb = kb - 4 * qt - 2
                    if 0 <= qb <= 1:
                        debt -= emit_pv(qb)
                    npops = 0
                    cap = 2
                    while workq and npops < cap:
                        # prefer proj units; spend outproj units only when
                        # nothing else is ready (saves them for the
                        # filler-starved late q-tiles)
                        pick = None
                        for i, (kind, cost, fn) in enumerate(workq):
                            if kind == "o":
                                continue
                            pick = i
                            break
                        if pick is None:
                            for i, (kind, cost, fn) in enumerate(workq):
                                if kind == "o" and attn_cnt[ogate[id(fn)]] >= HG // 2:
                                    pick = i
                                    break
                        if pick is None or (
                            workq[pick][1] > debt
                            and not (last and workq[pick][0] == "o")
                        ):
                            break
                        kind, cost, fn = workq.pop(pick)
                        r = fn()
                        debt -= r if r is not None else cost
                        npops += 1
                if not last:
                    deferred_pv.append(lambda e=emit_pv: e(2))
                    deferred_pv.append(lambda e=emit_pv: e(3))
                else:
                    # tail: PV per block, then its outproj immediately
                    for qb in range(4):
                        emit_pv(qb)
                        pop_work(1 << 30)
            while workq:
                pop_work(1 << 30)
    nc.finalize()
    return nc


_NC_CACHE = {}


def _get_nc():
    if "full" not in _NC_CACHE:
        _NC_CACHE["full"] = build_bass()
    return _NC_CACHE["full"]


def _pack_hilo(mT):
    """[D, C] fp32 -> [128, 4*ND2, C] fp8 hi/lo pack at scale 32.

    sub-index s = 4*j + 2*hl + i: (d-pair j, hi/lo, block i); value
    hi = e4m3(32*x), lo = e4m3(32*x - hi).
    """
    import ml_dtypes

    e4 = ml_dtypes.float8_e4m3
    D, C = mT.shape
    nd = D // P
    nd2 = nd // 2
    blocks = mT.reshape(nd2, 2, P, C)  # [j, i, p, c]
    hi = (32.0 * blocks).astype(e4)
    lo = (32.0 * blocks - hi.astype(np.float32)).astype(e4)
    out = np.empty((P, 4 * nd2, C), dtype=e4)
    for j in range(nd2):
        for i in range(2):
            out[:, 4 * j + i, :] = hi[j, i]
            out[:, 4 * j + 2 + i, :] = lo[j, i]
    return out


def make_in_maps(query, key_value, Wq, Wk, Wv, Wo):
    import ml_dtypes

    query = np.asarray(query, dtype=np.float32)
    key_value = np.asarray(key_value, dtype=np.float32)
    Wq, Wk, Wv, Wo = (np.asarray(w, dtype=np.float32) for w in (Wq, Wk, Wv, Wo))
    GO = Wq.shape[0] // 2
    bf = ml_dtypes.bfloat16
    xq8_b = [_pack_hilo(np.ascontiguousarray(query[b].T)) for b in range(B_FULL)]
    xkv8_b = [_pack_hilo(np.ascontiguousarray(key_value[b].T)) for b in range(B_FULL)]
    w8 = {}
    for g in range(2):
        sl = slice(g * GO, (g + 1) * GO)
        w8[g] = (
            _pack_hilo(np.ascontiguousarray(Wq[sl, :].T)),
            _pack_hilo(np.ascontiguousarray(Wk[sl, :].T)),
            _pack_hilo(np.ascontiguousarray(Wv[sl, :].T)),
            _pack_hilo(np.ascontiguousarray(Wo[:, sl].T)),
        )
    in_maps = []
    for c in range(N_CORES):
        b, g = c // 2, c % 2
        sl = slice(g * GO, (g + 1) * GO)
        in_maps.append(
            {
                "xq8": xq8_b[b],
                "xkv8": xkv8_b[b],
                "wq8": w8[g][0],
                "wk8": w8[g][1],
                "wv8": w8[g][2],
                "wo8": w8[g][3],
            }
        )
    return in_maps


def kernel(query, key_value, Wq, Wk, Wv, Wo):
    from concourse import bass_utils

    nc = _get_nc()
    in_maps = make_in_maps(query, key_value, Wq, Wk, Wv, Wo)
    res = bass_utils.run_bass_kernel_spmd(nc, in_maps, core_ids=list(range(N_CORES)))
    ys = [r["y"] for r in res.results]
    out = np.stack([ys[2 * b] + ys[2 * b + 1] for b in range(B_FULL)])
    return out.astype(np.float32)



# revision 85
# speedup vs baseline: 1.0173x; 1.0030x over previous
"""Causal cross-attention kernel for 8 trn2 NeuronCores.

Sharding: 4-way data-parallel over batch x 2-way tensor-parallel over heads
(8 heads per core).  Per core:
  - Q/K/V/O projections run as fp8e4m3 DoubleRow matmuls (2 k-subtiles per
    instruction at 0.5 cyc/row) with an error-compensated hi/lo split:
    x = xh + xl and w = wh + wl quantized at scale 32, and the three
    products xh*wh + xl*wh + xh*wl accumulate at a common PSUM scale of
    1024, so projection error is below bf16 rounding at 0.75x bf16 cost.
  - Scores are a single one-sided DoubleRow matmul per (head, k-block):
    kT is stored as an exact fp8 hi/lo pair, qT as single fp8, with q
    broadcast across both subtiles (2x bf16 throughput, ~1.5e-2 rel err).
  - Attention in transposed layout: scores^T[k,q] -> exp on ACT (a slice of
    late-q-tile exps uses a bf16 Schraudolph fast-exp on DVE to unload
    ACT) -> stationary-P^T PV step with the 65-wide bf16 V-aug (ones
    column accumulates the softmax denominator).  Normalized O blocks are
    DMA-transposed and split into fp8 hi/lo for the DoubleRow outproj.
  - Orchestration is a fine-grained pull model: scores (hp, qt, kb) pull
    exactly Qproj(qt, hp)/Kproj(kb//4, hp) just in time; PV units emit
    inline one k-block behind the score stream (trailing blocks spill into
    the next group); remaining projection and outproj work is debt-paced
    PE filler between score emissions.

All host-side work (transposes, hi/lo packing) is data marshaling; the
device kernel is a single NEFF launch per core.
"""

import sys

sys.path.insert(0, "/opt/trn_rl_repo")

import numpy as np

import concourse.bass as bass
import concourse.tile as tile
from concourse import bacc, mybir
from concourse.bass import ts
from concourse.masks import make_upper_triangular

F32 = mybir.dt.float32
F32R = mybir.dt.float32r
BF16 = mybir.dt.bfloat16
FP8 = mybir.dt.float8e4
I16 = mybir.dt.int16
P = 128

# full-problem constants
B_FULL = 4
S_FULL = 2048
D_FULL = 1024
HG_FULL = 8  # heads per core (16 heads / 2-way TP)
N_CORES = 8


def build_bass(S=S_FULL, D=D_FULL, HG=HG_FULL):
    """One-core program; SPMD across 8 cores with different data."""
    GO = HG * 64  # output-feature width of this core's head group
    ND = D // P  # d-blocks (contraction)
    NM = GO // P  # o-tiles of Q/K projections
    NQT = S // 512  # q-tiles (512 wide)
    NTB = S // P  # token blocks of 128
    TCH = 512  # projection t-chunk (one q-tile per chunk)
    NCH = S // TCH

    ND2 = ND // 2  # d-block pairs for fp8 DoubleRow
    # fp8 hi/lo packed inputs: sub-index s = 4*j + 2*hl + i selects
    # (d-pair j, hi/lo, d-block within pair); value = e4m3 of 32*x (hi)
    # or 32*x - hi (lo).  PSUM accumulates at scale 1024.
    nc = bacc.Bacc("TRN2", target_bir_lowering=False, debug=False)
    xq8 = nc.dram_tensor("xq8", [P, 4 * ND2, S], FP8, kind="ExternalInput")
    xkv8 = nc.dram_tensor("xkv8", [P, 4 * ND2, S], FP8, kind="ExternalInput")
    wq8 = nc.dram_tensor("wq8", [P, 4 * ND2, GO], FP8, kind="ExternalInput")
    wk8 = nc.dram_tensor("wk8", [P, 4 * ND2, GO], FP8, kind="ExternalInput")
    wv8 = nc.dram_tensor("wv8", [P, 4 * ND2, GO], FP8, kind="ExternalInput")
    wo8 = nc.dram_tensor("wo8", [P, 4 * (GO // P // 2), D], FP8, kind="ExternalInput")
    y = nc.dram_tensor("y", [S, D], F32, kind="ExternalOutput")

    Exp = mybir.ActivationFunctionType.Exp
    Mult = mybir.AluOpType.mult
    Add = mybir.AluOpType.add
    DR = mybir.MatmulPerfMode.DoubleRow
    EXPSCALE = 0.125 / 1024.0  # scores psum = (32q).(32k) = 1024*s
    # bf16 Schraudolph fast-exp for the Pool engine: int16(x*A + B) bitcast
    # to bf16 ~= exp(x) within ~3.6%; softmax renormalization absorbs most
    # of the sawtooth.  A fraction of exp tiles go to Pool to unload ACT.
    SCH_A = 128.0 * 1.4426950408889634
    SCH_B = 16251.0
    SCH_MOD = 6  # 1/6 of late-q-tile exps take the fast-exp path

    with tile.TileContext(nc) as tc:
        from contextlib import ExitStack

        with ExitStack() as ctx:
            ctx.enter_context(
                nc.allow_low_precision(reason="bf16/fp32r matmul input rounding")
            )
            # ---- persistent SBUF buffers ----
            pers = ctx.enter_context(tc.tile_pool(name="pers", bufs=1))
            # qT: e4m3 at scale 32; kT: [hi, lo] e4m3 pair at scale 32
            qT = [pers.tile([P, S], FP8, tag=f"qT{i}", name=f"qT{i}") for i in range(NM)]
            kT = [pers.tile([P, 2, S], FP8, tag=f"kT{i}", name=f"kT{i}") for i in range(NM)]
            vaug = [pers.tile([P, HG * 65], BF16, tag=f"va{i}", name=f"va{i}") for i in range(NTB)]
            # attention output in fp8 hi/lo pairs per ob-pair jj for the
            # DoubleRow output projection; oT blocks are transient
            oh8 = [pers.tile([P, 2, S], FP8, tag=f"oh{j}", name=f"oh{j}") for j in range(NM // 2)]
            ol8 = [pers.tile([P, 2, S], FP8, tag=f"ol{j}", name=f"ol{j}") for j in range(NM // 2)]
            consts = ctx.enter_context(tc.tile_pool(name="consts", bufs=1))
            tri_f = consts.tile([P, P], F32)  # tri[k,q] = 1 if q >= k else 0
            make_upper_triangular(nc, tri_f[:], val=1.0, diag=True)
            # two side-by-side bf16 copies so one strided op masks 2 heads
            tri2 = consts.tile([P, 2 * P], BF16)
            nc.vector.tensor_copy(tri2[:, 0:P], tri_f[:])
            nc.vector.tensor_copy(tri2[:, P : 2 * P], tri_f[:])
            for i in range(NTB):
                # ones columns survive the V evictions (cols h*65+64)
                nc.gpsimd.memset(vaug[i][:], 1.0)

            w_pool = ctx.enter_context(tc.tile_pool(name="wp", bufs=1))
            x_pool = ctx.enter_context(tc.tile_pool(name="xp", bufs=2))
            big_pool = ctx.enter_context(tc.tile_pool(name="big", bufs=2, space="PSUM"))
            spool = ctx.enter_context(tc.tile_pool(name="ps_s", bufs=2, space="PSUM"))
            opool = ctx.enter_context(tc.tile_pool(name="ps_o", bufs=2, space="PSUM"))
            apool = ctx.enter_context(tc.tile_pool(name="att", bufs=2))
            apool2 = ctx.enter_context(tc.tile_pool(name="attn2", bufs=2))
            opool_sb = ctx.enter_context(tc.tile_pool(name="osb", bufs=2))
            y_pool = ctx.enter_context(tc.tile_pool(name="yev", bufs=3))

            wq_t = w_pool.tile([P, 4 * ND2, GO], FP8, tag="wq", name="wq")
            wk_t = w_pool.tile([P, 4 * ND2, GO], FP8, tag="wk", name="wk")
            wv_t = w_pool.tile([P, 4 * ND2, GO], FP8, tag="wv", name="wv")
            wo_t = w_pool.tile([P, 4 * (NM // 2), D], FP8, tag="wo", name="wo")
            ot_pool = ctx.enter_context(tc.tile_pool(name="otp", bufs=2))
            def emit_w_dmas():
                # interleave wq/wk so the Q- and K-proj data paths become
                # ready together (first score waits on the later of the two)
                nc.sync.dma_start(wk_t[:, 0:4, :], wk8[:, 0:4, :])
                for j in range(1, ND2):
                    nc.sync.dma_start(wq_t[:, 4 * j : 4 * j + 4, :], wq8[:, 4 * j : 4 * j + 4, :])
                    nc.sync.dma_start(wk_t[:, 4 * j : 4 * j + 4, :], wk8[:, 4 * j : 4 * j + 4, :])

            def emit_w_dmas_late():
                for j in range(ND2):
                    nc.sync.dma_start(wv_t[:, 4 * j : 4 * j + 4, :], wv8[:, 4 * j : 4 * j + 4, :])
                for j in range(NM // 2):
                    nc.sync.dma_start(wo_t[:, 4 * j : 4 * j + 4, :], wo8[:, 4 * j : 4 * j + 4, :])

            # ---------------- emitter units ----------------
            xq_tiles = {}
            xkv_tiles = {}
            ysb_tiles = {}
            x_dmas_done = set()

            def emit_x_dma(tc_i):
                x_dmas_done.add(tc_i)
                xq = x_pool.tile([P, 4 * ND2, TCH], FP8, tag="xq", name=f"xq_{tc_i}")
                xkv = x_pool.tile([P, 4 * ND2, TCH], FP8, tag="xk", name=f"xkv_{tc_i}")
                for h in range(2):
                    sl = slice(8 * h, 8 * h + 8)
                    nc.sync.dma_start(xq[:, sl, :], xq8[:, sl, ts(tc_i, TCH)])
                    nc.sync.dma_start(xkv[:, sl, :], xkv8[:, sl, ts(tc_i, TCH)])
                xq_tiles[tc_i] = xq
                xkv_tiles[tc_i] = xkv

            def dr3(ps, wt, xt, mcols, j, first, last):
                # 3-term error-compensated fp8 DoubleRow: hi*hi + hi*lo + lo*hi
                wh = wt[:, 4 * j : 4 * j + 2, mcols]
                wl = wt[:, 4 * j + 2 : 4 * j + 4, mcols]
                xh = xt[:, 4 * j : 4 * j + 2, :]
                xl = xt[:, 4 * j + 2 : 4 * j + 4, :]
                nc.tensor.matmul(ps, wh, xh, start=first, stop=False, perf_mode=DR)
                nc.tensor.matmul(ps, wh, xl, start=False, stop=False, perf_mode=DR)
                nc.tensor.matmul(ps, wl, xh, start=False, stop=last, perf_mode=DR)

            def emit_proj_q_m(tc_i, m):
                xq = xq_tiles[tc_i]
                ps = big_pool.tile([P, 512], F32, tag="big", name=f"pq{tc_i}_{m}")
                for j in range(ND2):
                    dr3(ps[:, 0:TCH], wq_t, xq, ts(m, P), j, j == 0, j == ND2 - 1)
                # evict 1024q -> e4m3(32q)
                nc.vector.tensor_scalar_mul(qT[m][:, ts(tc_i, TCH)], ps[:, 0:TCH], 1.0 / 32.0)

            def emit_proj_k_m(tc_i, m):
                xkv = xkv_tiles[tc_i]
                ps = big_pool.tile([P, 512], F32, tag="big", name=f"pk{tc_i}_{m}")
                for j in range(ND2):
                    dr3(ps[:, 0:TCH], wk_t, xkv, ts(m, P), j, j == 0, j == ND2 - 1)
                # evict 1024k -> hi = e4m3(32k), lo = e4m3(32k - hi)
                nc.vector.tensor_scalar_mul(kT[m][:, 0, ts(tc_i, TCH)], ps[:, 0:TCH], 1.0 / 32.0)
                nc.vector.scalar_tensor_tensor(
                    kT[m][:, 1, ts(tc_i, TCH)],
                    ps[:, 0:TCH],
                    1.0 / 32.0,
                    kT[m][:, 0, ts(tc_i, TCH)],
                    Mult,
                    mybir.AluOpType.subtract,
                )

            def emit_proj_v_mt(tc_i, mt):
                xkv = xkv_tiles[tc_i]
                ps = big_pool.tile([P, 512], F32, tag="big", name=f"pv{tc_i}_{mt}")
                for j in range(ND2):
                    xh = xkv[:, 4 * j : 4 * j + 2, ts(mt, P)]
                    xl = xkv[:, 4 * j + 2 : 4 * j + 4, ts(mt, P)]
                    wh = wv_t[:, 4 * j : 4 * j + 2, :]
                    wl = wv_t[:, 4 * j + 2 : 4 * j + 4, :]
                    nc.tensor.matmul(ps[:], xh, wh, start=(j == 0), stop=False, perf_mode=DR)
                    nc.tensor.matmul(ps[:], xl, wh, start=False, stop=False, perf_mode=DR)
                    nc.tensor.matmul(ps[:], xh, wl, start=False, stop=(j == ND2 - 1), perf_mode=DR)
                vt = vaug[tc_i * (TCH // P) + mt]
                nc.vector.tensor_copy(
                    vt[:].rearrange("p (h c) -> p h c", c=65)[:, :, 0:64],
                    ps[:].rearrange("p (h c) -> p h c", c=64),
                )

            def emit_proj_q(tc_i):
                for m in range(NM):
                    emit_proj_q_m(tc_i, m)

            def emit_proj_k(tc_i):
                for m in range(NM):
                    emit_proj_k_m(tc_i, m)

            def emit_proj_v(tc_i):
                for mt in range(TCH // P):
                    emit_proj_v_mt(tc_i, mt)
                del xq_tiles[tc_i], xkv_tiles[tc_i]

            def emit_outproj_nt(mt, nt):
                # fp8 DoubleRow 3-term: psum = 1024*y over ob-pairs jj
                ps = big_pool.tile([P, 512], F32, tag="big", name=f"y{mt}_{nt}")
                NJ = NM // 2
                for jj in range(NJ):
                    oh = oh8[jj][:, :, ts(mt, P)]
                    ol = ol8[jj][:, :, ts(mt, P)]
                    wh = wo_t[:, 4 * jj : 4 * jj + 2, ts(nt, 512)]
                    wl = wo_t[:, 4 * jj + 2 : 4 * jj + 4, ts(nt, 512)]
                    nc.tensor.matmul(ps[:], oh, wh, start=(jj == 0), stop=False, perf_mode=DR)
                    nc.tensor.matmul(ps[:], ol, wh, start=False, stop=False, perf_mode=DR)
                    nc.tensor.matmul(ps[:], oh, wl, start=False, stop=(jj == NJ - 1), perf_mode=DR)
                ysb = y_pool.tile([P, 512], F32, tag="ysb", name=f"ysb{mt}_{nt}")
                nc.vector.tensor_scalar_mul(ysb[:], ps[:], 1.0 / 1024.0)
                nc.sync.dma_start(y[ts(mt, P), ts(nt, 512)], ysb[:])

            def emit_score_kb(hp, qt, kb):
                j = kb - 4 * qt
                ce = max(j, 0) * P
                # both heads' scores in one 2-bank PSUM tile so a single
                # strided activation does both exps
                pss = spool.tile([P, 1024], F32, tag="s", name=f"s{hp}_{qt}_{kb}")
                w1 = 512 - ce
                for g, po in ((0, 0), (1, 64)):
                    # one fp8 DoubleRow matmul: (kh + kl) . q, q broadcast
                    qv = (
                        qT[hp][po : po + 64, qt * 512 + ce : (qt + 1) * 512]
                        .unsqueeze(1)
                        .broadcast_to([64, 2, w1])
                    )
                    nc.tensor.matmul(
                        pss[:, g * 512 + ce : (g + 1) * 512],
                        kT[hp][po : po + 64, :, ts(kb, P)],
                        qv,
                        start=True,
                        stop=True,
                        perf_mode=DR,
                    )
                pexp = apool.tile([P, 1024], BF16, tag=f"p{kb}", name=f"p{hp}_{qt}_{kb}")
                out_ap = pexp[:].rearrange("p (g c) -> p g c", g=2)[:, :, ce:]
                in_ap = pss[:].rearrange("p (g c) -> p g c", g=2)[:, :, ce:]
                if qt >= 2 and (kb + hp) % SCH_MOD == 0:
                    # DVE fast-exp, only in the ACT-bound late q-tiles
                    nc.vector.tensor_scalar(
                        out_ap.bitcast(I16), in_ap, SCH_A * EXPSCALE, SCH_B, Mult, Add
                    )
                else:
                    nc.scalar.activation(out_ap, in_ap, Exp, scale=EXPSCALE)
                if j >= 0:
                    # mask the boundary block for both heads in one op
                    nc.gpsimd.tensor_tensor(
                        pexp[:].rearrange("p (g c) -> p g c", g=2)[:, :, ts(j, P)],
                        pexp[:].rearrange("p (g c) -> p g c", g=2)[:, :, ts(j, P)],
                        tri2[:].rearrange("p (g c) -> p g c", g=2),
                        Mult,
                    )
                return pexp



            def make_pv_unit(hp, qt, qb, pexps, osb, otq):
                def fn():
                    # both heads' PV accumulation groups, sequentially, into
                    # one [128, 130] PSUM tile (cols h*65+64 = denominators);
                    # each group runs start-to-stop before the next opens
                    # (2KB PSUM zero-region rule).
                    pv = opool.tile([P, 130], F32, tag="pv", name=f"pv{hp}_{qt}_{qb}")
                    for g, hh in ((0, 2 * hp), (1, 2 * hp + 1)):
                        for kb in range(4 * qt + qb + 1):
                            nc.tensor.matmul(
                                pv[:, g * 65 : g * 65 + 65],
                                pexps[kb][:, g * 512 + qb * P : g * 512 + (qb + 1) * P],
                                vaug[kb][:, hh * 65 : hh * 65 + 65],
                                start=(kb == 0),
                                stop=(kb == 4 * qt + qb),
                            )
                    rec = apool2.tile([P, 2], F32, tag="rec", name=f"rec{hp}_{qt}_{qb}")
                    nc.vector.reciprocal(
                        rec[:].rearrange("p (g c) -> p g c", c=1),
                        pv[:].rearrange("p (g c) -> p g c", c=65)[:, :, 64:65],
                    )
                    for g in (0, 1):
                        # pv holds sum(p * 1024*v); rescale by 1/1024 here
                        nc.vector.tensor_scalar(
                            osb[:, g * 64 : (g + 1) * 64],
                            pv[:, g * 65 : g * 65 + 64],
                            rec[:, g : g + 1],
                            1.0 / 1024.0,
                            Mult,
                            Mult,
                        )
                    # one DMA-transpose moves both heads' normalized O[q, dh]
                    # block into O^T[dh, q] inside the per-(hp,qt) staging
                    # tile; after the last block, DVE splits the 512-wide
                    # strip into the fp8 hi/lo pair the outproj consumes
                    nc.sync.dma_start_transpose(otq[:, ts(qb, P)], osb[:])
                    if qt == NQT - 1:
                        # last q-tile: split per block so outproj token
                        # blocks unlock as early as possible (short tail)
                        blk = slice((4 * qt + qb) * P, (4 * qt + qb + 1) * P)
                        ohs = oh8[hp // 2][:, hp % 2, blk]
                        nc.vector.tensor_scalar_mul(ohs, otq[:, ts(qb, P)], 32.0)
                        nc.vector.scalar_tensor_tensor(
                            ol8[hp // 2][:, hp % 2, blk],
                            otq[:, ts(qb, P)],
                            32.0,
                            ohs,
                            Mult,
                            mybir.AluOpType.subtract,
                        )
                        attn_cnt[4 * qt + qb] += 1
                    elif qb == 3:
                        blk = slice(qt * 512, (qt + 1) * 512)
                        ohs = oh8[hp // 2][:, hp % 2, blk]
                        nc.vector.tensor_scalar_mul(ohs, otq[:], 32.0)
                        nc.vector.scalar_tensor_tensor(
                            ol8[hp // 2][:, hp % 2, blk],
                            otq[:],
                            32.0,
                            ohs,
                            Mult,
                            mybir.AluOpType.subtract,
                        )
                        for i in range(4):
                            attn_cnt[4 * qt + i] += 1

                return (2 * (4 * qt + qb + 1) * 65 * 0.42 + 120, fn)

            # ---------------- orchestration ----------------
            # Fine-grained pull model: scores (hp, qt, kb) pull exactly
            # Qproj(qt, hp) and Kproj(kb//4, hp) just in time, so exp work
            # flows to ACT as early as the data allows.  Remaining proj
            # units, PV units and outproj blocks are debt-paced PE filler.
            NVT = TCH // P  # V-proj token-blocks per chunk
            q_done = [[False] * NM for _ in range(NCH)]
            k_done = [[False] * NM for _ in range(NCH)]
            v_done = [[False] * NVT for _ in range(NCH)]

            def chunk_all_done(c):
                return all(q_done[c]) and all(k_done[c]) and all(v_done[c])

            def ensure_x(c):
                if c not in x_dmas_done:
                    # tile-slot hazard: chunk c's x DMA reuses chunk c-2's
                    # buffers; all chunk c-2 readers must be emitted first
                    if c >= 2 and not chunk_all_done(c - 2):
                        pull_chunk(c - 2)
                    emit_x_dma(c)

            def pull_q(c, m):
                if not q_done[c][m]:
                    q_done[c][m] = True
                    ensure_x(c)
                    emit_proj_q_m(c, m)
                    return 1280
                return 0

            def pull_k(c, m):
                if not k_done[c][m]:
                    k_done[c][m] = True
                    ensure_x(c)
                    emit_proj_k_m(c, m)
                    return 1280
                return 0

            def pull_v(c, mt):
                if not v_done[c][mt]:
                    v_done[c][mt] = True
                    ensure_x(c)
                    emit_proj_v_mt(c, mt)
                    return 1280
                return 0

            def pull_chunk(c):
                for m in range(NM):
                    pull_q(c, m)
                for m in range(NM):
                    pull_k(c, m)
                for mt in range(NVT):
                    pull_v(c, mt)

            # workq: debt-paced PE filler (cost_ns, fn); fn returns actual
            # cost (0 if the unit was already pulled directly)
            workq = []
            ogate = {}
            attn_cnt = [0] * NTB  # per token block: heads with split done

            def queue_fillers():
                for c in range(NCH):
                    for m in range(NM):
                        workq.append((f"c{c}", 1280, lambda c=c, m=m: pull_q(c, m)))
                    for m in range(NM):
                        workq.append((f"c{c}", 1280, lambda c=c, m=m: pull_k(c, m)))
                    for mt in range(NVT):
                        workq.append((f"c{c}", 1280, lambda c=c, mt=mt: pull_v(c, mt)))
                for mt in range(NTB):
                    for nt in range(D // 512):
                        def fo(mt=mt, nt=nt):
                            emit_outproj_nt(mt, nt)
                            return 853

                        ogate[id(fo)] = mt
                        workq.append(("o", 640, fo))

            def pop_work(budget_ns):
                spent = 0.0
                i = 0
                while i < len(workq) and spent < budget_ns:
                    kind, cost, fn = workq[i]
                    if kind == "o" and attn_cnt[ogate[id(fn)]] < HG // 2:
                        i += 1
                        continue
                    r = fn()
                    spent += cost if r is None else r
                    workq.pop(i)
                return spent

            # critical-path DMAs first: wq, x chunk 0, wk feed the first
            # scores; wv/wo and chunk 1 follow
            nc.sync.dma_start(wq_t[:, 0:4, :], wq8[:, 0:4, :])
            emit_x_dma(0)
            emit_w_dmas()
            emit_w_dmas_late()
            emit_x_dma(1)
            queue_fillers()
            # PE prewarm: dummy matmuls on the tri constant ramp the tensor
            # engine to full clock while the first weight/x DMAs land
            pwt = big_pool.tile([P, 512], F32, tag="big", name="prewarm")
            for _ in range(44):
                nc.tensor.matmul(
                    pwt[:, 0:P], tri2[:, 0:P], tri2[:, 0:P], start=True, stop=True
                )

            # debt-paced weave: pop a PE filler unit only once the consumer
            # engine's exp backlog exceeds its cost, so the PE stays just
            # behind ACT/DVE.  PV units are emitted inline, one k-block
            # behind the score stream, so nothing drains at the end.
            debt = 0.0
            deferred_pv = []
            groups = [(qt, hp) for qt in range(NQT) for hp in range(HG // 2)]
            for gi, (qt, hp) in enumerate(groups):
                if hp == 0 and qt + 1 < NCH:
                    ensure_x(qt + 1)
                pull_q(qt, hp)
                nkb = 4 * qt + 4
                pexps = []
                osbs = [
                    opool_sb.tile([P, P], BF16, tag=f"osb{qb}", name=f"osb{hp}_{qt}_{qb}")
                    for qb in range(4)
                ]
                otq = ot_pool.tile([P, 512], BF16, tag=f"ot{hp}", name=f"ot{hp}_{qt}")

                def emit_pv(qb, qt=qt, hp=hp, pexps=pexps, osbs=osbs, otq=otq):
                    # vaug writes must be emitted before the PV reads them
                    for j in range(4 * qt + qb + 1):
                        pull_v(j // 4, j % 4)
                    cost, fn = make_pv_unit(hp, qt, qb, pexps, osbs[qb], otq)
                    fn()
                    return cost

                last = gi == len(groups) - 1
                for kb in range(nkb):
                    pull_k(kb // 4, hp)
                    pexps.append(emit_score_kb(hp, qt, kb))
                    if kb == 1 and not last:
                        # prefetch the next group's Q/K so its first score
                        # fires the moment this group's exps are drained
                        nqt, nhp = groups[gi + 1]
                        pull_q(nqt, nhp)
                        pull_k(0, nhp)
                    if kb % 4 == 2 and kb + 2 < nkb:
                        # prefetch the next k-chunk's K projection so its
                        # DVE eviction lands before the scores need it
                        pull_k((kb + 2) // 4, hp)
                    dpv_slot = (4, 5) if nkb > 8 else ((2, 3) if nkb > 4 else (0, 1))
                    if kb in dpv_slot and deferred_pv:
                        # previous group's trailing PV blocks: their exps are
                        # long done, so no PE stall and no ACT gap
                        debt -= deferred_pv.pop(0)()
                    w = 2 * (512 - max(kb - 4 * qt, 0) * P)
                    if qt >= 2 and (kb + hp) % SCH_MOD == 0:
                        debt += (w * 1.04 + 170) - (w * 0.21 + 10)
                    else:
                        debt += (w * 0.833 + 242) - (w * 0.21 + 10)
                    if last:
                        continue  # emit the final scores back-to-back
                    qb = kb - 4 * qt - 2
                    if 0 <= qb <= 1:
                        debt -= emit_pv(qb)
                    npops = 0
                    cap = 3
                    while workq and npops < cap:
                        # prefer proj units; spend outproj units only when
                        # nothing else is ready (saves them for the
                        # filler-starved late q-tiles)
                        pick = None
                        for i, (kind, cost, fn) in enumerate(workq):
                            if kind == "o":
                                continue
                            pick = i
                            break
                        if pick is None:
                            for i, (kind, cost, fn) in enumerate(workq):
                                if kind == "o" and attn_cnt[ogate[id(fn)]] >= HG // 2:
                                    pick = i
                                    break
                        if pick is None or (
                            workq[pick][1] > debt
                            and not (last and workq[pick][0] == "o")
                        ):
                            break
                        kind, cost, fn = workq.pop(pick)
                        r = fn()
                        debt -= r if r is not None else cost
                        npops += 1
                if not last:
                    deferred_pv.append(lambda e=emit_pv: e(2))
                    deferred_pv.append(lambda e=emit_pv: e(3))
                else:
                    # tail: PV per block, then its outproj immediately
                    for qb in range(4):
                        emit_pv(qb)
                        pop_work(1 << 30)
            while workq:
                pop_work(1 << 30)
    nc.finalize()
    return nc


_NC_CACHE = {}


def _get_nc():
    if "full" not in _NC_CACHE:
        _NC_CACHE["full"] = build_bass()
    return _NC_CACHE["full"]


def _pack_hilo(mT):
    """[D, C] fp32 -> [128, 4*ND2, C] fp8 hi/lo pack at scale 32.

    sub-index s = 4*j + 2*hl + i: (d-pair j, hi/lo, block i); value
    hi = e4m3(32*x), lo = e4m3(32*x - hi).
    """
    import ml_dtypes

    e4 = ml_dtypes.float8_e4m3
    D, C = mT.shape
    nd = D // P
    nd2 = nd // 2
    blocks = mT.reshape(nd2, 2, P, C)  # [j, i, p, c]
    hi = (32.0 * blocks).astype(e4)
    lo = (32.0 * blocks - hi.astype(np.float32)).astype(e4)
    out = np.empty((P, 4 * nd2, C), dtype=e4)
    for j in range(nd2):
        for i in range(2):
            out[:, 4 * j + i, :] = hi[j, i]
            out[:, 4 * j + 2 + i, :] = lo[j, i]
    return out


def make_in_maps(query, key_value, Wq, Wk, Wv, Wo):
    import ml_dtypes

    query = np.asarray(query, dtype=np.float32)
    key_value = np.asarray(key_value, dtype=np.float32)
    Wq, Wk, Wv, Wo = (np.asarray(w, dtype=np.float32) for w in (Wq, Wk, Wv, Wo))
    GO = Wq.shape[0] // 2
    bf = ml_dtypes.bfloat16
    xq8_b = [_pack_hilo(np.ascontiguousarray(query[b].T)) for b in range(B_FULL)]
    xkv8_b = [_pack_hilo(np.ascontiguousarray(key_value[b].T)) for b in range(B_FULL)]
    w8 = {}
    for g in range(2):
        sl = slice(g * GO, (g + 1) * GO)
        w8[g] = (
            _pack_hilo(np.ascontiguousarray(Wq[sl, :].T)),
            _pack_hilo(np.ascontiguousarray(Wk[sl, :].T)),
            _pack_hilo(np.ascontiguousarray(Wv[sl, :].T)),
            _pack_hilo(np.ascontiguousarray(Wo[:, sl].T)),
        )
    in_maps = []
    for c in range(N_CORES):
        b, g = c // 2, c % 2
        sl = slice(g * GO, (g + 1) * GO)
        in_maps.append(
            {
                "xq8": xq8_b[b],
                "xkv8": xkv8_b[b],
                "wq8": w8[g][0],
                "wk8": w8[g][1],
                "wv8": w8[g][2],
                "wo8": w8[g][3],
            }
        )
    return in_maps


def kernel(query, key_value, Wq, Wk, Wv, Wo):
    from concourse import bass_utils

    nc = _get_nc()
    in_maps = make_in_maps(query, key_value, Wq, Wk, Wv, Wo)
    res = bass_utils.run_bass_kernel_spmd(nc, in_maps, core_ids=list(range(N_CORES)))
    ys = [r["y"] for r in res.results]
    out = np.stack([ys[2 * b] + ys[2 * b + 1] for b in range(B_FULL)])
    return out.astype(np.float32)



# revision 86
# speedup vs baseline: 1.0286x; 1.0111x over previous
"""Causal cross-attention kernel for 8 trn2 NeuronCores.

Sharding: 4-way data-parallel over batch x 2-way tensor-parallel over heads
(8 heads per core).  Per core:
  - Q/K/V/O projections run as fp8e4m3 DoubleRow matmuls (2 k-subtiles per
    instruction at 0.5 cyc/row) with an error-compensated hi/lo split:
    x = xh + xl and w = wh + wl quantized at scale 32, and the three
    products xh*wh + xl*wh + xh*wl accumulate at a common PSUM scale of
    1024, so projection error is below bf16 rounding at 0.75x bf16 cost.
  - Scores are a single one-sided DoubleRow matmul per (head, k-block):
    kT is stored as an exact fp8 hi/lo pair, qT as single fp8, with q
    broadcast across both subtiles (2x bf16 throughput, ~1.5e-2 rel err).
  - Attention in transposed layout: scores^T[k,q] -> exp on ACT (a slice of
    late-q-tile exps uses a bf16 Schraudolph fast-exp on DVE to unload
    ACT) -> stationary-P^T PV step with the 65-wide bf16 V-aug (ones
    column accumulates the softmax denominator).  Normalized O blocks are
    DMA-transposed and split into fp8 hi/lo for the DoubleRow outproj.
  - Orchestration is a fine-grained pull model: scores (hp, qt, kb) pull
    exactly Qproj(qt, hp)/Kproj(kb//4, hp) just in time; PV units emit
    inline one k-block behind the score stream (trailing blocks spill into
    the next group); remaining projection and outproj work is debt-paced
    PE filler between score emissions.

All host-side work (transposes, hi/lo packing) is data marshaling; the
device kernel is a single NEFF launch per core.
"""

import sys

sys.path.insert(0, "/opt/trn_rl_repo")

import numpy as np

import concourse.bass as bass
import concourse.tile as tile
from concourse import bacc, mybir
from concourse.bass import ts
from concourse.masks import make_upper_triangular

F32 = mybir.dt.float32
F32R = mybir.dt.float32r
BF16 = mybir.dt.bfloat16
FP8 = mybir.dt.float8e4
I16 = mybir.dt.int16
P = 128

# full-problem constants
B_FULL = 4
S_FULL = 2048
D_FULL = 1024
HG_FULL = 8  # heads per core (16 heads / 2-way TP)
N_CORES = 8


def build_bass(S=S_FULL, D=D_FULL, HG=HG_FULL):
    """One-core program; SPMD across 8 cores with different data."""
    GO = HG * 64  # output-feature width of this core's head group
    ND = D // P  # d-blocks (contraction)
    NM = GO // P  # o-tiles of Q/K projections
    NQT = S // 512  # q-tiles (512 wide)
    NTB = S // P  # token blocks of 128
    TCH = 512  # projection t-chunk (one q-tile per chunk)
    NCH = S // TCH

    ND2 = ND // 2  # d-block pairs for fp8 DoubleRow
    # fp8 hi/lo packed inputs: sub-index s = 4*j + 2*hl + i selects
    # (d-pair j, hi/lo, d-block within pair); value = e4m3 of 32*x (hi)
    # or 32*x - hi (lo).  PSUM accumulates at scale 1024.
    nc = bacc.Bacc("TRN2", target_bir_lowering=False, debug=False)
    xq8 = nc.dram_tensor("xq8", [P, 4 * ND2, S], FP8, kind="ExternalInput")
    xkv8 = nc.dram_tensor("xkv8", [P, 4 * ND2, S], FP8, kind="ExternalInput")
    wq8 = nc.dram_tensor("wq8", [P, 4 * ND2, GO], FP8, kind="ExternalInput")
    wk8 = nc.dram_tensor("wk8", [P, 4 * ND2, GO], FP8, kind="ExternalInput")
    wv8 = nc.dram_tensor("wv8", [P, 4 * ND2, GO], FP8, kind="ExternalInput")
    wo8 = nc.dram_tensor("wo8", [P, 4 * (GO // P // 2), D], FP8, kind="ExternalInput")
    y = nc.dram_tensor("y", [S, D], F32, kind="ExternalOutput")

    Exp = mybir.ActivationFunctionType.Exp
    Mult = mybir.AluOpType.mult
    Add = mybir.AluOpType.add
    DR = mybir.MatmulPerfMode.DoubleRow
    EXPSCALE = 0.125 / 1024.0  # scores psum = (32q).(32k) = 1024*s
    # bf16 Schraudolph fast-exp for the Pool engine: int16(x*A + B) bitcast
    # to bf16 ~= exp(x) within ~3.6%; softmax renormalization absorbs most
    # of the sawtooth.  A fraction of exp tiles go to Pool to unload ACT.
    SCH_A = 128.0 * 1.4426950408889634
    SCH_B = 16251.0
    SCH_MOD = 6  # 1/6 of late-q-tile exps take the fast-exp path

    with tile.TileContext(nc) as tc:
        from contextlib import ExitStack

        with ExitStack() as ctx:
            ctx.enter_context(
                nc.allow_low_precision(reason="bf16/fp32r matmul input rounding")
            )
            # ---- persistent SBUF buffers ----
            pers = ctx.enter_context(tc.tile_pool(name="pers", bufs=1))
            # qT: e4m3 at scale 32; kT: [hi, lo] e4m3 pair at scale 32
            qT = [pers.tile([P, S], FP8, tag=f"qT{i}", name=f"qT{i}") for i in range(NM)]
            kT = [pers.tile([P, 2, S], FP8, tag=f"kT{i}", name=f"kT{i}") for i in range(NM)]
            vaug = [pers.tile([P, HG * 65], BF16, tag=f"va{i}", name=f"va{i}") for i in range(NTB)]
            # attention output in fp8 hi/lo pairs per ob-pair jj for the
            # DoubleRow output projection; oT blocks are transient
            oh8 = [pers.tile([P, 2, S], FP8, tag=f"oh{j}", name=f"oh{j}") for j in range(NM // 2)]
            ol8 = [pers.tile([P, 2, S], FP8, tag=f"ol{j}", name=f"ol{j}") for j in range(NM // 2)]
            consts = ctx.enter_context(tc.tile_pool(name="consts", bufs=1))
            tri_f = consts.tile([P, P], F32)  # tri[k,q] = 1 if q >= k else 0
            make_upper_triangular(nc, tri_f[:], val=1.0, diag=True)
            # two side-by-side bf16 copies so one strided op masks 2 heads
            tri2 = consts.tile([P, 2 * P], BF16)
            nc.vector.tensor_copy(tri2[:, 0:P], tri_f[:])
            nc.vector.tensor_copy(tri2[:, P : 2 * P], tri_f[:])
            for i in range(NTB):
                # ones columns survive the V evictions (cols h*65+64)
                nc.gpsimd.memset(vaug[i][:], 1.0)

            w_pool = ctx.enter_context(tc.tile_pool(name="wp", bufs=1))
            x_pool = ctx.enter_context(tc.tile_pool(name="xp", bufs=2))
            big_pool = ctx.enter_context(tc.tile_pool(name="big", bufs=2, space="PSUM"))
            spool = ctx.enter_context(tc.tile_pool(name="ps_s", bufs=2, space="PSUM"))
            opool = ctx.enter_context(tc.tile_pool(name="ps_o", bufs=2, space="PSUM"))
            apool = ctx.enter_context(tc.tile_pool(name="att", bufs=2))
            apool2 = ctx.enter_context(tc.tile_pool(name="attn2", bufs=2))
            opool_sb = ctx.enter_context(tc.tile_pool(name="osb", bufs=2))
            y_pool = ctx.enter_context(tc.tile_pool(name="yev", bufs=3))

            wq_t = w_pool.tile([P, 4 * ND2, GO], FP8, tag="wq", name="wq")
            wk_t = w_pool.tile([P, 4 * ND2, GO], FP8, tag="wk", name="wk")
            wv_t = w_pool.tile([P, 4 * ND2, GO], FP8, tag="wv", name="wv")
            wo_t = w_pool.tile([P, 4 * (NM // 2), D], FP8, tag="wo", name="wo")
            ot_pool = ctx.enter_context(tc.tile_pool(name="otp", bufs=2))
            def emit_w_dmas():
                # interleave wq/wk so the Q- and K-proj data paths become
                # ready together (first score waits on the later of the two)
                nc.sync.dma_start(wk_t[:, 0:4, :], wk8[:, 0:4, :])
                for j in range(1, ND2):
                    nc.sync.dma_start(wq_t[:, 4 * j : 4 * j + 4, :], wq8[:, 4 * j : 4 * j + 4, :])
                    nc.sync.dma_start(wk_t[:, 4 * j : 4 * j + 4, :], wk8[:, 4 * j : 4 * j + 4, :])

            def emit_w_dmas_late():
                for j in range(ND2):
                    nc.sync.dma_start(wv_t[:, 4 * j : 4 * j + 4, :], wv8[:, 4 * j : 4 * j + 4, :])
                for j in range(NM // 2):
                    nc.sync.dma_start(wo_t[:, 4 * j : 4 * j + 4, :], wo8[:, 4 * j : 4 * j + 4, :])

            # ---------------- emitter units ----------------
            xq_tiles = {}
            xkv_tiles = {}
            ysb_tiles = {}
            x_dmas_done = set()

            def emit_x_dma(tc_i):
                x_dmas_done.add(tc_i)
                xq = x_pool.tile([P, 4 * ND2, TCH], FP8, tag="xq", name=f"xq_{tc_i}")
                xkv = x_pool.tile([P, 4 * ND2, TCH], FP8, tag="xk", name=f"xkv_{tc_i}")
                for h in range(2):
                    sl = slice(8 * h, 8 * h + 8)
                    nc.sync.dma_start(xq[:, sl, :], xq8[:, sl, ts(tc_i, TCH)])
                    nc.sync.dma_start(xkv[:, sl, :], xkv8[:, sl, ts(tc_i, TCH)])
                xq_tiles[tc_i] = xq
                xkv_tiles[tc_i] = xkv

            def dr3(ps, wt, xt, mcols, j, first, last):
                # 3-term error-compensated fp8 DoubleRow: hi*hi + hi*lo + lo*hi
                wh = wt[:, 4 * j : 4 * j + 2, mcols]
                wl = wt[:, 4 * j + 2 : 4 * j + 4, mcols]
                xh = xt[:, 4 * j : 4 * j + 2, :]
                xl = xt[:, 4 * j + 2 : 4 * j + 4, :]
                nc.tensor.matmul(ps, wh, xh, start=first, stop=False, perf_mode=DR)
                nc.tensor.matmul(ps, wh, xl, start=False, stop=False, perf_mode=DR)
                nc.tensor.matmul(ps, wl, xh, start=False, stop=last, perf_mode=DR)

            def emit_proj_q_m(tc_i, m):
                xq = xq_tiles[tc_i]
                ps = big_pool.tile([P, 512], F32, tag="big", name=f"pq{tc_i}_{m}")
                for j in range(ND2):
                    dr3(ps[:, 0:TCH], wq_t, xq, ts(m, P), j, j == 0, j == ND2 - 1)
                # evict 1024q -> e4m3(32q)
                nc.vector.tensor_scalar_mul(qT[m][:, ts(tc_i, TCH)], ps[:, 0:TCH], 1.0 / 32.0)

            def emit_proj_k_m(tc_i, m):
                xkv = xkv_tiles[tc_i]
                ps = big_pool.tile([P, 512], F32, tag="big", name=f"pk{tc_i}_{m}")
                for j in range(ND2):
                    dr3(ps[:, 0:TCH], wk_t, xkv, ts(m, P), j, j == 0, j == ND2 - 1)
                # evict 1024k -> hi = e4m3(32k), lo = e4m3(32k - hi)
                nc.vector.tensor_scalar_mul(kT[m][:, 0, ts(tc_i, TCH)], ps[:, 0:TCH], 1.0 / 32.0)
                nc.vector.scalar_tensor_tensor(
                    kT[m][:, 1, ts(tc_i, TCH)],
                    ps[:, 0:TCH],
                    1.0 / 32.0,
                    kT[m][:, 0, ts(tc_i, TCH)],
                    Mult,
                    mybir.AluOpType.subtract,
                )

            def emit_proj_v_mt(tc_i, mt):
                xkv = xkv_tiles[tc_i]
                ps = big_pool.tile([P, 512], F32, tag="big", name=f"pv{tc_i}_{mt}")
                for j in range(ND2):
                    xh = xkv[:, 4 * j : 4 * j + 2, ts(mt, P)]
                    xl = xkv[:, 4 * j + 2 : 4 * j + 4, ts(mt, P)]
                    wh = wv_t[:, 4 * j : 4 * j + 2, :]
                    wl = wv_t[:, 4 * j + 2 : 4 * j + 4, :]
                    nc.tensor.matmul(ps[:], xh, wh, start=(j == 0), stop=False, perf_mode=DR)
                    nc.tensor.matmul(ps[:], xl, wh, start=False, stop=False, perf_mode=DR)
                    nc.tensor.matmul(ps[:], xh, wl, start=False, stop=(j == ND2 - 1), perf_mode=DR)
                vt = vaug[tc_i * (TCH // P) + mt]
                nc.vector.tensor_copy(
                    vt[:].rearrange("p (h c) -> p h c", c=65)[:, :, 0:64],
                    ps[:].rearrange("p (h c) -> p h c", c=64),
                )

            def emit_proj_q(tc_i):
                for m in range(NM):
                    emit_proj_q_m(tc_i, m)

            def emit_proj_k(tc_i):
                for m in range(NM):
                    emit_proj_k_m(tc_i, m)

            def emit_proj_v(tc_i):
                for mt in range(TCH // P):
                    emit_proj_v_mt(tc_i, mt)
                del xq_tiles[tc_i], xkv_tiles[tc_i]

            def emit_outproj_nt(mt, nt):
                # fp8 DoubleRow 3-term: psum = 1024*y over ob-pairs jj
                ps = big_pool.tile([P, 512], F32, tag="big", name=f"y{mt}_{nt}")
                NJ = NM // 2
                for jj in range(NJ):
                    oh = oh8[jj][:, :, ts(mt, P)]
                    ol = ol8[jj][:, :, ts(mt, P)]
                    wh = wo_t[:, 4 * jj : 4 * jj + 2, ts(nt, 512)]
                    wl = wo_t[:, 4 * jj + 2 : 4 * jj + 4, ts(nt, 512)]
                    nc.tensor.matmul(ps[:], oh, wh, start=(jj == 0), stop=False, perf_mode=DR)
                    nc.tensor.matmul(ps[:], ol, wh, start=False, stop=False, perf_mode=DR)
                    nc.tensor.matmul(ps[:], oh, wl, start=False, stop=(jj == NJ - 1), perf_mode=DR)
                ysb = y_pool.tile([P, 512], F32, tag="ysb", name=f"ysb{mt}_{nt}")
                nc.vector.tensor_scalar_mul(ysb[:], ps[:], 1.0 / 1024.0)
                nc.sync.dma_start(y[ts(mt, P), ts(nt, 512)], ysb[:])

            def emit_score_kb(hp, qt, kb):
                j = kb - 4 * qt
                ce = max(j, 0) * P
                # both heads' scores in one 2-bank PSUM tile so a single
                # strided activation does both exps
                pss = spool.tile([P, 1024], F32, tag="s", name=f"s{hp}_{qt}_{kb}")
                w1 = 512 - ce
                for g, po in ((0, 0), (1, 64)):
                    # one fp8 DoubleRow matmul: (kh + kl) . q, q broadcast
                    qv = (
                        qT[hp][po : po + 64, qt * 512 + ce : (qt + 1) * 512]
                        .unsqueeze(1)
                        .broadcast_to([64, 2, w1])
                    )
                    nc.tensor.matmul(
                        pss[:, g * 512 + ce : (g + 1) * 512],
                        kT[hp][po : po + 64, :, ts(kb, P)],
                        qv,
                        start=True,
                        stop=True,
                        perf_mode=DR,
                    )
                pexp = apool.tile([P, 1024], BF16, tag=f"p{kb}", name=f"p{hp}_{qt}_{kb}")
                out_ap = pexp[:].rearrange("p (g c) -> p g c", g=2)[:, :, ce:]
                in_ap = pss[:].rearrange("p (g c) -> p g c", g=2)[:, :, ce:]
                dense_tail = qt == NQT - 1 and hp == HG // 2 - 1 and kb % 2 == 1
                if (qt >= 2 and (kb + hp) % SCH_MOD == 0) or dense_tail:
                    # DVE fast-exp in the ACT-bound late q-tiles; the final
                    # group splits its exps ACT/DVE so the last one (which
                    # gates the whole output tail) lands ~5us earlier
                    nc.vector.tensor_scalar(
                        out_ap.bitcast(I16), in_ap, SCH_A * EXPSCALE, SCH_B, Mult, Add
                    )
                else:
                    nc.scalar.activation(out_ap, in_ap, Exp, scale=EXPSCALE)
                if j >= 0:
                    # mask the boundary block for both heads in one op
                    nc.gpsimd.tensor_tensor(
                        pexp[:].rearrange("p (g c) -> p g c", g=2)[:, :, ts(j, P)],
                        pexp[:].rearrange("p (g c) -> p g c", g=2)[:, :, ts(j, P)],
                        tri2[:].rearrange("p (g c) -> p g c", g=2),
                        Mult,
                    )
                return pexp



            def make_pv_unit(hp, qt, qb, pexps, osb, otq):
                def fn():
                    # both heads' PV accumulation groups, sequentially, into
                    # one [128, 130] PSUM tile (cols h*65+64 = denominators);
                    # each group runs start-to-stop before the next opens
                    # (2KB PSUM zero-region rule).
                    pv = opool.tile([P, 130], F32, tag="pv", name=f"pv{hp}_{qt}_{qb}")
                    for g, hh in ((0, 2 * hp), (1, 2 * hp + 1)):
                        for kb in range(4 * qt + qb + 1):
                            nc.tensor.matmul(
                                pv[:, g * 65 : g * 65 + 65],
                                pexps[kb][:, g * 512 + qb * P : g * 512 + (qb + 1) * P],
                                vaug[kb][:, hh * 65 : hh * 65 + 65],
                                start=(kb == 0),
                                stop=(kb == 4 * qt + qb),
                            )
                    rec = apool2.tile([P, 2], F32, tag="rec", name=f"rec{hp}_{qt}_{qb}")
                    nc.vector.reciprocal(
                        rec[:].rearrange("p (g c) -> p g c", c=1),
                        pv[:].rearrange("p (g c) -> p g c", c=65)[:, :, 64:65],
                    )
                    for g in (0, 1):
                        # pv holds sum(p * 1024*v); rescale by 1/1024 here
                        nc.vector.tensor_scalar(
                            osb[:, g * 64 : (g + 1) * 64],
                            pv[:, g * 65 : g * 65 + 64],
                            rec[:, g : g + 1],
                            1.0 / 1024.0,
                            Mult,
                            Mult,
                        )
                    # one DMA-transpose moves both heads' normalized O[q, dh]
                    # block into O^T[dh, q] inside the per-(hp,qt) staging
                    # tile; after the last block, DVE splits the 512-wide
                    # strip into the fp8 hi/lo pair the outproj consumes
                    nc.sync.dma_start_transpose(otq[:, ts(qb, P)], osb[:])
                    if qt == NQT - 1:
                        # last q-tile: split per block so outproj token
                        # blocks unlock as early as possible (short tail)
                        blk = slice((4 * qt + qb) * P, (4 * qt + qb + 1) * P)
                        ohs = oh8[hp // 2][:, hp % 2, blk]
                        nc.vector.tensor_scalar_mul(ohs, otq[:, ts(qb, P)], 32.0)
                        nc.vector.scalar_tensor_tensor(
                            ol8[hp // 2][:, hp % 2, blk],
                            otq[:, ts(qb, P)],
                            32.0,
                            ohs,
                            Mult,
                            mybir.AluOpType.subtract,
                        )
                        attn_cnt[4 * qt + qb] += 1
                    elif qb == 3:
                        blk = slice(qt * 512, (qt + 1) * 512)
                        ohs = oh8[hp // 2][:, hp % 2, blk]
                        nc.vector.tensor_scalar_mul(ohs, otq[:], 32.0)
                        nc.vector.scalar_tensor_tensor(
                            ol8[hp // 2][:, hp % 2, blk],
                            otq[:],
                            32.0,
                            ohs,
                            Mult,
                            mybir.AluOpType.subtract,
                        )
                        for i in range(4):
                            attn_cnt[4 * qt + i] += 1

                return (2 * (4 * qt + qb + 1) * 65 * 0.42 + 120, fn)

            # ---------------- orchestration ----------------
            # Fine-grained pull model: scores (hp, qt, kb) pull exactly
            # Qproj(qt, hp) and Kproj(kb//4, hp) just in time, so exp work
            # flows to ACT as early as the data allows.  Remaining proj
            # units, PV units and outproj blocks are debt-paced PE filler.
            NVT = TCH // P  # V-proj token-blocks per chunk
            q_done = [[False] * NM for _ in range(NCH)]
            k_done = [[False] * NM for _ in range(NCH)]
            v_done = [[False] * NVT for _ in range(NCH)]

            def chunk_all_done(c):
                return all(q_done[c]) and all(k_done[c]) and all(v_done[c])

            def ensure_x(c):
                if c not in x_dmas_done:
                    # tile-slot hazard: chunk c's x DMA reuses chunk c-2's
                    # buffers; all chunk c-2 readers must be emitted first
                    if c >= 2 and not chunk_all_done(c - 2):
                        pull_chunk(c - 2)
                    emit_x_dma(c)

            def pull_q(c, m):
                if not q_done[c][m]:
                    q_done[c][m] = True
                    ensure_x(c)
                    emit_proj_q_m(c, m)
                    return 1280
                return 0

            def pull_k(c, m):
                if not k_done[c][m]:
                    k_done[c][m] = True
                    ensure_x(c)
                    emit_proj_k_m(c, m)
                    return 1280
                return 0

            def pull_v(c, mt):
                if not v_done[c][mt]:
                    v_done[c][mt] = True
                    ensure_x(c)
                    emit_proj_v_mt(c, mt)
                    return 1280
                return 0

            def pull_chunk(c):
                for m in range(NM):
                    pull_q(c, m)
                for m in range(NM):
                    pull_k(c, m)
                for mt in range(NVT):
                    pull_v(c, mt)

            # workq: debt-paced PE filler (cost_ns, fn); fn returns actual
            # cost (0 if the unit was already pulled directly)
            workq = []
            ogate = {}
            attn_cnt = [0] * NTB  # per token block: heads with split done

            def queue_fillers():
                for c in range(NCH):
                    for m in range(NM):
                        workq.append((f"c{c}", 1280, lambda c=c, m=m: pull_q(c, m)))
                    for m in range(NM):
                        workq.append((f"c{c}", 1280, lambda c=c, m=m: pull_k(c, m)))
                    for mt in range(NVT):
                        workq.append((f"c{c}", 1280, lambda c=c, mt=mt: pull_v(c, mt)))
                for mt in range(NTB):
                    for nt in range(D // 512):
                        def fo(mt=mt, nt=nt):
                            emit_outproj_nt(mt, nt)
                            return 853

                        ogate[id(fo)] = mt
                        workq.append(("o", 640, fo))

            def pop_work(budget_ns):
                spent = 0.0
                i = 0
                while i < len(workq) and spent < budget_ns:
                    kind, cost, fn = workq[i]
                    if kind == "o" and attn_cnt[ogate[id(fn)]] < HG // 2:
                        i += 1
                        continue
                    r = fn()
                    spent += cost if r is None else r
                    workq.pop(i)
                return spent

            # critical-path DMAs first: wq, x chunk 0, wk feed the first
            # scores; wv/wo and chunk 1 follow
            nc.sync.dma_start(wq_t[:, 0:4, :], wq8[:, 0:4, :])
            emit_x_dma(0)
            emit_w_dmas()
            emit_w_dmas_late()
            emit_x_dma(1)
            queue_fillers()
            # PE prewarm: dummy matmuls on the tri constant ramp the tensor
            # engine to full clock while the first weight/x DMAs land
            pwt = big_pool.tile([P, 512], F32, tag="big", name="prewarm")
            for _ in range(44):
                nc.tensor.matmul(
                    pwt[:, 0:P], tri2[:, 0:P], tri2[:, 0:P], start=True, stop=True
                )

            # debt-paced weave: pop a PE filler unit only once the consumer
            # engine's exp backlog exceeds its cost, so the PE stays just
            # behind ACT/DVE.  PV units are emitted inline, one k-block
            # behind the score stream, so nothing drains at the end.
            debt = 0.0
            deferred_pv = []
            groups = [(qt, hp) for qt in range(NQT) for hp in range(HG // 2)]
            for gi, (qt, hp) in enumerate(groups):
                if hp == 0 and qt + 1 < NCH:
                    ensure_x(qt + 1)
                pull_q(qt, hp)
                nkb = 4 * qt + 4
                pexps = []
                osbs = [
                    opool_sb.tile([P, P], BF16, tag=f"osb{qb}", name=f"osb{hp}_{qt}_{qb}")
                    for qb in range(4)
                ]
                otq = ot_pool.tile([P, 512], BF16, tag=f"ot{hp}", name=f"ot{hp}_{qt}")

                def emit_pv(qb, qt=qt, hp=hp, pexps=pexps, osbs=osbs, otq=otq):
                    # vaug writes must be emitted before the PV reads them
                    for j in range(4 * qt + qb + 1):
                        pull_v(j // 4, j % 4)
                    cost, fn = make_pv_unit(hp, qt, qb, pexps, osbs[qb], otq)
                    fn()
                    return cost

                last = gi == len(groups) - 1
                for kb in range(nkb):
                    pull_k(kb // 4, hp)
                    pexps.append(emit_score_kb(hp, qt, kb))
                    if kb == 1 and not last:
                        # prefetch the next group's Q/K so its first score
                        # fires the moment this group's exps are drained
                        nqt, nhp = groups[gi + 1]
                        pull_q(nqt, nhp)
                        pull_k(0, nhp)
                    if kb % 4 == 2 and kb + 2 < nkb:
                        # prefetch the next k-chunk's K projection so its
                        # DVE eviction lands before the scores need it
                        pull_k((kb + 2) // 4, hp)
                    dpv_slot = (4, 5) if nkb > 8 else ((2, 3) if nkb > 4 else (0, 1))
                    if kb in dpv_slot and deferred_pv:
                        # previous group's trailing PV blocks: their exps are
                        # long done, so no PE stall and no ACT gap
                        debt -= deferred_pv.pop(0)()
                    w = 2 * (512 - max(kb - 4 * qt, 0) * P)
                    if qt >= 2 and (kb + hp) % SCH_MOD == 0:
                        debt += (w * 1.04 + 170) - (w * 0.21 + 10)
                    else:
                        debt += (w * 0.833 + 242) - (w * 0.21 + 10)
                    if last:
                        continue  # emit the final scores back-to-back
                    qb = kb - 4 * qt - 2
                    if 0 <= qb <= 1:
                        debt -= emit_pv(qb)
                    npops = 0
                    cap = 3
                    while workq and npops < cap:
                        # prefer proj units; spend outproj units only when
                        # nothing else is ready (saves them for the
                        # filler-starved late q-tiles)
                        pick = None
                        for i, (kind, cost, fn) in enumerate(workq):
                            if kind == "o":
                                continue
                            pick = i
                            break
                        if pick is None:
                            for i, (kind, cost, fn) in enumerate(workq):
                                if kind == "o" and attn_cnt[ogate[id(fn)]] >= HG // 2:
                                    pick = i
                                    break
                        if pick is None or (
                            workq[pick][1] > debt
                            and not (last and workq[pick][0] == "o")
                        ):
                            break
                        kind, cost, fn = workq.pop(pick)
                        r = fn()
                        debt -= r if r is not None else cost
                        npops += 1
                if not last:
                    deferred_pv.append(lambda e=emit_pv: e(2))
                    deferred_pv.append(lambda e=emit_pv: e(3))
                else:
                    # tail: PV per block, then its outproj immediately
                    for qb in range(4):
                        emit_pv(qb)
                        pop_work(1 << 30)
            while workq:
                pop_work(1 << 30)
    nc.finalize()
    return nc


_NC_CACHE = {}


def _get_nc():
    if "full" not in _NC_CACHE:
        _NC_CACHE["full"] = build_bass()
    return _NC_CACHE["full"]


def _pack_hilo(mT):
    """[D, C] fp32 -> [128, 4*ND2, C] fp8 hi/lo pack at scale 32.

    sub-index s = 4*j + 2*hl + i: (d-pair j, hi/lo, block i); value
    hi = e4m3(32*x), lo = e4m3(32*x - hi).
    """
    import ml_dtypes

    e4 = ml_dtypes.float8_e4m3
    D, C = mT.shape
    nd = D // P
    nd2 = nd // 2
    blocks = mT.reshape(nd2, 2, P, C)  # [j, i, p, c]
    hi = (32.0 * blocks).astype(e4)
    lo = (32.0 * blocks - hi.astype(np.float32)).astype(e4)
    out = np.empty((P, 4 * nd2, C), dtype=e4)
    for j in range(nd2):
        for i in range(2):
            out[:, 4 * j + i, :] = hi[j, i]
            out[:, 4 * j + 2 + i, :] = lo[j, i]
    return out


def make_in_maps(query, key_value, Wq, Wk, Wv, Wo):
    import ml_dtypes

    query = np.asarray(query, dtype=np.float32)
    key_value = np.asarray(key_value, dtype=np.float32)
    Wq, Wk, Wv, Wo = (np.asarray(w, dtype=np.float32) for w in (Wq, Wk, Wv, Wo))
    GO = Wq.shape[0] // 2
    bf = ml_dtypes.bfloat16
    xq8_b = [_pack_hilo(np.ascontiguousarray(query[b].T)) for b in range(B_FULL)]
    xkv8_b = [_pack_hilo(np.ascontiguousarray(key_value[b].T)) for b in range(B_FULL)]
    w8 = {}
    for g in range(2):
        sl = slice(g * GO, (g + 1) * GO)
        w8[g] = (
            _pack_hilo(np.ascontiguousarray(Wq[sl, :].T)),
            _pack_hilo(np.ascontiguousarray(Wk[sl, :].T)),
            _pack_hilo(np.ascontiguousarray(Wv[sl, :].T)),
            _pack_hilo(np.ascontiguousarray(Wo[:, sl].T)),
        )
    in_maps = []
    for c in range(N_CORES):
        b, g = c // 2, c % 2
        sl = slice(g * GO, (g + 1) * GO)
        in_maps.append(
            {
                "xq8": xq8_b[b],
                "xkv8": xkv8_b[b],
                "wq8": w8[g][0],
                "wk8": w8[g][1],
                "wv8": w8[g][2],
                "wo8": w8[g][3],
            }
        )
    return in_maps


def kernel(query, key_value, Wq, Wk, Wv, Wo):
    from concourse import bass_utils

    nc = _get_nc()
    in_maps = make_in_maps(query, key_value, Wq, Wk, Wv, Wo)
    res = bass_utils.run_bass_kernel_spmd(nc, in_maps, core_ids=list(range(N_CORES)))
    ys = [r["y"] for r in res.results]
    out = np.stack([ys[2 * b] + ys[2 * b + 1] for b in range(B_FULL)])
    return out.astype(np.float32)



# revision 87
# speedup vs baseline: 1.0319x; 1.0032x over previous
"""Causal cross-attention kernel for 8 trn2 NeuronCores.

Sharding: 4-way data-parallel over batch x 2-way tensor-parallel over heads
(8 heads per core).  Per core:
  - Q/K/V/O projections run as fp8e4m3 DoubleRow matmuls (2 k-subtiles per
    instruction at 0.5 cyc/row) with an error-compensated hi/lo split:
    x = xh + xl and w = wh + wl quantized at scale 32, and the three
    products xh*wh + xl*wh + xh*wl accumulate at a common PSUM scale of
    1024, so projection error is below bf16 rounding at 0.75x bf16 cost.
  - Scores are a single one-sided DoubleRow matmul per (head, k-block):
    kT is stored as an exact fp8 hi/lo pair, qT as single fp8, with q
    broadcast across both subtiles (2x bf16 throughput, ~1.5e-2 rel err).
  - Attention in transposed layout: scores^T[k,q] -> exp on ACT (a slice of
    late-q-tile exps uses a bf16 Schraudolph fast-exp on DVE to unload
    ACT) -> stationary-P^T PV step with the 65-wide bf16 V-aug (ones
    column accumulates the softmax denominator).  Normalized O blocks are
    DMA-transposed and split into fp8 hi/lo for the DoubleRow outproj.
  - Orchestration is a fine-grained pull model: scores (hp, qt, kb) pull
    exactly Qproj(qt, hp)/Kproj(kb//4, hp) just in time; PV units emit
    inline one k-block behind the score stream (trailing blocks spill into
    the next group); remaining projection and outproj work is debt-paced
    PE filler between score emissions.

All host-side work (transposes, hi/lo packing) is data marshaling; the
device kernel is a single NEFF launch per core.
"""

import sys

sys.path.insert(0, "/opt/trn_rl_repo")

import numpy as np

import concourse.bass as bass
import concourse.tile as tile
from concourse import bacc, mybir
from concourse.bass import ts
from concourse.masks import make_upper_triangular

F32 = mybir.dt.float32
F32R = mybir.dt.float32r
BF16 = mybir.dt.bfloat16
FP8 = mybir.dt.float8e4
I16 = mybir.dt.int16
P = 128

# full-problem constants
B_FULL = 4
S_FULL = 2048
D_FULL = 1024
HG_FULL = 8  # heads per core (16 heads / 2-way TP)
N_CORES = 8


def build_bass(S=S_FULL, D=D_FULL, HG=HG_FULL):
    """One-core program; SPMD across 8 cores with different data."""
    GO = HG * 64  # output-feature width of this core's head group
    ND = D // P  # d-blocks (contraction)
    NM = GO // P  # o-tiles of Q/K projections
    NQT = S // 512  # q-tiles (512 wide)
    NTB = S // P  # token blocks of 128
    TCH = 512  # projection t-chunk (one q-tile per chunk)
    NCH = S // TCH

    ND2 = ND // 2  # d-block pairs for fp8 DoubleRow
    # fp8 hi/lo packed inputs: sub-index s = 4*j + 2*hl + i selects
    # (d-pair j, hi/lo, d-block within pair); value = e4m3 of 32*x (hi)
    # or 32*x - hi (lo).  PSUM accumulates at scale 1024.
    nc = bacc.Bacc("TRN2", target_bir_lowering=False, debug=False)
    xq8 = nc.dram_tensor("xq8", [P, 4 * ND2, S], FP8, kind="ExternalInput")
    xkv8 = nc.dram_tensor("xkv8", [P, 4 * ND2, S], FP8, kind="ExternalInput")
    wq8 = nc.dram_tensor("wq8", [P, 4 * ND2, GO], FP8, kind="ExternalInput")
    wk8 = nc.dram_tensor("wk8", [P, 4 * ND2, GO], FP8, kind="ExternalInput")
    wv8 = nc.dram_tensor("wv8", [P, 4 * ND2, GO], FP8, kind="ExternalInput")
    wo8 = nc.dram_tensor("wo8", [P, 4 * (GO // P // 2), D], FP8, kind="ExternalInput")
    y = nc.dram_tensor("y", [S, D], F32, kind="ExternalOutput")

    Exp = mybir.ActivationFunctionType.Exp
    Mult = mybir.AluOpType.mult
    Add = mybir.AluOpType.add
    DR = mybir.MatmulPerfMode.DoubleRow
    EXPSCALE = 0.125 / 1024.0  # scores psum = (32q).(32k) = 1024*s
    # bf16 Schraudolph fast-exp for the Pool engine: int16(x*A + B) bitcast
    # to bf16 ~= exp(x) within ~3.6%; softmax renormalization absorbs most
    # of the sawtooth.  A fraction of exp tiles go to Pool to unload ACT.
    SCH_A = 128.0 * 1.4426950408889634
    SCH_B = 16251.0
    SCH_MOD = 6  # 1/6 of late-q-tile exps take the fast-exp path

    with tile.TileContext(nc) as tc:
        from contextlib import ExitStack

        with ExitStack() as ctx:
            ctx.enter_context(
                nc.allow_low_precision(reason="bf16/fp32r matmul input rounding")
            )
            # ---- persistent SBUF buffers ----
            pers = ctx.enter_context(tc.tile_pool(name="pers", bufs=1))
            # qT: e4m3 at scale 32; kT: [hi, lo] e4m3 pair at scale 32
            qT = [pers.tile([P, S], FP8, tag=f"qT{i}", name=f"qT{i}") for i in range(NM)]
            kT = [pers.tile([P, 2, S], FP8, tag=f"kT{i}", name=f"kT{i}") for i in range(NM)]
            vaug = [pers.tile([P, HG * 65], BF16, tag=f"va{i}", name=f"va{i}") for i in range(NTB)]
            # attention output in fp8 hi/lo pairs per ob-pair jj for the
            # DoubleRow output projection; oT blocks are transient
            oh8 = [pers.tile([P, 2, S], FP8, tag=f"oh{j}", name=f"oh{j}") for j in range(NM // 2)]
            ol8 = [pers.tile([P, 2, S], FP8, tag=f"ol{j}", name=f"ol{j}") for j in range(NM // 2)]
            consts = ctx.enter_context(tc.tile_pool(name="consts", bufs=1))
            tri_f = consts.tile([P, P], F32)  # tri[k,q] = 1 if q >= k else 0
            make_upper_triangular(nc, tri_f[:], val=1.0, diag=True)
            # two side-by-side bf16 copies so one strided op masks 2 heads
            tri2 = consts.tile([P, 2 * P], BF16)
            nc.vector.tensor_copy(tri2[:, 0:P], tri_f[:])
            nc.vector.tensor_copy(tri2[:, P : 2 * P], tri_f[:])
            for i in range(NTB):
                # ones columns survive the V evictions (cols h*65+64)
                nc.gpsimd.memset(vaug[i][:], 1.0)

            w_pool = ctx.enter_context(tc.tile_pool(name="wp", bufs=1))
            x_pool = ctx.enter_context(tc.tile_pool(name="xp", bufs=2))
            big_pool = ctx.enter_context(tc.tile_pool(name="big", bufs=2, space="PSUM"))
            spool = ctx.enter_context(tc.tile_pool(name="ps_s", bufs=2, space="PSUM"))
            opool = ctx.enter_context(tc.tile_pool(name="ps_o", bufs=2, space="PSUM"))
            apool = ctx.enter_context(tc.tile_pool(name="att", bufs=2))
            apool2 = ctx.enter_context(tc.tile_pool(name="attn2", bufs=2))
            opool_sb = ctx.enter_context(tc.tile_pool(name="osb", bufs=2))
            y_pool = ctx.enter_context(tc.tile_pool(name="yev", bufs=3))

            wq_t = w_pool.tile([P, 4 * ND2, GO], FP8, tag="wq", name="wq")
            wk_t = w_pool.tile([P, 4 * ND2, GO], FP8, tag="wk", name="wk")
            wv_t = w_pool.tile([P, 4 * ND2, GO], FP8, tag="wv", name="wv")
            wo_t = w_pool.tile([P, 4 * (NM // 2), D], FP8, tag="wo", name="wo")
            ot_pool = ctx.enter_context(tc.tile_pool(name="otp", bufs=2))
            def emit_w_dmas():
                # interleave wq/wk so the Q- and K-proj data paths become
                # ready together (first score waits on the later of the two)
                nc.sync.dma_start(wk_t[:, 0:4, :], wk8[:, 0:4, :])
                for j in range(1, ND2):
                    nc.sync.dma_start(wq_t[:, 4 * j : 4 * j + 4, :], wq8[:, 4 * j : 4 * j + 4, :])
                    nc.sync.dma_start(wk_t[:, 4 * j : 4 * j + 4, :], wk8[:, 4 * j : 4 * j + 4, :])

            def emit_w_dmas_late():
                for j in range(ND2):
                    nc.sync.dma_start(wv_t[:, 4 * j : 4 * j + 4, :], wv8[:, 4 * j : 4 * j + 4, :])
                for j in range(NM // 2):
                    nc.sync.dma_start(wo_t[:, 4 * j : 4 * j + 4, :], wo8[:, 4 * j : 4 * j + 4, :])

            # ---------------- emitter units ----------------
            xq_tiles = {}
            xkv_tiles = {}
            ysb_tiles = {}
            x_dmas_done = set()

            def emit_x_dma(tc_i):
                x_dmas_done.add(tc_i)
                xq = x_pool.tile([P, 4 * ND2, TCH], FP8, tag="xq", name=f"xq_{tc_i}")
                xkv = x_pool.tile([P, 4 * ND2, TCH], FP8, tag="xk", name=f"xkv_{tc_i}")
                for h in range(2):
                    sl = slice(8 * h, 8 * h + 8)
                    nc.sync.dma_start(xq[:, sl, :], xq8[:, sl, ts(tc_i, TCH)])
                    nc.sync.dma_start(xkv[:, sl, :], xkv8[:, sl, ts(tc_i, TCH)])
                xq_tiles[tc_i] = xq
                xkv_tiles[tc_i] = xkv

            def dr3(ps, wt, xt, mcols, j, first, last):
                # 3-term error-compensated fp8 DoubleRow: hi*hi + hi*lo + lo*hi
                wh = wt[:, 4 * j : 4 * j + 2, mcols]
                wl = wt[:, 4 * j + 2 : 4 * j + 4, mcols]
                xh = xt[:, 4 * j : 4 * j + 2, :]
                xl = xt[:, 4 * j + 2 : 4 * j + 4, :]
                nc.tensor.matmul(ps, wh, xh, start=first, stop=False, perf_mode=DR)
                nc.tensor.matmul(ps, wh, xl, start=False, stop=False, perf_mode=DR)
                nc.tensor.matmul(ps, wl, xh, start=False, stop=last, perf_mode=DR)

            def emit_proj_q_m(tc_i, m):
                xq = xq_tiles[tc_i]
                ps = big_pool.tile([P, 512], F32, tag="big", name=f"pq{tc_i}_{m}")
                for j in range(ND2):
                    dr3(ps[:, 0:TCH], wq_t, xq, ts(m, P), j, j == 0, j == ND2 - 1)
                # evict 1024q -> e4m3(32q)
                nc.vector.tensor_scalar_mul(qT[m][:, ts(tc_i, TCH)], ps[:, 0:TCH], 1.0 / 32.0)

            def emit_proj_k_m(tc_i, m):
                xkv = xkv_tiles[tc_i]
                ps = big_pool.tile([P, 512], F32, tag="big", name=f"pk{tc_i}_{m}")
                for j in range(ND2):
                    dr3(ps[:, 0:TCH], wk_t, xkv, ts(m, P), j, j == 0, j == ND2 - 1)
                # evict 1024k -> hi = e4m3(32k), lo = e4m3(32k - hi)
                nc.vector.tensor_scalar_mul(kT[m][:, 0, ts(tc_i, TCH)], ps[:, 0:TCH], 1.0 / 32.0)
                nc.vector.scalar_tensor_tensor(
                    kT[m][:, 1, ts(tc_i, TCH)],
                    ps[:, 0:TCH],
                    1.0 / 32.0,
                    kT[m][:, 0, ts(tc_i, TCH)],
                    Mult,
                    mybir.AluOpType.subtract,
                )

            def emit_proj_v_mt(tc_i, mt):
                xkv = xkv_tiles[tc_i]
                ps = big_pool.tile([P, 512], F32, tag="big", name=f"pv{tc_i}_{mt}")
                for j in range(ND2):
                    xh = xkv[:, 4 * j : 4 * j + 2, ts(mt, P)]
                    xl = xkv[:, 4 * j + 2 : 4 * j + 4, ts(mt, P)]
                    wh = wv_t[:, 4 * j : 4 * j + 2, :]
                    wl = wv_t[:, 4 * j + 2 : 4 * j + 4, :]
                    nc.tensor.matmul(ps[:], xh, wh, start=(j == 0), stop=False, perf_mode=DR)
                    nc.tensor.matmul(ps[:], xl, wh, start=False, stop=False, perf_mode=DR)
                    nc.tensor.matmul(ps[:], xh, wl, start=False, stop=(j == ND2 - 1), perf_mode=DR)
                vt = vaug[tc_i * (TCH // P) + mt]
                nc.vector.tensor_copy(
                    vt[:].rearrange("p (h c) -> p h c", c=65)[:, :, 0:64],
                    ps[:].rearrange("p (h c) -> p h c", c=64),
                )

            def emit_proj_q(tc_i):
                for m in range(NM):
                    emit_proj_q_m(tc_i, m)

            def emit_proj_k(tc_i):
                for m in range(NM):
                    emit_proj_k_m(tc_i, m)

            def emit_proj_v(tc_i):
                for mt in range(TCH // P):
                    emit_proj_v_mt(tc_i, mt)
                del xq_tiles[tc_i], xkv_tiles[tc_i]

            def emit_outproj_nt(mt, nt):
                # fp8 DoubleRow 3-term: psum = 1024*y over ob-pairs jj
                ps = big_pool.tile([P, 512], F32, tag="big", name=f"y{mt}_{nt}")
                NJ = NM // 2
                for jj in range(NJ):
                    oh = oh8[jj][:, :, ts(mt, P)]
                    ol = ol8[jj][:, :, ts(mt, P)]
                    wh = wo_t[:, 4 * jj : 4 * jj + 2, ts(nt, 512)]
                    wl = wo_t[:, 4 * jj + 2 : 4 * jj + 4, ts(nt, 512)]
                    nc.tensor.matmul(ps[:], oh, wh, start=(jj == 0), stop=False, perf_mode=DR)
                    nc.tensor.matmul(ps[:], ol, wh, start=False, stop=False, perf_mode=DR)
                    nc.tensor.matmul(ps[:], oh, wl, start=False, stop=(jj == NJ - 1), perf_mode=DR)
                ysb = y_pool.tile([P, 512], F32, tag="ysb", name=f"ysb{mt}_{nt}")
                nc.vector.tensor_scalar_mul(ysb[:], ps[:], 1.0 / 1024.0)
                nc.sync.dma_start(y[ts(mt, P), ts(nt, 512)], ysb[:])

            def emit_score_kb(hp, qt, kb):
                j = kb - 4 * qt
                ce = max(j, 0) * P
                # both heads' scores in one 2-bank PSUM tile so a single
                # strided activation does both exps
                pss = spool.tile([P, 1024], F32, tag="s", name=f"s{hp}_{qt}_{kb}")
                w1 = 512 - ce
                for g, po in ((0, 0), (1, 64)):
                    # one fp8 DoubleRow matmul: (kh + kl) . q, q broadcast
                    qv = (
                        qT[hp][po : po + 64, qt * 512 + ce : (qt + 1) * 512]
                        .unsqueeze(1)
                        .broadcast_to([64, 2, w1])
                    )
                    nc.tensor.matmul(
                        pss[:, g * 512 + ce : (g + 1) * 512],
                        kT[hp][po : po + 64, :, ts(kb, P)],
                        qv,
                        start=True,
                        stop=True,
                        perf_mode=DR,
                    )
                pexp = apool.tile([P, 1024], BF16, tag=f"p{kb}", name=f"p{hp}_{qt}_{kb}")
                out_ap = pexp[:].rearrange("p (g c) -> p g c", g=2)[:, :, ce:]
                in_ap = pss[:].rearrange("p (g c) -> p g c", g=2)[:, :, ce:]
                if qt == NQT - 1:
                    # ACT is the serial consumer through the last q-tile and
                    # DVE is otherwise idle there: give DVE a dense share,
                    # densest in the final group (it gates the output tail)
                    dense = (kb % 2 == 1) if hp == HG // 2 - 1 else ((kb + hp) % 4 == 0)
                else:
                    dense = False
                if (qt >= 2 and (kb + hp) % SCH_MOD == 0) or dense:
                    # DVE fast-exp in the ACT-bound late q-tiles; the final
                    # group splits its exps ACT/DVE so the last one (which
                    # gates the whole output tail) lands ~5us earlier
                    nc.vector.tensor_scalar(
                        out_ap.bitcast(I16), in_ap, SCH_A * EXPSCALE, SCH_B, Mult, Add
                    )
                else:
                    nc.scalar.activation(out_ap, in_ap, Exp, scale=EXPSCALE)
                if j >= 0:
                    # mask the boundary block for both heads in one op
                    nc.gpsimd.tensor_tensor(
                        pexp[:].rearrange("p (g c) -> p g c", g=2)[:, :, ts(j, P)],
                        pexp[:].rearrange("p (g c) -> p g c", g=2)[:, :, ts(j, P)],
                        tri2[:].rearrange("p (g c) -> p g c", g=2),
                        Mult,
                    )
                return pexp



            def make_pv_unit(hp, qt, qb, pexps, osb, otq):
                def fn():
                    # both heads' PV accumulation groups, sequentially, into
                    # one [128, 130] PSUM tile (cols h*65+64 = denominators);
                    # each group runs start-to-stop before the next opens
                    # (2KB PSUM zero-region rule).
                    pv = opool.tile([P, 130], F32, tag="pv", name=f"pv{hp}_{qt}_{qb}")
                    for g, hh in ((0, 2 * hp), (1, 2 * hp + 1)):
                        for kb in range(4 * qt + qb + 1):
                            nc.tensor.matmul(
                                pv[:, g * 65 : g * 65 + 65],
                                pexps[kb][:, g * 512 + qb * P : g * 512 + (qb + 1) * P],
                                vaug[kb][:, hh * 65 : hh * 65 + 65],
                                start=(kb == 0),
                                stop=(kb == 4 * qt + qb),
                            )
                    rec = apool2.tile([P, 2], F32, tag="rec", name=f"rec{hp}_{qt}_{qb}")
                    nc.vector.reciprocal(
                        rec[:].rearrange("p (g c) -> p g c", c=1),
                        pv[:].rearrange("p (g c) -> p g c", c=65)[:, :, 64:65],
                    )
                    for g in (0, 1):
                        # pv holds sum(p * 1024*v); rescale by 1/1024 here
                        nc.vector.tensor_scalar(
                            osb[:, g * 64 : (g + 1) * 64],
                            pv[:, g * 65 : g * 65 + 64],
                            rec[:, g : g + 1],
                            1.0 / 1024.0,
                            Mult,
                            Mult,
                        )
                    # one DMA-transpose moves both heads' normalized O[q, dh]
                    # block into O^T[dh, q] inside the per-(hp,qt) staging
                    # tile; after the last block, DVE splits the 512-wide
                    # strip into the fp8 hi/lo pair the outproj consumes
                    nc.sync.dma_start_transpose(otq[:, ts(qb, P)], osb[:])
                    if qt == NQT - 1:
                        # last q-tile: split per block so outproj token
                        # blocks unlock as early as possible (short tail)
                        blk = slice((4 * qt + qb) * P, (4 * qt + qb + 1) * P)
                        ohs = oh8[hp // 2][:, hp % 2, blk]
                        nc.vector.tensor_scalar_mul(ohs, otq[:, ts(qb, P)], 32.0)
                        nc.vector.scalar_tensor_tensor(
                            ol8[hp // 2][:, hp % 2, blk],
                            otq[:, ts(qb, P)],
                            32.0,
                            ohs,
                            Mult,
                            mybir.AluOpType.subtract,
                        )
                        attn_cnt[4 * qt + qb] += 1
                    elif qb == 3:
                        blk = slice(qt * 512, (qt + 1) * 512)
                        ohs = oh8[hp // 2][:, hp % 2, blk]
                        nc.vector.tensor_scalar_mul(ohs, otq[:], 32.0)
                        nc.vector.scalar_tensor_tensor(
                            ol8[hp // 2][:, hp % 2, blk],
                            otq[:],
                            32.0,
                            ohs,
                            Mult,
                            mybir.AluOpType.subtract,
                        )
                        for i in range(4):
                            attn_cnt[4 * qt + i] += 1

                return (2 * (4 * qt + qb + 1) * 65 * 0.42 + 120, fn)

            # ---------------- orchestration ----------------
            # Fine-grained pull model: scores (hp, qt, kb) pull exactly
            # Qproj(qt, hp) and Kproj(kb//4, hp) just in time, so exp work
            # flows to ACT as early as the data allows.  Remaining proj
            # units, PV units and outproj blocks are debt-paced PE filler.
            NVT = TCH // P  # V-proj token-blocks per chunk
            q_done = [[False] * NM for _ in range(NCH)]
            k_done = [[False] * NM for _ in range(NCH)]
            v_done = [[False] * NVT for _ in range(NCH)]

            def chunk_all_done(c):
                return all(q_done[c]) and all(k_done[c]) and all(v_done[c])

            def ensure_x(c):
                if c not in x_dmas_done:
                    # tile-slot hazard: chunk c's x DMA reuses chunk c-2's
                    # buffers; all chunk c-2 readers must be emitted first
                    if c >= 2 and not chunk_all_done(c - 2):
                        pull_chunk(c - 2)
                    emit_x_dma(c)

            def pull_q(c, m):
                if not q_done[c][m]:
                    q_done[c][m] = True
                    ensure_x(c)
                    emit_proj_q_m(c, m)
                    return 1280
                return 0

            def pull_k(c, m):
                if not k_done[c][m]:
                    k_done[c][m] = True
                    ensure_x(c)
                    emit_proj_k_m(c, m)
                    return 1280
                return 0

            def pull_v(c, mt):
                if not v_done[c][mt]:
                    v_done[c][mt] = True
                    ensure_x(c)
                    emit_proj_v_mt(c, mt)
                    return 1280
                return 0

            def pull_chunk(c):
                for m in range(NM):
                    pull_q(c, m)
                for m in range(NM):
                    pull_k(c, m)
                for mt in range(NVT):
                    pull_v(c, mt)

            # workq: debt-paced PE filler (cost_ns, fn); fn returns actual
            # cost (0 if the unit was already pulled directly)
            workq = []
            ogate = {}
            attn_cnt = [0] * NTB  # per token block: heads with split done

            def queue_fillers():
                for c in range(NCH):
                    for m in range(NM):
                        workq.append((f"c{c}", 1280, lambda c=c, m=m: pull_q(c, m)))
                    for m in range(NM):
                        workq.append((f"c{c}", 1280, lambda c=c, m=m: pull_k(c, m)))
                    for mt in range(NVT):
                        workq.append((f"c{c}", 1280, lambda c=c, mt=mt: pull_v(c, mt)))
                for mt in range(NTB):
                    for nt in range(D // 512):
                        def fo(mt=mt, nt=nt):
                            emit_outproj_nt(mt, nt)
                            return 853

                        ogate[id(fo)] = mt
                        workq.append(("o", 640, fo))

            def pop_work(budget_ns):
                spent = 0.0
                i = 0
                while i < len(workq) and spent < budget_ns:
                    kind, cost, fn = workq[i]
                    if kind == "o" and attn_cnt[ogate[id(fn)]] < HG // 2:
                        i += 1
                        continue
                    r = fn()
                    spent += cost if r is None else r
                    workq.pop(i)
                return spent

            # critical-path DMAs first: wq, x chunk 0, wk feed the first
            # scores; wv/wo and chunk 1 follow
            nc.sync.dma_start(wq_t[:, 0:4, :], wq8[:, 0:4, :])
            emit_x_dma(0)
            emit_w_dmas()
            emit_w_dmas_late()
            emit_x_dma(1)
            queue_fillers()
            # PE prewarm: dummy matmuls on the tri constant ramp the tensor
            # engine to full clock while the first weight/x DMAs land
            pwt = big_pool.tile([P, 512], F32, tag="big", name="prewarm")
            for _ in range(44):
                nc.tensor.matmul(
                    pwt[:, 0:P], tri2[:, 0:P], tri2[:, 0:P], start=True, stop=True
                )

            # debt-paced weave: pop a PE filler unit only once the consumer
            # engine's exp backlog exceeds its cost, so the PE stays just
            # behind ACT/DVE.  PV units are emitted inline, one k-block
            # behind the score stream, so nothing drains at the end.
            debt = 0.0
            deferred_pv = []
            groups = [(qt, hp) for qt in range(NQT) for hp in range(HG // 2)]
            for gi, (qt, hp) in enumerate(groups):
                if hp == 0 and qt + 1 < NCH:
                    ensure_x(qt + 1)
                pull_q(qt, hp)
                nkb = 4 * qt + 4
                pexps = []
                osbs = [
                    opool_sb.tile([P, P], BF16, tag=f"osb{qb}", name=f"osb{hp}_{qt}_{qb}")
                    for qb in range(4)
                ]
                otq = ot_pool.tile([P, 512], BF16, tag=f"ot{hp}", name=f"ot{hp}_{qt}")

                def emit_pv(qb, qt=qt, hp=hp, pexps=pexps, osbs=osbs, otq=otq):
                    # vaug writes must be emitted before the PV reads them
                    for j in range(4 * qt + qb + 1):
                        pull_v(j // 4, j % 4)
                    cost, fn = make_pv_unit(hp, qt, qb, pexps, osbs[qb], otq)
                    fn()
                    return cost

                last = gi == len(groups) - 1
                for kb in range(nkb):
                    pull_k(kb // 4, hp)
                    pexps.append(emit_score_kb(hp, qt, kb))
                    if kb == 1 and not last:
                        # prefetch the next group's Q/K so its first score
                        # fires the moment this group's exps are drained
                        nqt, nhp = groups[gi + 1]
                        pull_q(nqt, nhp)
                        pull_k(0, nhp)
                    if kb % 4 == 2 and kb + 2 < nkb:
                        # prefetch the next k-chunk's K projection so its
                        # DVE eviction lands before the scores need it
                        pull_k((kb + 2) // 4, hp)
                    dpv_slot = (4, 5) if nkb > 8 else ((2, 3) if nkb > 4 else (0, 1))
                    if kb in dpv_slot and deferred_pv:
                        # previous group's trailing PV blocks: their exps are
                        # long done, so no PE stall and no ACT gap
                        debt -= deferred_pv.pop(0)()
                    w = 2 * (512 - max(kb - 4 * qt, 0) * P)
                    if qt >= 2 and (kb + hp) % SCH_MOD == 0:
                        debt += (w * 1.04 + 170) - (w * 0.21 + 10)
                    else:
                        debt += (w * 0.833 + 242) - (w * 0.21 + 10)
                    if last:
                        continue  # emit the final scores back-to-back
                    qb = kb - 4 * qt - 2
                    if 0 <= qb <= 1:
                        debt -= emit_pv(qb)
                    npops = 0
                    cap = 3
                    while workq and npops < cap:
                        # prefer proj units; spend outproj units only when
                        # nothing else is ready (saves them for the
                        # filler-starved late q-tiles)
                        pick = None
                        for i, (kind, cost, fn) in enumerate(workq):
                            if kind == "o":
                                continue
                            pick = i
                            break
                        if pick is None:
                            for i, (kind, cost, fn) in enumerate(workq):
                                if kind == "o" and attn_cnt[ogate[id(fn)]] >= HG // 2:
                                    pick = i
                                    break
                        if pick is None or (
                            workq[pick][1] > debt
                            and not (last and workq[pick][0] == "o")
                        ):
                            break
                        kind, cost, fn = workq.pop(pick)
                        r = fn()
                        debt -= r if r is not None else cost
                        npops += 1
                if not last:
                    deferred_pv.append(lambda e=emit_pv: e(2))
                    deferred_pv.append(lambda e=emit_pv: e(3))
                else:
                    # tail: PV per block, then its outproj immediately
                    for qb in range(4):
                        emit_pv(qb)
                        pop_work(1 << 30)
            while workq:
                pop_work(1 << 30)
    nc.finalize()
    return nc


_NC_CACHE = {}


def _get_nc():
    if "full" not in _NC_CACHE:
        _NC_CACHE["full"] = build_bass()
    return _NC_CACHE["full"]


def _pack_hilo(mT):
    """[D, C] fp32 -> [128, 4*ND2, C] fp8 hi/lo pack at scale 32.

    sub-index s = 4*j + 2*hl + i: (d-pair j, hi/lo, block i); value
    hi = e4m3(32*x), lo = e4m3(32*x - hi).
    """
    import ml_dtypes

    e4 = ml_dtypes.float8_e4m3
    D, C = mT.shape
    nd = D // P
    nd2 = nd // 2
    blocks = mT.reshape(nd2, 2, P, C)  # [j, i, p, c]
    hi = (32.0 * blocks).astype(e4)
    lo = (32.0 * blocks - hi.astype(np.float32)).astype(e4)
    out = np.empty((P, 4 * nd2, C), dtype=e4)
    for j in range(nd2):
        for i in range(2):
            out[:, 4 * j + i, :] = hi[j, i]
            out[:, 4 * j + 2 + i, :] = lo[j, i]
    return out


def make_in_maps(query, key_value, Wq, Wk, Wv, Wo):
    import ml_dtypes

    query = np.asarray(query, dtype=np.float32)
    key_value = np.asarray(key_value, dtype=np.float32)
    Wq, Wk, Wv, Wo = (np.asarray(w, dtype=np.float32) for w in (Wq, Wk, Wv, Wo))
    GO = Wq.shape[0] // 2
    bf = ml_dtypes.bfloat16
    xq8_b = [_pack_hilo(np.ascontiguousarray(query[b].T)) for b in range(B_FULL)]
    xkv8_b = [_pack_hilo(np.ascontiguousarray(key_value[b].T)) for b in range(B_FULL)]
    w8 = {}
    for g in range(2):
        sl = slice(g * GO, (g + 1) * GO)
        w8[g] = (
            _pack_hilo(np.ascontiguousarray(Wq[sl, :].T)),
            _pack_hilo(np.ascontiguousarray(Wk[sl, :].T)),
            _pack_hilo(np.ascontiguousarray(Wv[sl, :].T)),
            _pack_hilo(np.ascontiguousarray(Wo[:, sl].T)),
        )
    in_maps = []
    for c in range(N_CORES):
        b, g = c // 2, c % 2
        sl = slice(g * GO, (g + 1) * GO)
        in_maps.append(
            {
                "xq8": xq8_b[b],
                "xkv8": xkv8_b[b],
                "wq8": w8[g][0],
                "wk8": w8[g][1],
                "wv8": w8[g][2],
                "wo8": w8[g][3],
            }
        )
    return in_maps


def kernel(query, key_value, Wq, Wk, Wv, Wo):
    from concourse import bass_utils

    nc = _get_nc()
    in_maps = make_in_maps(query, key_value, Wq, Wk, Wv, Wo)
    res = bass_utils.run_bass_kernel_spmd(nc, in_maps, core_ids=list(range(N_CORES)))
    ys = [r["y"] for r in res.results]
    out = np.stack([ys[2 * b] + ys[2 * b + 1] for b in range(B_FULL)])
    return out.astype(np.float32)

